# revision 16
# baseline (speedup 1.0000x reference)
"""2-layer GAT (nn_GAT_31490700214331) on 8 Trainium2 NeuronCores.

Strategy (dst-sharded, SPMD, per-core-rotated node layout) — v2:
  - Nodes block-partitioned: core c owns nodes [c*6250, (c+1)*6250), every
    table uses the rotated row order (n - c*6250) mod 50000 so the SPMD
    program has no core-dependent offsets.
  - Layer-0 node features (h0|as0|ad0 = x @ W0cat) are computed replicated
    into a rotated DRAM table with 512B rows; layer-1 rows (h1|as1|ad1)
    are 1280B with 1056B content.  Edges are grouped by 128-dst tiles and
    source rows fetched with dma_gather through lo/hi table views (int16
    indices), batched over groups of tiles to amortize SWDGE overhead.
  - alpha_dst values are NOT dma-gathered per edge (256B/slot minimum).
    Instead each core keeps a transposed local table adT[16, 6272] (f32,
    built with PE transposes during phases A/D) and the per-slot lookup
    runs on gpsimd via ap_gather (indices in epack), then 16-wide PE
    transposes bring the result back slot-major through PSUM.
  - Edge softmax (safe without segment-max: |e| small) and the weighted
    aggregation are fused into per-chunk 128x128 incidence matmuls; the
    exp(e) values ride as 8 extra columns of the moving operand so a
    single PSUM accumulation produces numerator and denominator.
  - Between layers the ELU'd hidden state is AllGather'd (feature-major)
    and rotated into per-core order with partition-id-offset DMA reads.
  - alpha projections fold into the weight matmuls on the host:
    W0cat=[256,144] gives h0|as0|ad0; W1cat=[128,528] gives h1|as1|ad1.

Self-contained: call kernel(**inputs) with the full-problem arrays.
"""
import numpy as np
from contextlib import ExitStack

import concourse.bacc as bacc
import concourse.bass as bass
import concourse.mybir as mybir
from concourse.tile import TileContext
from concourse.bass_utils import run_bass_kernel_spmd

F16 = mybir.dt.float16
F32 = mybir.dt.float32
I16 = mybir.dt.int16

N = 50000
NFEAT = 256
NHID = 128
NCLASS = 64
HEADS = 8
SLOPE = 0.2
NCORES = 8
NLOC = N // NCORES           # 6250
LT = (NLOC + 127) // 128     # 49 local dst tiles
LAST_ROWS = NLOC - (LT - 1) * 128   # 106 rows in the last tile
GT = 392                     # global node tiles (392*128 = 50176)
GROWS = GT * 128
SPLIT = 25088                # low/high gather-table split (196 tiles)
SENT = 300.0                 # dst_rel sentinel for padding slots
T0W = 256                    # t0 row: [h0(128)|as0(8)|ad0(8)|junk] f16
T1W = 640                    # t1 row: [h1(512)|as1(8)|ad1(8)|junk] f16

GROUPS0 = [(7 * i, 7) for i in range(7)]                     # L0 tile groups
GROUPS1 = [(2 * i, 2) for i in range(24)] + [(48, 1)]        # L1 tile groups
HALF = 3200                  # adT split: dsts [0,3200) core0, rest core1
ADTE = HALF + 1              # num_elems per half (last col = zero slot)

_cache = {}


# --------------------------------------------------------------------------
# host-side preparation
# --------------------------------------------------------------------------

def _wrap_idx(idx):
    """[n] int -> [128, n//16] int16 wrapped gather-index layout."""
    n = idx.shape[0]
    assert n % 16 == 0
    w = idx.reshape(n // 16, 16).T.astype(np.int16)
    return np.tile(w, (8, 1))


def _wrap16(idx):
    """[n] int -> [16, n//16] int16 (single gpsimd core)."""
    n = idx.shape[0]
    assert n % 16 == 0
    return idx.reshape(n // 16, 16).T.astype(np.int16)


def _epw(gsz, NL, NH):
    CH = NL + NH
    w = gsz * (NL * 8 + NH * 8 + CH + CH * 8)
    return w, ((w + 15) // 16) * 16


def _prep_edges(src, dst):
    cores = []
    for c in range(NCORES):
        m = (dst >= c * NLOC) & (dst < (c + 1) * NLOC)
        s = src[m].astype(np.int64)
        d = dst[m].astype(np.int64) - c * NLOC
        order = np.argsort(d, kind="stable")
        s, d = s[order], d[order]
        s_rot = (s - c * NLOC) % N
        tiles = []
        for t in range(LT):
            sel = (d >= t * 128) & (d < (t + 1) * 128)
            st, dt = s_rot[sel], d[sel] - t * 128
            lo = st < SPLIT
            tiles.append((st[lo], dt[lo], st[~lo] - SPLIT, dt[~lo]))
        cores.append(tiles)
    nl = max(len(t[0]) for tl in cores for t in tl)
    nh = max(len(t[2]) for tl in cores for t in tl)
    NL = max(1, (nl + 127) // 128)
    NH = max(1, (nh + 127) // 128)
    assert NL * 128 <= 1024 and NH * 128 <= 1024, (NL, NH)
    CH = NL + NH

    out = []
    for c in range(NCORES):
        tiles = []
        for t in range(LT):
            sl, dl, sh, dh = cores[c][t]
            il = np.zeros(NL * 128, np.int64)
            il[: len(sl)] = sl
            ih = np.zeros(NH * 128, np.int64)
            ih[: len(sh)] = sh
            axg = np.zeros(CH * 128, np.int64)
            axg[: len(dl)] = t * 128 + dl
            axg[NL * 128: NL * 128 + len(dh)] = t * 128 + dh
            ax0 = np.where(axg < HALF, axg, HALF)          # core-0 indices
            ax1 = np.where(axg >= HALF, axg - HALF, HALF)  # core-1 indices
            rl = np.full(NL * 128, SENT)
            rl[: len(dl)] = dl
            rh = np.full(NH * 128, SENT)
            rh[: len(dh)] = dh
            dr = np.concatenate([rl, rh]).reshape(CH, 128).T  # [128, CH]
            tiles.append((il, ih, ax0, ax1, dr.astype(np.float16)))

        def pack(groups):
            _, epwp = _epw(groups[0][1], NL, NH)
            blks = np.zeros((len(groups), 128, epwp), np.int16)
            for gi, (t0, gsz) in enumerate(groups):
                o_ih = gsz * NL * 8
                o_dr = gsz * (NL + NH) * 8
                o_ax = o_dr + gsz * CH
                il = np.concatenate([tiles[t][0] for t in range(t0, t0 + gsz)])
                ih = np.concatenate([tiles[t][1] for t in range(t0, t0 + gsz)])
                ax0 = np.concatenate(
                    [tiles[t][2] for t in range(t0, t0 + gsz)])
                ax1 = np.concatenate(
                    [tiles[t][3] for t in range(t0, t0 + gsz)])
                dr = np.concatenate([tiles[t][4] for t in range(t0, t0 + gsz)],
                                    axis=1)
                blks[gi][:, 0:o_ih] = _wrap_idx(il)
                blks[gi][:, o_ih:o_dr] = _wrap_idx(ih)
                blks[gi][:, o_dr:o_ax] = dr.view(np.int16)
                blks[gi][0:16, o_ax:o_ax + gsz * CH * 8] = _wrap16(ax0)
                blks[gi][32:48, o_ax:o_ax + gsz * CH * 8] = _wrap16(ax1)
            return blks

        out.append(dict(ep0=np.ascontiguousarray(pack(GROUPS0)),
                        ep1=np.ascontiguousarray(pack(GROUPS1))))
    return NL, NH, out


def _prep_inputs(x, edge_index, W0, a_src0, a_dst0, b0, W1, a_src1, a_dst1,
                 b1):
    src = np.asarray(edge_index[0]).astype(np.int64)
    dst = np.asarray(edge_index[1]).astype(np.int64)
    NL, NH, edata = _prep_edges(src, dst)

    def bd(a):  # [H, D] -> blockdiag [H*D, H]
        a = np.asarray(a, np.float32)
        H, D = a.shape
        m = np.zeros((H * D, H), np.float32)
        for h in range(H):
            m[h * D:(h + 1) * D, h] = a[h]
        return m

    W0 = np.asarray(W0, np.float32)
    W1 = np.asarray(W1, np.float32)
    W0a = np.concatenate([W0 @ bd(a_src0), W0 @ bd(a_dst0)], 1)  # [256, 16]
    # head-innermost feature interleave: new col d*8+h <- old col h*D+d
    perm0 = np.array([(f % 8) * 16 + f // 8 for f in range(128)])
    perm1 = np.array([(f % 8) * 64 + f // 8 for f in range(512)])
    W0cat = np.concatenate([W0[:, perm0], W0a], 1)               # [256, 144]
    W1a = np.concatenate([W1 @ bd(a_src1), W1 @ bd(a_dst1)], 1)  # [128, 16]
    W1cat = np.concatenate([W1[perm0][:, perm1], W1a[perm0]], 1)  # [128, 528]

    x = np.asarray(x, np.float32)
    ident = np.eye(128, dtype=np.float16)
    colio = np.tile(np.arange(128, dtype=np.float16)[None, :], (128, 1))
    b0b = np.tile(np.asarray(b0, np.float32)[None, :], (128, 1))
    b1b = np.tile(np.asarray(b1, np.float32)[None, :], (128, 1))

    in_maps = []
    for c in range(NCORES):
        rot = np.roll(np.arange(N), -c * NLOC)
        xr = np.zeros((GROWS, NFEAT), np.float16)
        xr[:N] = x[rot].astype(np.float16)
        xtt = xr.reshape(GROWS // 128, 128, 2, 128).transpose(0, 3, 2, 1)
        m = dict(
            xT=np.ascontiguousarray(xtt),
            W0=np.ascontiguousarray(
                W0cat.astype(np.float16).reshape(2, 128, NHID + 16)),
            W1cat=np.ascontiguousarray(W1cat.astype(np.float16)),
            b0b=np.ascontiguousarray(b0b[:, perm0]), b1b=b1b,
            ident=ident, colio=colio,
            **edata[c],
        )
        in_maps.append(m)
    return NL, NH, in_maps


# --------------------------------------------------------------------------
# device program
# --------------------------------------------------------------------------

def build(NL, NH, lt=LT, gt=GT, debug=False, phases="ABCDE"):
    CH = NL + NH
    HID16 = NHID + 16

    _, EPW0P = _epw(GROUPS0[0][1], NL, NH)
    _, EPW1P = _epw(GROUPS1[0][1], NL, NH)
    nc = bacc.Bacc("TRN2")
    xT = nc.dram_tensor("xT", [GROWS // 128, 128, 2, 128], F16,
                        kind="ExternalInput")
    W0i = nc.dram_tensor("W0", [2, 128, HID16], F16, kind="ExternalInput")
    W1cati = nc.dram_tensor("W1cat", [NHID, 528], F16, kind="ExternalInput")
    b0bi = nc.dram_tensor("b0b", [128, NHID], F32, kind="ExternalInput")
    b1bi = nc.dram_tensor("b1b", [128, NCLASS], F32, kind="ExternalInput")
    identi = nc.dram_tensor("ident", [128, 128], F16, kind="ExternalInput")
    colioi = nc.dram_tensor("colio", [128, 128], F16, kind="ExternalInput")
    ep0i = nc.dram_tensor("ep0", [len(GROUPS0), 128, EPW0P], I16,
                          kind="ExternalInput")
    ep1i = nc.dram_tensor("ep1", [len(GROUPS1), 128, EPW1P], I16,
                          kind="ExternalInput")
    out = nc.dram_tensor("out", [NLOC, NCLASS], F32, kind="ExternalOutput")

    with TileContext(nc) as tc, ExitStack() as stk:
        regs = {}
        for _, g in GROUPS0 + GROUPS1:
            for n in (g * NL * 128, g * NH * 128):
                if n not in regs:
                    regs[n] = nc.gpsimd.to_reg(n)
        dpool = stk.enter_context(
            tc.tile_pool(name="dram", bufs=1, space="DRAM"))
        t0lo = dpool.tile([SPLIT, T0W], F16, tag="t0lo")
        t0hi = dpool.tile([GROWS - SPLIT, T0W], F16, tag="t0hi")
        t1lo = dpool.tile([SPLIT, T1W], F16, tag="t1lo")
        t1hi = dpool.tile([GROWS - SPLIT, T1W], F16, tag="t1hi")
        agin = dpool.tile([128, NLOC], F16, tag="agin")
        agout = dpool.tile([NCORES * 128, NLOC], F16, tag="agout",
                           addr_space="Shared")

        cpool = stk.enter_context(tc.tile_pool(name="const", bufs=1))
        W0s = cpool.tile([128, 2, HID16], F16)
        nc.sync.dma_start(out=W0s[:], in_=W0i.rearrange("k p n -> p k n"))
        W1s = cpool.tile([128, 528], F16)
        nc.sync.dma_start(out=W1s[:], in_=W1cati[:])
        b0s = cpool.tile([128, NHID], F32)
        nc.sync.dma_start(out=b0s[:], in_=b0bi[:])
        b1s = cpool.tile([128, NCLASS], F32)
        nc.sync.dma_start(out=b1s[:], in_=b1bi[:])
        idents = cpool.tile([128, 128], F16)
        nc.sync.dma_start(out=idents[:], in_=identi[:])
        colios = cpool.tile([128, 128], F16)
        nc.sync.dma_start(out=colios[:], in_=colioi[:])
        ident64f = cpool.tile([64, 64], F32)
        nc.vector.tensor_copy(ident64f[:], idents[0:64, 0:64])
        adT0 = cpool.tile([64, ADTE], F32)
        nc.vector.memset(adT0[:], 0)
        adT1 = cpool.tile([64, ADTE], F32)
        nc.vector.memset(adT1[:], 0)

        def adt_store(adT, pt, g):
            hf = 1 if g * 128 >= HALF else 0
            nc.vector.tensor_copy(
                adT[32 * hf:32 * hf + 8,
                    g * 128 - HALF * hf:(g + 1) * 128 - HALF * hf], pt[:])

        # ---------------- phase A: layer-0 tables (replicated) ------------
        if "A" in phases:
            with ExitStack() as pa:
                xp = pa.enter_context(tc.tile_pool(name="pa_x", bufs=4))
                pp = pa.enter_context(
                    tc.tile_pool(name="pa_ps", bufs=2, space="PSUM"))
                rp = pa.enter_context(tc.tile_pool(name="pa_row", bufs=4))
                assert gt % 2 == 0
                for gg in range(gt // 2):
                    xa = xp.tile([128, 2, 2, 128], F16, tag="xa")
                    nc.sync.dma_start(
                        out=xa[:],
                        in_=xT[2 * gg:2 * gg + 2].rearrange(
                            "g p k j -> p g k j"))
                    row = rp.tile([128, 2, T0W], F16, tag="row")
                    for g2 in range(2):
                        ps = pp.tile([128, HID16], F32, tag=f"ps{g2}")
                        for k in range(2):
                            nc.tensor.matmul(ps[:], xa[:, g2, k, :],
                                             W0s[:, k, :],
                                             start=(k == 0), stop=(k == 1))
                        nc.scalar.copy(row[:, g2, 0:HID16], ps[:])
                    eng = nc.scalar if gg % 2 else nc.sync
                    g0 = 2 * gg * 128
                    if g0 + 256 <= SPLIT:
                        eng.dma_start(
                            out=t0lo[g0:g0 + 256, :]
                            .rearrange("(g p) w -> p g w", p=128),
                            in_=row[:])
                    else:
                        o = g0 - SPLIT
                        eng.dma_start(
                            out=t0hi[o:o + 256, :]
                            .rearrange("(g p) w -> p g w", p=128),
                            in_=row[:])
                    for g2 in range(2):
                        g = 2 * gg + g2
                        if g < lt:
                            pt = pp.tile([8, 128], F16, tag="pt")
                            nc.tensor.transpose(
                                pt[:], row[:, g2, NHID + 8:HID16],
                                idents[:])
                            adt_store(adT0, pt, g)

        # ---------------- shared edge phase -------------------------------
        def edge_phase(layer, tbl_lo, tbl_hi, adT, fdim, trow, groups, epi,
                       post_fn, fin):
            fd8 = fdim + 8
            D = fdim // HEADS
            with ExitStack() as pb:
                ip = pb.enter_context(
                    tc.tile_pool(name=f"ix{layer}", bufs=2))
                gp = pb.enter_context(
                    tc.tile_pool(name=f"gg{layer}", bufs=2))
                apd = pb.enter_context(
                    tc.tile_pool(name=f"ga{layer}", bufs=2))
                rp2 = pb.enter_context(
                    tc.tile_pool(name=f"rh{layer}", bufs=3))
                pp2 = pb.enter_context(
                    tc.tile_pool(name=f"ps{layer}", bufs=2, space="PSUM"))
                op = pb.enter_context(
                    tc.tile_pool(name=f"po{layer}", bufs=3))
                for gi, (t0g, gsz) in enumerate(groups):
                    o_ih = gsz * NL * 8
                    o_dr = gsz * (NL + NH) * 8
                    o_ax = o_dr + gsz * CH
                    nli, nhi = gsz * NL * 128, gsz * NH * 128
                    ep = ip.tile([128, epi.shape[2]], I16, tag="ep")
                    nc.sync.dma_start(out=ep[:], in_=epi[gi])
                    Glo = gp.tile([128, gsz * NL, trow], F16, tag="Glo")
                    nc.gpsimd.dma_gather(Glo[:], tbl_lo[:],
                                         ep[:, 0:o_ih], nli, regs[nli],
                                         trow)
                    Ghi = gp.tile([128, gsz * NH, trow], F16, tag="Ghi")
                    nc.gpsimd.dma_gather(Ghi[:], tbl_hi[:],
                                         ep[:, o_ih:o_dr], nhi, regs[nhi],
                                         trow)
                    for tl in range(gsz):
                        t = t0g + tl
                        admT = apd.tile([64, CH * 128], F32, tag="admT")
                        nc.gpsimd.ap_gather(
                            admT[:].unsqueeze(-1), adT[:].unsqueeze(-1),
                            ep[0:64, o_ax + tl * CH * 8:
                               o_ax + (tl + 1) * CH * 8],
                            channels=64, num_elems=ADTE, d=1,
                            num_idxs=CH * 128)
                        dr = ep[:, o_dr + tl * CH:
                                o_dr + (tl + 1) * CH].bitcast(F16)
                        admP = pp2.tile([128, CH * 64], F32, tag="admP")
                        for mm in range(CH):
                            nc.tensor.transpose(
                                admP[:, mm * 64:(mm + 1) * 64],
                                admT[:, mm * 128:(mm + 1) * 128],
                                ident64f[:])
                        adm = apd.tile([128, CH, 8], F16, tag="adm")
                        admPv = admP[:].rearrange("p (c w) -> p c w", w=64)
                        nc.vector.tensor_tensor(
                            out=adm[:], in0=admPv[:, :, 0:8],
                            in1=admPv[:, :, 32:40], op=mybir.AluOpType.add)

                        glo = Glo[:, tl * NL:(tl + 1) * NL, :]
                        ghi = Ghi[:, tl * NH:(tl + 1) * NH, :]
                        inc = rp2.tile([128, CH, 128], F16, tag="inc")
                        nc.vector.tensor_tensor(
                            out=inc[:],
                            in0=dr.unsqueeze(-1)
                            .broadcast_to([128, CH, 128]),
                            in1=colios[:].unsqueeze(1)
                            .broadcast_to([128, CH, 128]),
                            op=mybir.AluOpType.is_equal)
                        EX = op.tile([128, CH, 8], F16, tag="EX")
                        nc.vector.tensor_tensor(
                            out=EX[:, 0:NL, :],
                            in0=glo[:, :, fdim:fd8],
                            in1=adm[:, 0:NL, :], op=mybir.AluOpType.add)
                        nc.vector.tensor_tensor(
                            out=EX[:, NL:CH, :],
                            in0=ghi[:, :, fdim:fd8],
                            in1=adm[:, NL:CH, :], op=mybir.AluOpType.add)
                        nc.scalar.activation(
                            EX[:], EX[:],
                            mybir.ActivationFunctionType.Prelu, alpha=SLOPE)
                        nc.scalar.activation(
                            EX[:], EX[:], mybir.ActivationFunctionType.Exp)

                        R2 = rp2.tile([128, CH, fd8], F16, tag="R2")
                        nc.vector.tensor_copy(R2[:, :, fdim:fd8], EX[:])
                        nc.vector.tensor_tensor(
                            out=R2[:, 0:NL, 0:fdim]
                            .rearrange("p c (d h) -> p c d h", h=HEADS),
                            in0=glo[:, :, 0:fdim]
                            .rearrange("p c (d h) -> p c d h", h=HEADS),
                            in1=EX[:, 0:NL, :].unsqueeze(2)
                            .broadcast_to([128, NL, D, HEADS]),
                            op=mybir.AluOpType.mult)
                        nc.vector.tensor_tensor(
                            out=R2[:, NL:CH, 0:fdim]
                            .rearrange("p c (d h) -> p c d h", h=HEADS),
                            in0=ghi[:, :, 0:fdim]
                            .rearrange("p c (d h) -> p c d h", h=HEADS),
                            in1=EX[:, NL:CH, :].unsqueeze(2)
                            .broadcast_to([128, NH, D, HEADS]),
                            op=mybir.AluOpType.mult)

                        P1 = pp2.tile([128, fd8], F32, tag="P1")
                        for ch in range(CH):
                            nc.tensor.matmul(P1[:], inc[:, ch, :],
                                             R2[:, ch, :],
                                             start=(ch == 0),
                                             stop=(ch == CH - 1))
                        post_fn(t, P1, op, pp2, fin)

        # ---- L0 post: softmax-div, +b0, ELU, transpose, store ------------
        def post0(t, P1, op, pp2, fin):
            rows = 128 if t < lt - 1 else LAST_ROWS
            r8 = op.tile([128, 8], F32, tag="r8")
            nc.vector.tensor_scalar_add(r8[:], P1[:, NHID:NHID + 8], 1e-16)
            nc.vector.reciprocal(r8[:], r8[:])
            z = op.tile([128, NHID], F32, tag="z")
            nc.vector.tensor_tensor(
                out=z[:].rearrange("p (d h) -> p d h", h=HEADS),
                in0=P1[:, 0:NHID].rearrange("p (d h) -> p d h", h=HEADS),
                in1=r8[:].unsqueeze(1).broadcast_to([128, 16, HEADS]),
                op=mybir.AluOpType.mult)
            nc.vector.tensor_tensor(out=z[:], in0=z[:], in1=b0s[:],
                                    op=mybir.AluOpType.add)
            zm = op.tile([128, NHID], F32, tag="zm")
            nc.vector.tensor_scalar_min(zm[:], z[:], 0.0)
            nc.scalar.activation(zm[:], zm[:],
                                 mybir.ActivationFunctionType.Exp)
            zp = op.tile([128, NHID], F32, tag="zp")
            nc.vector.tensor_scalar_max(zp[:], z[:], 0.0)
            nc.vector.tensor_tensor(out=zp[:], in0=zp[:], in1=zm[:],
                                    op=mybir.AluOpType.add)
            h1 = op.tile([128, NHID], F16, tag="h1")
            nc.vector.tensor_scalar_add(h1[:], zp[:], -1.0)
            pst = pp2.tile([128, 128], F16, tag="pst")
            nc.tensor.transpose(pst[:], h1[:], idents[:])
            hT = op.tile([128, 128], F16, tag="hT")
            nc.vector.tensor_copy(hT[:], pst[:])
            nc.sync.dma_start(
                out=agin[:, t * 128:t * 128 + rows], in_=hT[:, 0:rows])

        if "B" in phases:
            edge_phase(0, t0lo, t0hi, adT0, NHID, T0W, GROUPS0, ep0i,
                       post0, None)

        # ---------------- phase C: AllGather + rotation -------------------
        sregs = None
        if "C" in phases:
            nc.gpsimd.collective_compute(
                "AllGather", mybir.AluOpType.bypass,
                replica_groups=[list(range(NCORES))],
                ins=[agin[:]], outs=[agout[:]])
            pid = nc.partition_id(engines=[mybir.EngineType.SP])
            sregs = [nc.sync.snap(((j + pid) % NCORES) * 128)
                     for j in range(NCORES)]

        # ---------------- phase D: layer-1 tables -------------------------
        if "D" in phases and sregs is not None:
            with ExitStack() as pd:
                ngt = min(gt, (N + 127) // 128)
                dsup = [(a, min(a + 2, ngt)) for a in range(0, ngt, 2)]
                xp1 = pd.enter_context(tc.tile_pool(name="pd_x", bufs=4))
                pp1 = pd.enter_context(
                    tc.tile_pool(name="pd_ps", bufs=2, space="PSUM"))
                rp1 = pd.enter_context(tc.tile_pool(name="pd_row", bufs=4))
                for ga, gb in dsup:
                    nsub = gb - ga
                    hx = xp1.tile([128, 2, 128], F16, tag="hx")
                    r0, r1 = ga * 128, min(gb * 128, N)
                    hxf = hx[:].rearrange("p g j -> p (g j)")
                    w0 = 0
                    r = r0
                    while r < r1:
                        j = r // NLOC
                        seg = min(r1, (j + 1) * NLOC) - r
                        nc.sync.dma_start(
                            out=hxf[:, w0:w0 + seg],
                            in_=agout[bass.ds(sregs[j % NCORES], 128),
                                      r - j * NLOC:r - j * NLOC + seg])
                        w0 += seg
                        r += seg
                    row = rp1.tile([128, 2, T1W], F16, tag="row")
                    for g2 in range(nsub):
                        ps = pp1.tile([128, 528], F32, tag="ps")
                        nc.tensor.matmul(ps[:], hx[:, g2, :], W1s[:],
                                         start=True, stop=True)
                        nc.scalar.copy(row[:, g2, 0:264], ps[:, 0:264])
                        nc.vector.tensor_copy(row[:, g2, 264:528],
                                              ps[:, 264:528])
                    eng = nc.scalar if ga % 4 else nc.sync
                    g0 = ga * 128
                    if nsub == 2 and g0 + 256 <= SPLIT:
                        eng.dma_start(
                            out=t1lo[g0:g0 + 256, 0:528]
                            .rearrange("(g p) w -> p g w", p=128),
                            in_=row[:, :, 0:528])
                    elif nsub == 2:
                        o = g0 - SPLIT
                        eng.dma_start(
                            out=t1hi[o:o + 256, 0:528]
                            .rearrange("(g p) w -> p g w", p=128),
                            in_=row[:, :, 0:528])
                    else:
                        o = g0 - SPLIT
                        eng.dma_start(out=t1hi[o:o + 128, 0:528],
                                      in_=row[:, 0, 0:528])
                    for g2 in range(nsub):
                        g = ga + g2
                        if g < lt:
                            pt = pp1.tile([8, 128], F16, tag="pt")
                            nc.tensor.transpose(
                                pt[:], row[:, g2, 520:528], idents[:])
                            adt_store(adT1, pt, g)

        # ---------------- phase E: layer-1 edges + epilogue ---------------
        def post1(t, P1, op, pp2, fin):
            zbig, nmxb, seb = fin
            r8 = op.tile([128, 8], F32, tag="r8")
            nc.vector.tensor_scalar_add(r8[:], P1[:, 512:520], 1e-16)
            nc.vector.reciprocal(r8[:], r8[:])
            nc.vector.tensor_scalar_mul(r8[:], r8[:], 1.0 / HEADS)
            zw = op.tile([128, 512], F32, tag="zw")
            nc.vector.tensor_tensor(
                out=zw[:].rearrange("p (d h) -> p d h", h=HEADS),
                in0=P1[:, 0:512].rearrange("p (d h) -> p d h", h=HEADS),
                in1=r8[:].unsqueeze(1).broadcast_to([128, 64, HEADS]),
                op=mybir.AluOpType.mult)
            z = zbig[:, t * NCLASS:(t + 1) * NCLASS]
            nc.vector.reduce_sum(
                z, zw[:].rearrange("p (d h) -> p d h", h=HEADS),
                axis=mybir.AxisListType.X)
            nc.vector.tensor_tensor(out=z, in0=z, in1=b1s[:],
                                    op=mybir.AluOpType.add)
            nmx = nmxb[:, t:t + 1]
            nc.vector.reduce_max(nmx, z, axis=mybir.AxisListType.X,
                                 negate=True)
            ez = op.tile([128, NCLASS], F32, tag="ez")
            nc.scalar.activation(ez[:], z,
                                 mybir.ActivationFunctionType.Exp,
                                 bias=nmx, accum_out=seb[:, t:t + 1])

        if "E" in phases:
            fpool = stk.enter_context(tc.tile_pool(name="fin", bufs=1))
            zbig = fpool.tile([128, lt * NCLASS], F32)
            nmxb = fpool.tile([128, lt], F32)
            seb = fpool.tile([128, lt], F32)
            edge_phase(1, t1lo, t1hi, adT1, 512, T1W, GROUPS1, ep1i,
                       post1, (zbig, nmxb, seb))
            # batched log-softmax tail: one Ln + two broadcast ops + 2 DMAs
            nc.scalar.activation(seb[:], seb[:],
                                 mybir.ActivationFunctionType.Ln)
            nc.vector.tensor_tensor(
                out=zbig[:].rearrange("p (t c) -> p t c", c=NCLASS),
                in0=zbig[:].rearrange("p (t c) -> p t c", c=NCLASS),
                in1=nmxb[:].unsqueeze(-1).broadcast_to([128, lt, NCLASS]),
                op=mybir.AluOpType.add)
            nc.vector.tensor_tensor(
                out=zbig[:].rearrange("p (t c) -> p t c", c=NCLASS),
                in0=zbig[:].rearrange("p (t c) -> p t c", c=NCLASS),
                in1=seb[:].unsqueeze(-1).broadcast_to([128, lt, NCLASS]),
                op=mybir.AluOpType.subtract)
            nfull = (lt - 1) * 128
            rlast = LAST_ROWS if lt == LT else 128
            nc.sync.dma_start(
                out=out[0:nfull, :].rearrange("(t p) c -> p t c", p=128),
                in_=zbig[:].rearrange("p (t c) -> p t c", c=NCLASS)
                [:, 0:lt - 1, :])
            nc.sync.dma_start(
                out=out[nfull:nfull + rlast, :],
                in_=zbig[0:rlast, (lt - 1) * NCLASS:lt * NCLASS])

    nc.compile()
    return nc


# --------------------------------------------------------------------------
# entry point
# --------------------------------------------------------------------------

def kernel(**inputs) -> np.ndarray:
    NLk, NHk, in_maps = _prep_inputs(**inputs)
    key = (NLk, NHk)
    if key not in _cache:
        _cache[key] = build(NLk, NHk)
    nc = _cache[key]
    res = run_bass_kernel_spmd(nc, in_maps, list(range(NCORES)))
    return np.concatenate([res.results[c]["out"] for c in range(NCORES)], 0)


# revision 22
# speedup vs baseline: 1.0634x; 1.0634x over previous
"""2-layer GAT (nn_GAT_31490700214331) on 8 Trainium2 NeuronCores.

Strategy (dst-sharded, SPMD, per-core-rotated node layout) — v2:
  - Nodes block-partitioned: core c owns nodes [c*6250, (c+1)*6250), every
    table uses the rotated row order (n - c*6250) mod 50000 so the SPMD
    program has no core-dependent offsets.
  - Layer-0 node features (h0|as0|ad0 = x @ W0cat) are computed replicated
    into a rotated DRAM table with 512B rows; layer-1 rows (h1|as1|ad1)
    are 1280B with 1056B content.  Edges are grouped by 128-dst tiles and
    source rows fetched with dma_gather through lo/hi table views (int16
    indices), batched over groups of tiles to amortize SWDGE overhead.
  - alpha_dst values are NOT dma-gathered per edge (256B/slot minimum).
    Instead each core keeps a transposed local table adT[16, 6272] (f32,
    built with PE transposes during phases A/D) and the per-slot lookup
    runs on gpsimd via ap_gather (indices in epack), then 16-wide PE
    transposes bring the result back slot-major through PSUM.
  - Edge softmax (safe without segment-max: |e| small) and the weighted
    aggregation are fused into per-chunk 128x128 incidence matmuls; the
    exp(e) values ride as 8 extra columns of the moving operand so a
    single PSUM accumulation produces numerator and denominator.
  - Between layers the ELU'd hidden state is AllGather'd (feature-major)
    and rotated into per-core order with partition-id-offset DMA reads.
  - alpha projections fold into the weight matmuls on the host:
    W0cat=[256,144] gives h0|as0|ad0; W1cat=[128,528] gives h1|as1|ad1.

Self-contained: call kernel(**inputs) with the full-problem arrays.
"""
import numpy as np
from contextlib import ExitStack

import concourse.bacc as bacc
import concourse.bass as bass
import concourse.mybir as mybir
from concourse.tile import TileContext
from concourse.bass_utils import run_bass_kernel_spmd

F16 = mybir.dt.float16
F32 = mybir.dt.float32
I16 = mybir.dt.int16

N = 50000
NFEAT = 256
NHID = 128
NCLASS = 64
HEADS = 8
SLOPE = 0.2
NCORES = 8
NLOC = N // NCORES           # 6250
LT = (NLOC + 127) // 128     # 49 local dst tiles
LAST_ROWS = NLOC - (LT - 1) * 128   # 106 rows in the last tile
GT = 392                     # global node tiles (392*128 = 50176)
GROWS = GT * 128
SPLIT = 25088                # low/high gather-table split (196 tiles)
SENT = 300.0                 # dst_rel sentinel for padding slots
T0W = 256                    # t0 row: [h0(128)|as0(8)|ad0(8)|junk] f16
T1W = 640                    # t1 row: [h1(512)|as1(8)|ad1(8)|junk] f16

GROUPS0 = [(7 * i, 7) for i in range(7)]                     # L0 tile groups
GROUPS1 = [(2 * i, 2) for i in range(24)] + [(48, 1)]        # L1 tile groups
HALF = 3200                  # adT split: dsts [0,3200) core0, rest core1
ADTE = HALF + 1              # num_elems per half (last col = zero slot)

_cache = {}


# --------------------------------------------------------------------------
# host-side preparation
# --------------------------------------------------------------------------

def _wrap_idx(idx):
    """[n] int -> [128, n//16] int16 wrapped gather-index layout."""
    n = idx.shape[0]
    assert n % 16 == 0
    w = idx.reshape(n // 16, 16).T.astype(np.int16)
    return np.tile(w, (8, 1))


def _wrap16(idx):
    """[n] int -> [16, n//16] int16 (single gpsimd core)."""
    n = idx.shape[0]
    assert n % 16 == 0
    return idx.reshape(n // 16, 16).T.astype(np.int16)


def _epw(gsz, NL, NH):
    CH = NL + NH
    w = gsz * (NL * 8 + NH * 8 + CH + CH * 8)
    return w, ((w + 15) // 16) * 16


def _prep_edges(src, dst):
    cores = []
    for c in range(NCORES):
        m = (dst >= c * NLOC) & (dst < (c + 1) * NLOC)
        s = src[m].astype(np.int64)
        d = dst[m].astype(np.int64) - c * NLOC
        order = np.argsort(d, kind="stable")
        s, d = s[order], d[order]
        s_rot = (s - c * NLOC) % N
        tiles = []
        for t in range(LT):
            sel = (d >= t * 128) & (d < (t + 1) * 128)
            st, dt = s_rot[sel], d[sel] - t * 128
            lo = st < SPLIT
            tiles.append((st[lo], dt[lo], st[~lo] - SPLIT, dt[~lo]))
        cores.append(tiles)
    nl = max(len(t[0]) for tl in cores for t in tl)
    nh = max(len(t[2]) for tl in cores for t in tl)
    NL = max(1, (nl + 127) // 128)
    NH = max(1, (nh + 127) // 128)
    assert NL * 128 <= 1024 and NH * 128 <= 1024, (NL, NH)
    CH = NL + NH

    out = []
    for c in range(NCORES):
        tiles = []
        for t in range(LT):
            sl, dl, sh, dh = cores[c][t]
            il = np.zeros(NL * 128, np.int64)
            il[: len(sl)] = sl
            ih = np.zeros(NH * 128, np.int64)
            ih[: len(sh)] = sh
            axg = np.zeros(CH * 128, np.int64)
            axg[: len(dl)] = t * 128 + dl
            axg[NL * 128: NL * 128 + len(dh)] = t * 128 + dh
            ax0 = np.where(axg < HALF, axg, HALF)          # core-0 indices
            ax1 = np.where(axg >= HALF, axg - HALF, HALF)  # core-1 indices
            rl = np.full(NL * 128, SENT)
            rl[: len(dl)] = dl
            rh = np.full(NH * 128, SENT)
            rh[: len(dh)] = dh
            dr = np.concatenate([rl, rh]).reshape(CH, 128).T  # [128, CH]
            tiles.append((il, ih, ax0, ax1, dr.astype(np.float16)))

        def pack(groups):
            _, epwp = _epw(groups[0][1], NL, NH)
            blks = np.zeros((len(groups), 128, epwp), np.int16)
            for gi, (t0, gsz) in enumerate(groups):
                o_ih = gsz * NL * 8
                o_dr = gsz * (NL + NH) * 8
                o_ax = o_dr + gsz * CH
                il = np.concatenate([tiles[t][0] for t in range(t0, t0 + gsz)])
                ih = np.concatenate([tiles[t][1] for t in range(t0, t0 + gsz)])
                ax0 = np.concatenate(
                    [tiles[t][2] for t in range(t0, t0 + gsz)])
                ax1 = np.concatenate(
                    [tiles[t][3] for t in range(t0, t0 + gsz)])
                dr = np.concatenate([tiles[t][4] for t in range(t0, t0 + gsz)],
                                    axis=1)
                blks[gi][:, 0:o_ih] = _wrap_idx(il)
                blks[gi][:, o_ih:o_dr] = _wrap_idx(ih)
                blks[gi][:, o_dr:o_ax] = dr.view(np.int16)
                blks[gi][0:16, o_ax:o_ax + gsz * CH * 8] = _wrap16(ax0)
                blks[gi][32:48, o_ax:o_ax + gsz * CH * 8] = _wrap16(ax1)
            return blks

        out.append(dict(ep0=np.ascontiguousarray(pack(GROUPS0)),
                        ep1=np.ascontiguousarray(pack(GROUPS1))))
    return NL, NH, out


def _prep_inputs(x, edge_index, W0, a_src0, a_dst0, b0, W1, a_src1, a_dst1,
                 b1):
    src = np.asarray(edge_index[0]).astype(np.int64)
    dst = np.asarray(edge_index[1]).astype(np.int64)
    NL, NH, edata = _prep_edges(src, dst)

    def bd(a):  # [H, D] -> blockdiag [H*D, H]
        a = np.asarray(a, np.float32)
        H, D = a.shape
        m = np.zeros((H * D, H), np.float32)
        for h in range(H):
            m[h * D:(h + 1) * D, h] = a[h]
        return m

    W0 = np.asarray(W0, np.float32)
    W1 = np.asarray(W1, np.float32)
    W0a = np.concatenate([W0 @ bd(a_src0), W0 @ bd(a_dst0)], 1)  # [256, 16]
    # head-innermost feature interleave: new col d*8+h <- old col h*D+d
    perm0 = np.array([(f % 8) * 16 + f // 8 for f in range(128)])
    perm1 = np.array([(f % 8) * 64 + f // 8 for f in range(512)])
    W0cat = np.concatenate([W0[:, perm0], W0a], 1)               # [256, 144]
    W1a = np.concatenate([W1 @ bd(a_src1), W1 @ bd(a_dst1)], 1)  # [128, 16]
    W1cat = np.concatenate([W1[perm0][:, perm1], W1a[perm0]], 1)  # [128, 528]

    x = np.asarray(x, np.float32)
    ident = np.eye(128, dtype=np.float16)
    colio = np.tile(np.arange(128, dtype=np.float16)[None, :], (128, 1))
    b0b = np.tile(np.asarray(b0, np.float32)[None, :], (128, 1))
    b1b = np.tile(np.asarray(b1, np.float32)[None, :], (128, 1))

    in_maps = []
    for c in range(NCORES):
        rot = np.roll(np.arange(N), -c * NLOC)
        xr = np.zeros((GROWS, NFEAT), np.float16)
        xr[:N] = x[rot].astype(np.float16)
        xtt = xr.reshape(GROWS // 128, 128, 2, 128).transpose(0, 3, 2, 1)
        m = dict(
            xT=np.ascontiguousarray(xtt),
            W0=np.ascontiguousarray(
                W0cat.astype(np.float16).reshape(2, 128, NHID + 16)),
            W1cat=np.ascontiguousarray(W1cat.astype(np.float16)),
            b0b=np.ascontiguousarray(b0b[:, perm0]), b1b=b1b,
            ident=ident, colio=colio,
            **edata[c],
        )
        in_maps.append(m)
    return NL, NH, in_maps


# --------------------------------------------------------------------------
# device program
# --------------------------------------------------------------------------

def build(NL, NH, lt=LT, gt=GT, debug=False, phases="ABCDE"):
    CH = NL + NH
    HID16 = NHID + 16

    _, EPW0P = _epw(GROUPS0[0][1], NL, NH)
    _, EPW1P = _epw(GROUPS1[0][1], NL, NH)
    nc = bacc.Bacc("TRN2")
    xT = nc.dram_tensor("xT", [GROWS // 128, 128, 2, 128], F16,
                        kind="ExternalInput")
    W0i = nc.dram_tensor("W0", [2, 128, HID16], F16, kind="ExternalInput")
    W1cati = nc.dram_tensor("W1cat", [NHID, 528], F16, kind="ExternalInput")
    b0bi = nc.dram_tensor("b0b", [128, NHID], F32, kind="ExternalInput")
    b1bi = nc.dram_tensor("b1b", [128, NCLASS], F32, kind="ExternalInput")
    identi = nc.dram_tensor("ident", [128, 128], F16, kind="ExternalInput")
    colioi = nc.dram_tensor("colio", [128, 128], F16, kind="ExternalInput")
    ep0i = nc.dram_tensor("ep0", [len(GROUPS0), 128, EPW0P], I16,
                          kind="ExternalInput")
    ep1i = nc.dram_tensor("ep1", [len(GROUPS1), 128, EPW1P], I16,
                          kind="ExternalInput")
    out = nc.dram_tensor("out", [NLOC, NCLASS], F32, kind="ExternalOutput")

    with TileContext(nc) as tc, ExitStack() as stk:
        regs = {}
        for _, g in GROUPS0 + GROUPS1:
            for n in (g * NL * 128, g * NH * 128):
                if n not in regs:
                    regs[n] = nc.gpsimd.to_reg(n)
        dpool = stk.enter_context(
            tc.tile_pool(name="dram", bufs=1, space="DRAM"))
        t0lo = dpool.tile([SPLIT, T0W], F16, tag="t0lo")
        t0hi = dpool.tile([GROWS - SPLIT, T0W], F16, tag="t0hi")
        t1lo = dpool.tile([SPLIT, T1W], F16, tag="t1lo")
        t1hi = dpool.tile([GROWS - SPLIT, T1W], F16, tag="t1hi")
        agin = dpool.tile([128, NLOC], F16, tag="agin")
        agout = dpool.tile([NCORES * 128, NLOC], F16, tag="agout",
                           addr_space="Shared")

        cpool = stk.enter_context(tc.tile_pool(name="const", bufs=1))
        W0s = cpool.tile([128, 2, HID16], F16)
        nc.sync.dma_start(out=W0s[:], in_=W0i.rearrange("k p n -> p k n"))
        W1s = cpool.tile([128, 528], F16)
        nc.sync.dma_start(out=W1s[:], in_=W1cati[:])
        b0s = cpool.tile([128, NHID], F32)
        nc.sync.dma_start(out=b0s[:], in_=b0bi[:])
        b1s = cpool.tile([128, NCLASS], F32)
        nc.sync.dma_start(out=b1s[:], in_=b1bi[:])
        idents = cpool.tile([128, 128], F16)
        nc.sync.dma_start(out=idents[:], in_=identi[:])
        colios = cpool.tile([128, 128], F16)
        nc.sync.dma_start(out=colios[:], in_=colioi[:])
        ident64f = cpool.tile([64, 64], F32)
        nc.vector.tensor_copy(ident64f[:], idents[0:64, 0:64])
        adT0 = cpool.tile([64, ADTE], F32)
        nc.vector.memset(adT0[:], 0)
        adT1 = cpool.tile([64, ADTE], F32)
        nc.vector.memset(adT1[:], 0)

        def adt_store(adT, pt, g):
            hf = 1 if g * 128 >= HALF else 0
            nc.vector.tensor_copy(
                adT[32 * hf:32 * hf + 8,
                    g * 128 - HALF * hf:(g + 1) * 128 - HALF * hf], pt[:])

        # ---------------- phase A: layer-0 tables (replicated) ------------
        if "A" in phases:
            with ExitStack() as pa:
                xp = pa.enter_context(tc.tile_pool(name="pa_x", bufs=4))
                pp = pa.enter_context(
                    tc.tile_pool(name="pa_ps", bufs=2, space="PSUM"))
                rp = pa.enter_context(tc.tile_pool(name="pa_row", bufs=4))
                assert gt % 4 == 0
                for gg in range(gt // 4):
                    xa = xp.tile([128, 4, 2, 128], F16, tag="xa")
                    (nc.sync if gg % 2 else nc.gpsimd).dma_start(
                        out=xa[:],
                        in_=xT[4 * gg:4 * gg + 4].rearrange(
                            "g p k j -> p g k j"))
                    row = rp.tile([128, 4, T0W], F16, tag="row")
                    for g2 in range(4):
                        ps = pp.tile([128, HID16], F32, tag="ps")
                        for k in range(2):
                            nc.tensor.matmul(ps[:], xa[:, g2, k, :],
                                             W0s[:, k, :],
                                             start=(k == 0), stop=(k == 1))
                        if g2 % 2:
                            nc.scalar.copy(row[:, g2, 0:HID16], ps[:])
                        else:
                            nc.vector.tensor_copy(row[:, g2, 0:HID16],
                                                  ps[:])
                        g = 4 * gg + g2
                        if g < lt:
                            pt = pp.tile([8, 128], F16, tag="pt")
                            nc.tensor.transpose(
                                pt[:], row[:, g2, NHID + 8:HID16],
                                idents[:])
                            adt_store(adT0, pt, g)
                    eng = nc.scalar if gg % 2 else nc.sync
                    g0 = 4 * gg * 128
                    if g0 + 512 <= SPLIT:
                        eng.dma_start(
                            out=t0lo[g0:g0 + 512, :]
                            .rearrange("(g p) w -> p g w", p=128),
                            in_=row[:])
                    else:
                        o = g0 - SPLIT
                        eng.dma_start(
                            out=t0hi[o:o + 512, :]
                            .rearrange("(g p) w -> p g w", p=128),
                            in_=row[:])

        # ---------------- shared edge phase -------------------------------
        def edge_phase(layer, tbl_lo, tbl_hi, adT, fdim, trow, groups, epi,
                       post_fn, fin):
            fd8 = fdim + 8
            D = fdim // HEADS
            with ExitStack() as pb:
                ip = pb.enter_context(
                    tc.tile_pool(name=f"ix{layer}", bufs=2))
                gp = pb.enter_context(
                    tc.tile_pool(name=f"gg{layer}", bufs=2))
                apd = pb.enter_context(
                    tc.tile_pool(name=f"ga{layer}", bufs=2))
                rp2 = pb.enter_context(
                    tc.tile_pool(name=f"rh{layer}", bufs=3))
                pp2 = pb.enter_context(
                    tc.tile_pool(name=f"ps{layer}", bufs=2, space="PSUM"))
                pp3 = pb.enter_context(
                    tc.tile_pool(name=f"px{layer}", bufs=3, space="PSUM"))
                pp4 = pb.enter_context(
                    tc.tile_pool(name=f"pa{layer}", bufs=1, space="PSUM"))
                op = pb.enter_context(
                    tc.tile_pool(name=f"po{layer}", bufs=3))
                for gi, (t0g, gsz) in enumerate(groups):
                    o_ih = gsz * NL * 8
                    o_dr = gsz * (NL + NH) * 8
                    o_ax = o_dr + gsz * CH
                    nli, nhi = gsz * NL * 128, gsz * NH * 128
                    ep = ip.tile([128, epi.shape[2]], I16, tag="ep")
                    nc.sync.dma_start(out=ep[:], in_=epi[gi])
                    Glo = gp.tile([128, gsz * NL, trow], F16, tag="Glo")
                    nc.gpsimd.dma_gather(Glo[:], tbl_lo[:],
                                         ep[:, 0:o_ih], nli, regs[nli],
                                         trow)
                    Ghi = gp.tile([128, gsz * NH, trow], F16, tag="Ghi")
                    nc.gpsimd.dma_gather(Ghi[:], tbl_hi[:],
                                         ep[:, o_ih:o_dr], nhi, regs[nhi],
                                         trow)
                    for tl in range(gsz):
                        t = t0g + tl
                        admT = apd.tile([64, CH * 128], F32, tag="admT")
                        nc.gpsimd.ap_gather(
                            admT[:].unsqueeze(-1), adT[:].unsqueeze(-1),
                            ep[0:64, o_ax + tl * CH * 8:
                               o_ax + (tl + 1) * CH * 8],
                            channels=64, num_elems=ADTE, d=1,
                            num_idxs=CH * 128)
                        dr = ep[:, o_dr + tl * CH:
                                o_dr + (tl + 1) * CH].bitcast(F16)
                        admP = pp4.tile([128, CH * 64], F32, tag="admP")
                        for mm in range(CH):
                            nc.tensor.transpose(
                                admP[:, mm * 64:(mm + 1) * 64],
                                admT[:, mm * 128:(mm + 1) * 128],
                                ident64f[:])
                        adm = apd.tile([128, CH, 8], F16, tag="adm")
                        adml = apd.tile([128, CH, 8], F16, tag="adml")
                        admPv = admP[:].rearrange("p (c w) -> p c w", w=64)
                        nc.scalar.copy(adml[:], admPv[:, :, 0:8])
                        nc.vector.tensor_tensor(
                            out=adm[:], in0=adml[:],
                            in1=admPv[:, :, 32:40], op=mybir.AluOpType.add)

                        glo = Glo[:, tl * NL:(tl + 1) * NL, :]
                        ghi = Ghi[:, tl * NH:(tl + 1) * NH, :]
                        inc = rp2.tile([128, CH, 128], F16, tag="inc")
                        nc.vector.tensor_tensor(
                            out=inc[:],
                            in0=dr.unsqueeze(-1)
                            .broadcast_to([128, CH, 128]),
                            in1=colios[:].unsqueeze(1)
                            .broadcast_to([128, CH, 128]),
                            op=mybir.AluOpType.is_equal)
                        EX = op.tile([128, CH, 8], F16, tag="EX")
                        nc.vector.tensor_tensor(
                            out=EX[:, 0:NL, :],
                            in0=glo[:, :, fdim:fd8],
                            in1=adm[:, 0:NL, :], op=mybir.AluOpType.add)
                        nc.vector.tensor_tensor(
                            out=EX[:, NL:CH, :],
                            in0=ghi[:, :, fdim:fd8],
                            in1=adm[:, NL:CH, :], op=mybir.AluOpType.add)
                        nc.scalar.activation(
                            EX[:], EX[:],
                            mybir.ActivationFunctionType.Prelu, alpha=SLOPE)
                        nc.scalar.activation(
                            EX[:], EX[:], mybir.ActivationFunctionType.Exp)

                        R2 = rp2.tile([128, CH, fd8], F16, tag="R2")
                        nc.scalar.copy(R2[:, :, fdim:fd8], EX[:])
                        nc.vector.tensor_tensor(
                            out=R2[:, 0:NL, 0:fdim]
                            .rearrange("p c (d h) -> p c d h", h=HEADS),
                            in0=glo[:, :, 0:fdim]
                            .rearrange("p c (d h) -> p c d h", h=HEADS),
                            in1=EX[:, 0:NL, :].unsqueeze(2)
                            .broadcast_to([128, NL, D, HEADS]),
                            op=mybir.AluOpType.mult)
                        nc.vector.tensor_tensor(
                            out=R2[:, NL:CH, 0:fdim]
                            .rearrange("p c (d h) -> p c d h", h=HEADS),
                            in0=ghi[:, :, 0:fdim]
                            .rearrange("p c (d h) -> p c d h", h=HEADS),
                            in1=EX[:, NL:CH, :].unsqueeze(2)
                            .broadcast_to([128, NH, D, HEADS]),
                            op=mybir.AluOpType.mult)

                        P1 = pp3.tile([128, fd8], F32, tag="P1")
                        for ch in range(CH):
                            nc.tensor.matmul(P1[:], inc[:, ch, :],
                                             R2[:, ch, :],
                                             start=(ch == 0),
                                             stop=(ch == CH - 1))
                        post_fn(t, P1, op, pp2, fin)

        # ---- L0 post: softmax-div, +b0, ELU, transpose, store ------------
        def post0(t, P1, op, pp2, fin):
            rows = 128 if t < lt - 1 else LAST_ROWS
            r8 = op.tile([128, 8], F32, tag="r8")
            nc.vector.tensor_scalar_add(r8[:], P1[:, NHID:NHID + 8], 1e-16)
            nc.vector.reciprocal(r8[:], r8[:])
            z = op.tile([128, NHID], F32, tag="z")
            nc.vector.tensor_tensor(
                out=z[:].rearrange("p (d h) -> p d h", h=HEADS),
                in0=P1[:, 0:NHID].rearrange("p (d h) -> p d h", h=HEADS),
                in1=r8[:].unsqueeze(1).broadcast_to([128, 16, HEADS]),
                op=mybir.AluOpType.mult)
            nc.vector.tensor_tensor(out=z[:], in0=z[:], in1=b0s[:],
                                    op=mybir.AluOpType.add)
            zr = op.tile([128, NHID], F32, tag="zr")
            nc.scalar.activation(zr[:], z[:],
                                 mybir.ActivationFunctionType.Relu)
            zm = op.tile([128, NHID], F32, tag="zm")
            nc.vector.tensor_tensor(out=zm[:], in0=z[:], in1=zr[:],
                                    op=mybir.AluOpType.subtract)
            nc.scalar.activation(zm[:], zm[:],
                                 mybir.ActivationFunctionType.Exp)
            h1 = op.tile([128, NHID], F16, tag="h1")
            nc.vector.scalar_tensor_tensor(
                h1[:], zm[:], -1.0, zr[:],
                op0=mybir.AluOpType.add, op1=mybir.AluOpType.add)
            pst = pp2.tile([128, 128], F16, tag="pst")
            nc.tensor.transpose(pst[:], h1[:], idents[:])
            hT = op.tile([128, 128], F16, tag="hT")
            nc.scalar.copy(hT[:], pst[:])
            nc.sync.dma_start(
                out=agin[:, t * 128:t * 128 + rows], in_=hT[:, 0:rows])

        if "B" in phases:
            edge_phase(0, t0lo, t0hi, adT0, NHID, T0W, GROUPS0, ep0i,
                       post0, None)

        # ---------------- phase C: AllGather + rotation -------------------
        sregs = None
        if "C" in phases:
            nc.gpsimd.collective_compute(
                "AllGather", mybir.AluOpType.bypass,
                replica_groups=[list(range(NCORES))],
                ins=[agin[:]], outs=[agout[:]])
            pid = nc.partition_id(engines=[mybir.EngineType.SP])
            sregs = [nc.sync.snap(((j + pid) % NCORES) * 128)
                     for j in range(NCORES)]

        # ---------------- phase D: layer-1 tables -------------------------
        if "D" in phases and sregs is not None:
            with ExitStack() as pd:
                ngt = min(gt, (N + 127) // 128)
                spans = [(a, min(a + 8, ngt)) for a in range(0, ngt, 8)]
                xp1 = pd.enter_context(tc.tile_pool(name="pd_x", bufs=3))
                pp1 = pd.enter_context(
                    tc.tile_pool(name="pd_ps", bufs=2, space="PSUM"))
                rp1 = pd.enter_context(tc.tile_pool(name="pd_row", bufs=4))
                for si, (sa, sb) in enumerate(spans):
                    hx = xp1.tile([128, 8, 128], F16, tag="hx")
                    r0, r1 = sa * 128, min(sb * 128, N)
                    hxf = hx[:].rearrange("p g j -> p (g j)")
                    w0 = 0
                    r = r0
                    heng = nc.sync
                    while r < r1:
                        j = r // NLOC
                        seg = min(r1, (j + 1) * NLOC) - r
                        heng.dma_start(
                            out=hxf[:, w0:w0 + seg],
                            in_=agout[bass.ds(sregs[j % NCORES], 128),
                                      r - j * NLOC:r - j * NLOC + seg])
                        w0 += seg
                        r += seg
                    for ga in range(sa, sb, 2):
                        gb = min(ga + 2, sb)
                        nsub = gb - ga
                        row = rp1.tile([128, 2, T1W], F16, tag="row")
                        ps2 = pp1.tile([128, 2, 528], F32, tag="ps")
                        for g2 in range(nsub):
                            nc.tensor.matmul(ps2[:, g2, :],
                                             hx[:, ga - sa + g2, :], W1s[:],
                                             start=True, stop=True)
                        nc.scalar.copy(row[:, 0:nsub, 0:176],
                                       ps2[:, 0:nsub, 0:176])
                        nc.vector.tensor_copy(row[:, 0:nsub, 176:352],
                                              ps2[:, 0:nsub, 176:352])
                        nc.gpsimd.tensor_copy(row[:, 0:nsub, 352:528],
                                              ps2[:, 0:nsub, 352:528])
                        eng = nc.scalar if ga % 4 else nc.sync
                        g0 = ga * 128
                        if nsub == 2 and g0 + 256 <= SPLIT:
                            eng.dma_start(
                                out=t1lo[g0:g0 + 256, 0:528]
                                .rearrange("(g p) w -> p g w", p=128),
                                in_=row[:, :, 0:528])
                        elif nsub == 2:
                            o = g0 - SPLIT
                            eng.dma_start(
                                out=t1hi[o:o + 256, 0:528]
                                .rearrange("(g p) w -> p g w", p=128),
                                in_=row[:, :, 0:528])
                        else:
                            o = g0 - SPLIT
                            eng.dma_start(out=t1hi[o:o + 128, 0:528],
                                          in_=row[:, 0, 0:528])
                        for g2 in range(nsub):
                            g = ga + g2
                            if g < lt:
                                pt = pp1.tile([8, 128], F16, tag="pt")
                                nc.tensor.transpose(
                                    pt[:], row[:, g2, 520:528], idents[:])
                                adt_store(adT1, pt, g)

        # ---------------- phase E: layer-1 edges + epilogue ---------------
        def post1(t, P1, op, pp2, fin):
            zbig, nmxb, seb = fin
            r8 = op.tile([128, 8], F32, tag="r8")
            nc.vector.tensor_scalar_add(r8[:], P1[:, 512:520], 1e-16)
            nc.vector.reciprocal(r8[:], r8[:])
            nc.vector.tensor_scalar_mul(r8[:], r8[:], 1.0 / HEADS)
            zw = op.tile([128, 512], F32, tag="zw")
            nc.vector.tensor_tensor(
                out=zw[:].rearrange("p (d h) -> p d h", h=HEADS),
                in0=P1[:, 0:512].rearrange("p (d h) -> p d h", h=HEADS),
                in1=r8[:].unsqueeze(1).broadcast_to([128, 64, HEADS]),
                op=mybir.AluOpType.mult)
            z = zbig[:, t * NCLASS:(t + 1) * NCLASS]
            nc.vector.reduce_sum(
                z, zw[:].rearrange("p (d h) -> p d h", h=HEADS),
                axis=mybir.AxisListType.X)
            nc.vector.tensor_tensor(out=z, in0=z, in1=b1s[:],
                                    op=mybir.AluOpType.add)
            nmx = nmxb[:, t:t + 1]
            nc.vector.reduce_max(nmx, z, axis=mybir.AxisListType.X,
                                 negate=True)
            ez = op.tile([128, NCLASS], F32, tag="ez")
            nc.scalar.activation(ez[:], z,
                                 mybir.ActivationFunctionType.Exp,
                                 bias=nmx, accum_out=seb[:, t:t + 1])

        if "E" in phases:
            fpool = stk.enter_context(tc.tile_pool(name="fin", bufs=1))
            zbig = fpool.tile([128, lt * NCLASS], F32)
            nmxb = fpool.tile([128, lt], F32)
            seb = fpool.tile([128, lt], F32)
            edge_phase(1, t1lo, t1hi, adT1, 512, T1W, GROUPS1, ep1i,
                       post1, (zbig, nmxb, seb))
            # batched log-softmax tail: one Ln + two broadcast ops + 2 DMAs
            nc.scalar.activation(seb[:], seb[:],
                                 mybir.ActivationFunctionType.Ln)
            nc.vector.tensor_tensor(
                out=zbig[:].rearrange("p (t c) -> p t c", c=NCLASS),
                in0=zbig[:].rearrange("p (t c) -> p t c", c=NCLASS),
                in1=nmxb[:].unsqueeze(-1).broadcast_to([128, lt, NCLASS]),
                op=mybir.AluOpType.add)
            nc.vector.tensor_tensor(
                out=zbig[:].rearrange("p (t c) -> p t c", c=NCLASS),
                in0=zbig[:].rearrange("p (t c) -> p t c", c=NCLASS),
                in1=seb[:].unsqueeze(-1).broadcast_to([128, lt, NCLASS]),
                op=mybir.AluOpType.subtract)
            nfull = (lt - 1) * 128
            rlast = LAST_ROWS if lt == LT else 128
            nc.sync.dma_start(
                out=out[0:nfull, :].rearrange("(t p) c -> p t c", p=128),
                in_=zbig[:].rearrange("p (t c) -> p t c", c=NCLASS)
                [:, 0:lt - 1, :])
            nc.sync.dma_start(
                out=out[nfull:nfull + rlast, :],
                in_=zbig[0:rlast, (lt - 1) * NCLASS:lt * NCLASS])

    nc.compile()
    return nc


# --------------------------------------------------------------------------
# entry point
# --------------------------------------------------------------------------

def kernel(**inputs) -> np.ndarray:
    NLk, NHk, in_maps = _prep_inputs(**inputs)
    key = (NLk, NHk)
    if key not in _cache:
        _cache[key] = build(NLk, NHk)
    nc = _cache[key]
    res = run_bass_kernel_spmd(nc, in_maps, list(range(NCORES)))
    return np.concatenate([res.results[c]["out"] for c in range(NCORES)], 0)


# revision 24
# speedup vs baseline: 1.0669x; 1.0033x over previous
"""2-layer GAT (nn_GAT_31490700214331) on 8 Trainium2 NeuronCores.

Strategy (dst-sharded, SPMD, per-core-rotated node layout) — v2:
  - Nodes block-partitioned: core c owns nodes [c*6250, (c+1)*6250), every
    table uses the rotated row order (n - c*6250) mod 50000 so the SPMD
    program has no core-dependent offsets.
  - Layer-0 node features (h0|as0|ad0 = x @ W0cat) are computed replicated
    into a rotated DRAM table with 512B rows; layer-1 rows (h1|as1|ad1)
    are 1280B with 1056B content.  Edges are grouped by 128-dst tiles and
    source rows fetched with dma_gather through lo/hi table views (int16
    indices), batched over groups of tiles to amortize SWDGE overhead.
  - alpha_dst values are NOT dma-gathered per edge (256B/slot minimum).
    Instead each core keeps a transposed local table adT[16, 6272] (f32,
    built with PE transposes during phases A/D) and the per-slot lookup
    runs on gpsimd via ap_gather (indices in epack), then 16-wide PE
    transposes bring the result back slot-major through PSUM.
  - Edge softmax (safe without segment-max: |e| small) and the weighted
    aggregation are fused into per-chunk 128x128 incidence matmuls; the
    exp(e) values ride as 8 extra columns of the moving operand so a
    single PSUM accumulation produces numerator and denominator.
  - Between layers the ELU'd hidden state is AllGather'd (feature-major)
    and rotated into per-core order with partition-id-offset DMA reads.
  - alpha projections fold into the weight matmuls on the host:
    W0cat=[256,144] gives h0|as0|ad0; W1cat=[128,528] gives h1|as1|ad1.

Self-contained: call kernel(**inputs) with the full-problem arrays.
"""
import numpy as np
from contextlib import ExitStack

import concourse.bacc as bacc
import concourse.bass as bass
import concourse.mybir as mybir
from concourse.tile import TileContext
from concourse.bass_utils import run_bass_kernel_spmd

F16 = mybir.dt.float16
F32 = mybir.dt.float32
I16 = mybir.dt.int16

N = 50000
NFEAT = 256
NHID = 128
NCLASS = 64
HEADS = 8
SLOPE = 0.2
NCORES = 8
NLOC = N // NCORES           # 6250
LT = (NLOC + 127) // 128     # 49 local dst tiles
LAST_ROWS = NLOC - (LT - 1) * 128   # 106 rows in the last tile
GT = 392                     # global node tiles (392*128 = 50176)
GROWS = GT * 128
SPLIT = 25088                # low/high gather-table split (196 tiles)
SENT = 300.0                 # dst_rel sentinel for padding slots
T0W = 256                    # t0 row: [h0(128)|as0(8)|ad0(8)|junk] f16
T1W = 640                    # t1 row: [h1(512)|as1(8)|ad1(8)|junk] f16

GROUPS0 = [(7 * i, 7) for i in range(7)]                     # L0 tile groups
GROUPS1 = [(2 * i, 2) for i in range(24)] + [(48, 1)]        # L1 tile groups
HALF = 3200                  # adT split: dsts [0,3200) core0, rest core1
ADTE = HALF + 1              # num_elems per half (last col = zero slot)

_cache = {}


# --------------------------------------------------------------------------
# host-side preparation
# --------------------------------------------------------------------------

def _wrap_idx(idx):
    """[n] int -> [128, n//16] int16 wrapped gather-index layout."""
    n = idx.shape[0]
    assert n % 16 == 0
    w = idx.reshape(n // 16, 16).T.astype(np.int16)
    return np.tile(w, (8, 1))


def _wrap16(idx):
    """[n] int -> [16, n//16] int16 (single gpsimd core)."""
    n = idx.shape[0]
    assert n % 16 == 0
    return idx.reshape(n // 16, 16).T.astype(np.int16)


def _epw(gsz, NL, NH):
    CH = NL + NH
    w = gsz * (NL * 8 + NH * 8 + CH + CH * 8)
    return w, ((w + 15) // 16) * 16


def _prep_edges(src, dst):
    cores = []
    for c in range(NCORES):
        m = (dst >= c * NLOC) & (dst < (c + 1) * NLOC)
        s = src[m].astype(np.int64)
        d = dst[m].astype(np.int64) - c * NLOC
        order = np.argsort(d, kind="stable")
        s, d = s[order], d[order]
        s_rot = (s - c * NLOC) % N
        tiles = []
        for t in range(LT):
            sel = (d >= t * 128) & (d < (t + 1) * 128)
            st, dt = s_rot[sel], d[sel] - t * 128
            lo = st < SPLIT
            tiles.append((st[lo], dt[lo], st[~lo] - SPLIT, dt[~lo]))
        cores.append(tiles)
    nl = max(len(t[0]) for tl in cores for t in tl)
    nh = max(len(t[2]) for tl in cores for t in tl)
    NL = max(1, (nl + 127) // 128)
    NH = max(1, (nh + 127) // 128)
    assert NL * 128 <= 1024 and NH * 128 <= 1024, (NL, NH)
    CH = NL + NH

    out = []
    for c in range(NCORES):
        tiles = []
        for t in range(LT):
            sl, dl, sh, dh = cores[c][t]
            il = np.zeros(NL * 128, np.int64)
            il[: len(sl)] = sl
            ih = np.zeros(NH * 128, np.int64)
            ih[: len(sh)] = sh
            axg = np.zeros(CH * 128, np.int64)
            axg[: len(dl)] = t * 128 + dl
            axg[NL * 128: NL * 128 + len(dh)] = t * 128 + dh
            ax0 = np.where(axg < HALF, axg, HALF)          # core-0 indices
            ax1 = np.where(axg >= HALF, axg - HALF, HALF)  # core-1 indices
            rl = np.full(NL * 128, SENT)
            rl[: len(dl)] = dl
            rh = np.full(NH * 128, SENT)
            rh[: len(dh)] = dh
            dr = np.concatenate([rl, rh]).reshape(CH, 128).T  # [128, CH]
            tiles.append((il, ih, ax0, ax1, dr.astype(np.float16)))

        def pack(groups):
            _, epwp = _epw(groups[0][1], NL, NH)
            blks = np.zeros((len(groups), 128, epwp), np.int16)
            for gi, (t0, gsz) in enumerate(groups):
                o_ih = gsz * NL * 8
                o_dr = gsz * (NL + NH) * 8
                o_ax = o_dr + gsz * CH
                il = np.concatenate([tiles[t][0] for t in range(t0, t0 + gsz)])
                ih = np.concatenate([tiles[t][1] for t in range(t0, t0 + gsz)])
                ax0 = np.concatenate(
                    [tiles[t][2] for t in range(t0, t0 + gsz)])
                ax1 = np.concatenate(
                    [tiles[t][3] for t in range(t0, t0 + gsz)])
                dr = np.concatenate([tiles[t][4] for t in range(t0, t0 + gsz)],
                                    axis=1)
                blks[gi][:, 0:o_ih] = _wrap_idx(il)
                blks[gi][:, o_ih:o_dr] = _wrap_idx(ih)
                blks[gi][:, o_dr:o_ax] = dr.view(np.int16)
                blks[gi][0:16, o_ax:o_ax + gsz * CH * 8] = _wrap16(ax0)
                blks[gi][32:48, o_ax:o_ax + gsz * CH * 8] = _wrap16(ax1)
            return blks

        out.append(dict(ep0=np.ascontiguousarray(pack(GROUPS0)),
                        ep1=np.ascontiguousarray(pack(GROUPS1))))
    return NL, NH, out


def _prep_inputs(x, edge_index, W0, a_src0, a_dst0, b0, W1, a_src1, a_dst1,
                 b1):
    src = np.asarray(edge_index[0]).astype(np.int64)
    dst = np.asarray(edge_index[1]).astype(np.int64)
    NL, NH, edata = _prep_edges(src, dst)

    def bd(a):  # [H, D] -> blockdiag [H*D, H]
        a = np.asarray(a, np.float32)
        H, D = a.shape
        m = np.zeros((H * D, H), np.float32)
        for h in range(H):
            m[h * D:(h + 1) * D, h] = a[h]
        return m

    W0 = np.asarray(W0, np.float32)
    W1 = np.asarray(W1, np.float32)
    W0a = np.concatenate([W0 @ bd(a_src0), W0 @ bd(a_dst0)], 1)  # [256, 16]
    # head-innermost feature interleave: new col d*8+h <- old col h*D+d
    perm0 = np.array([(f % 8) * 16 + f // 8 for f in range(128)])
    perm1 = np.array([(f % 8) * 64 + f // 8 for f in range(512)])
    W0cat = np.concatenate([W0[:, perm0], W0a], 1)               # [256, 144]
    W1a = np.concatenate([W1 @ bd(a_src1), W1 @ bd(a_dst1)], 1)  # [128, 16]
    W1cat = np.concatenate([W1[perm0][:, perm1], W1a[perm0]], 1)  # [128, 528]

    x = np.asarray(x, np.float32)
    ident = np.eye(128, dtype=np.float16)
    colio = np.tile(np.arange(128, dtype=np.float16)[None, :], (128, 1))
    b0b = np.tile(np.asarray(b0, np.float32)[None, :], (128, 1))
    b1b = np.tile(np.asarray(b1, np.float32)[None, :], (128, 1))

    in_maps = []
    for c in range(NCORES):
        rot = np.roll(np.arange(N), -c * NLOC)
        xr = np.zeros((GROWS, NFEAT), np.float16)
        xr[:N] = x[rot].astype(np.float16)
        xtt = xr.reshape(GROWS // 128, 128, 2, 128).transpose(0, 3, 2, 1)
        m = dict(
            xT=np.ascontiguousarray(xtt),
            W0=np.ascontiguousarray(
                W0cat.astype(np.float16).reshape(2, 128, NHID + 16)),
            W1cat=np.ascontiguousarray(W1cat.astype(np.float16)),
            b0b=np.ascontiguousarray(b0b[:, perm0]), b1b=b1b,
            ident=ident, colio=colio,
            **edata[c],
        )
        in_maps.append(m)
    return NL, NH, in_maps


# --------------------------------------------------------------------------
# device program
# --------------------------------------------------------------------------

def build(NL, NH, lt=LT, gt=GT, debug=False, phases="ABCDE"):
    CH = NL + NH
    HID16 = NHID + 16

    _, EPW0P = _epw(GROUPS0[0][1], NL, NH)
    _, EPW1P = _epw(GROUPS1[0][1], NL, NH)
    nc = bacc.Bacc("TRN2")
    xT = nc.dram_tensor("xT", [GROWS // 128, 128, 2, 128], F16,
                        kind="ExternalInput")
    W0i = nc.dram_tensor("W0", [2, 128, HID16], F16, kind="ExternalInput")
    W1cati = nc.dram_tensor("W1cat", [NHID, 528], F16, kind="ExternalInput")
    b0bi = nc.dram_tensor("b0b", [128, NHID], F32, kind="ExternalInput")
    b1bi = nc.dram_tensor("b1b", [128, NCLASS], F32, kind="ExternalInput")
    identi = nc.dram_tensor("ident", [128, 128], F16, kind="ExternalInput")
    colioi = nc.dram_tensor("colio", [128, 128], F16, kind="ExternalInput")
    ep0i = nc.dram_tensor("ep0", [len(GROUPS0), 128, EPW0P], I16,
                          kind="ExternalInput")
    ep1i = nc.dram_tensor("ep1", [len(GROUPS1), 128, EPW1P], I16,
                          kind="ExternalInput")
    out = nc.dram_tensor("out", [NLOC, NCLASS], F32, kind="ExternalOutput")

    with TileContext(nc) as tc, ExitStack() as stk:
        regs = {}
        for _, g in GROUPS0 + GROUPS1:
            for n in (g * NL * 128, g * NH * 128):
                if n not in regs:
                    regs[n] = nc.gpsimd.to_reg(n)
        dpool = stk.enter_context(
            tc.tile_pool(name="dram", bufs=1, space="DRAM"))
        t0lo = dpool.tile([SPLIT, T0W], F16, tag="t0lo")
        t0hi = dpool.tile([GROWS - SPLIT, T0W], F16, tag="t0hi")
        t1lo = dpool.tile([SPLIT, T1W], F16, tag="t1lo")
        t1hi = dpool.tile([GROWS - SPLIT, T1W], F16, tag="t1hi")
        agin = dpool.tile([128, NLOC], F16, tag="agin")
        agout = dpool.tile([NCORES * 128, NLOC], F16, tag="agout",
                           addr_space="Shared")

        cpool = stk.enter_context(tc.tile_pool(name="const", bufs=1))
        W0s = cpool.tile([128, 2, HID16], F16)
        nc.sync.dma_start(out=W0s[:], in_=W0i.rearrange("k p n -> p k n"))
        W1s = cpool.tile([128, 528], F16)
        nc.sync.dma_start(out=W1s[:], in_=W1cati[:])
        b0s = cpool.tile([128, NHID], F32)
        nc.sync.dma_start(out=b0s[:], in_=b0bi[:])
        b1s = cpool.tile([128, NCLASS], F32)
        nc.sync.dma_start(out=b1s[:], in_=b1bi[:])
        idents = cpool.tile([128, 128], F16)
        nc.sync.dma_start(out=idents[:], in_=identi[:])
        colios = cpool.tile([128, 128], F16)
        nc.sync.dma_start(out=colios[:], in_=colioi[:])
        ident64f = cpool.tile([64, 64], F32)
        nc.vector.tensor_copy(ident64f[:], idents[0:64, 0:64])
        adT0 = cpool.tile([64, ADTE], F32)
        nc.vector.memset(adT0[:], 0)
        adT1 = cpool.tile([64, ADTE], F32)
        nc.vector.memset(adT1[:], 0)

        def adt_store(adT, pt, g):
            hf = 1 if g * 128 >= HALF else 0
            nc.vector.tensor_copy(
                adT[32 * hf:32 * hf + 8,
                    g * 128 - HALF * hf:(g + 1) * 128 - HALF * hf], pt[:])

        # ---------------- phase A: layer-0 tables (replicated) ------------
        if "A" in phases:
            with ExitStack() as pa:
                xp = pa.enter_context(tc.tile_pool(name="pa_x", bufs=4))
                pp = pa.enter_context(
                    tc.tile_pool(name="pa_ps", bufs=2, space="PSUM"))
                rp = pa.enter_context(tc.tile_pool(name="pa_row", bufs=4))
                assert gt % 4 == 0
                for gg in range(gt // 4):
                    xa = xp.tile([128, 4, 2, 128], F16, tag="xa")
                    (nc.sync if gg % 2 else nc.gpsimd).dma_start(
                        out=xa[:],
                        in_=xT[4 * gg:4 * gg + 4].rearrange(
                            "g p k j -> p g k j"))
                    row = rp.tile([128, 4, T0W], F16, tag="row")
                    for g2 in range(4):
                        ps = pp.tile([128, HID16], F32, tag="ps")
                        for k in range(2):
                            nc.tensor.matmul(ps[:], xa[:, g2, k, :],
                                             W0s[:, k, :],
                                             start=(k == 0), stop=(k == 1))
                        if g2 % 2:
                            nc.scalar.copy(row[:, g2, 0:HID16], ps[:])
                        else:
                            nc.vector.tensor_copy(row[:, g2, 0:HID16],
                                                  ps[:])
                        g = 4 * gg + g2
                        if g < lt:
                            pt = pp.tile([8, 128], F16, tag="pt")
                            nc.tensor.transpose(
                                pt[:], row[:, g2, NHID + 8:HID16],
                                idents[:])
                            adt_store(adT0, pt, g)
                    eng = nc.scalar if gg % 2 else nc.sync
                    g0 = 4 * gg * 128
                    if g0 + 512 <= SPLIT:
                        eng.dma_start(
                            out=t0lo[g0:g0 + 512, :]
                            .rearrange("(g p) w -> p g w", p=128),
                            in_=row[:])
                    else:
                        o = g0 - SPLIT
                        eng.dma_start(
                            out=t0hi[o:o + 512, :]
                            .rearrange("(g p) w -> p g w", p=128),
                            in_=row[:])

        # ---------------- shared edge phase -------------------------------
        def edge_phase(layer, tbl_lo, tbl_hi, adT, fdim, trow, groups, epi,
                       post_fn, fin):
            fd8 = fdim + 8
            D = fdim // HEADS
            with ExitStack() as pb:
                ip = pb.enter_context(
                    tc.tile_pool(name=f"ix{layer}", bufs=2))
                gp = pb.enter_context(
                    tc.tile_pool(name=f"gg{layer}", bufs=2))
                apd = pb.enter_context(
                    tc.tile_pool(name=f"ga{layer}", bufs=2))
                rp2 = pb.enter_context(
                    tc.tile_pool(name=f"rh{layer}", bufs=3))
                pp2 = pb.enter_context(
                    tc.tile_pool(name=f"ps{layer}", bufs=2, space="PSUM"))
                pp3 = pb.enter_context(
                    tc.tile_pool(name=f"px{layer}", bufs=2, space="PSUM"))
                pp4 = pb.enter_context(
                    tc.tile_pool(name=f"pa{layer}", bufs=1, space="PSUM"))
                op = pb.enter_context(
                    tc.tile_pool(name=f"po{layer}", bufs=3))
                for gi, (t0g, gsz) in enumerate(groups):
                    o_ih = gsz * NL * 8
                    o_dr = gsz * (NL + NH) * 8
                    o_ax = o_dr + gsz * CH
                    nli, nhi = gsz * NL * 128, gsz * NH * 128
                    ep = ip.tile([128, epi.shape[2]], I16, tag="ep")
                    nc.sync.dma_start(out=ep[:], in_=epi[gi])
                    Glo = gp.tile([128, gsz * NL, trow], F16, tag="Glo")
                    nc.gpsimd.dma_gather(Glo[:], tbl_lo[:],
                                         ep[:, 0:o_ih], nli, regs[nli],
                                         trow)
                    Ghi = gp.tile([128, gsz * NH, trow], F16, tag="Ghi")
                    nc.gpsimd.dma_gather(Ghi[:], tbl_hi[:],
                                         ep[:, o_ih:o_dr], nhi, regs[nhi],
                                         trow)
                    for tl in range(gsz):
                        t = t0g + tl
                        admT = apd.tile([64, CH * 128], F32, tag="admT")
                        nc.gpsimd.ap_gather(
                            admT[:].unsqueeze(-1), adT[:].unsqueeze(-1),
                            ep[0:64, o_ax + tl * CH * 8:
                               o_ax + (tl + 1) * CH * 8],
                            channels=64, num_elems=ADTE, d=1,
                            num_idxs=CH * 128)
                        dr = ep[:, o_dr + tl * CH:
                                o_dr + (tl + 1) * CH].bitcast(F16)
                        admP = pp4.tile([128, CH * 64], F32, tag="admP")
                        for mm in range(CH):
                            nc.tensor.transpose(
                                admP[:, mm * 64:(mm + 1) * 64],
                                admT[:, mm * 128:(mm + 1) * 128],
                                ident64f[:])
                        adm = apd.tile([128, CH, 8], F16, tag="adm")
                        adml = apd.tile([128, CH, 8], F16, tag="adml")
                        admPv = admP[:].rearrange("p (c w) -> p c w", w=64)
                        nc.scalar.copy(adml[:], admPv[:, :, 0:8])
                        nc.vector.tensor_tensor(
                            out=adm[:], in0=adml[:],
                            in1=admPv[:, :, 32:40], op=mybir.AluOpType.add)

                        glo = Glo[:, tl * NL:(tl + 1) * NL, :]
                        ghi = Ghi[:, tl * NH:(tl + 1) * NH, :]
                        inc = rp2.tile([128, CH, 128], F16, tag="inc")
                        nc.vector.tensor_tensor(
                            out=inc[:],
                            in0=dr.unsqueeze(-1)
                            .broadcast_to([128, CH, 128]),
                            in1=colios[:].unsqueeze(1)
                            .broadcast_to([128, CH, 128]),
                            op=mybir.AluOpType.is_equal)
                        EX = op.tile([128, CH, 8], F16, tag="EX")
                        nc.vector.tensor_tensor(
                            out=EX[:, 0:NL, :],
                            in0=glo[:, :, fdim:fd8],
                            in1=adm[:, 0:NL, :], op=mybir.AluOpType.add)
                        nc.vector.tensor_tensor(
                            out=EX[:, NL:CH, :],
                            in0=ghi[:, :, fdim:fd8],
                            in1=adm[:, NL:CH, :], op=mybir.AluOpType.add)
                        nc.scalar.activation(
                            EX[:], EX[:],
                            mybir.ActivationFunctionType.Prelu, alpha=SLOPE)
                        nc.scalar.activation(
                            EX[:], EX[:], mybir.ActivationFunctionType.Exp)

                        split = fd8 > 512
                        rw = fdim if split else fd8
                        R2 = rp2.tile([128, CH, rw], F16, tag="R2")
                        if not split:
                            nc.scalar.copy(R2[:, :, fdim:fd8], EX[:])
                        nc.vector.tensor_tensor(
                            out=R2[:, 0:NL, 0:fdim]
                            .rearrange("p c (d h) -> p c d h", h=HEADS),
                            in0=glo[:, :, 0:fdim]
                            .rearrange("p c (d h) -> p c d h", h=HEADS),
                            in1=EX[:, 0:NL, :].unsqueeze(2)
                            .broadcast_to([128, NL, D, HEADS]),
                            op=mybir.AluOpType.mult)
                        nc.vector.tensor_tensor(
                            out=R2[:, NL:CH, 0:fdim]
                            .rearrange("p c (d h) -> p c d h", h=HEADS),
                            in0=ghi[:, :, 0:fdim]
                            .rearrange("p c (d h) -> p c d h", h=HEADS),
                            in1=EX[:, NL:CH, :].unsqueeze(2)
                            .broadcast_to([128, NH, D, HEADS]),
                            op=mybir.AluOpType.mult)

                        P1 = pp3.tile([128, rw], F32, tag="P1")
                        for ch in range(CH):
                            nc.tensor.matmul(P1[:], inc[:, ch, :],
                                             R2[:, ch, :],
                                             start=(ch == 0),
                                             stop=(ch == CH - 1))
                        if split:
                            P2 = pp2.tile([128, 8], F32, tag="P2")
                            for ch in range(CH):
                                nc.tensor.matmul(P2[:], inc[:, ch, :],
                                                 EX[:, ch, :],
                                                 start=(ch == 0),
                                                 stop=(ch == CH - 1))
                            post_fn(t, P1, P2, op, pp2, fin)
                        else:
                            post_fn(t, P1, P1, op, pp2, fin)

        # ---- L0 post: softmax-div, +b0, ELU, transpose, store ------------
        def post0(t, P1, P2, op, pp2, fin):
            rows = 128 if t < lt - 1 else LAST_ROWS
            r8 = op.tile([128, 8], F32, tag="r8")
            nc.vector.tensor_scalar_add(r8[:], P2[:, NHID:NHID + 8], 1e-16)
            nc.vector.reciprocal(r8[:], r8[:])
            z = op.tile([128, NHID], F32, tag="z")
            nc.vector.tensor_tensor(
                out=z[:].rearrange("p (d h) -> p d h", h=HEADS),
                in0=P1[:, 0:NHID].rearrange("p (d h) -> p d h", h=HEADS),
                in1=r8[:].unsqueeze(1).broadcast_to([128, 16, HEADS]),
                op=mybir.AluOpType.mult)
            nc.vector.tensor_tensor(out=z[:], in0=z[:], in1=b0s[:],
                                    op=mybir.AluOpType.add)
            zr = op.tile([128, NHID], F32, tag="zr")
            nc.scalar.activation(zr[:], z[:],
                                 mybir.ActivationFunctionType.Relu)
            zm = op.tile([128, NHID], F32, tag="zm")
            nc.vector.tensor_tensor(out=zm[:], in0=z[:], in1=zr[:],
                                    op=mybir.AluOpType.subtract)
            nc.scalar.activation(zm[:], zm[:],
                                 mybir.ActivationFunctionType.Exp)
            h1 = op.tile([128, NHID], F16, tag="h1")
            nc.vector.scalar_tensor_tensor(
                h1[:], zm[:], -1.0, zr[:],
                op0=mybir.AluOpType.add, op1=mybir.AluOpType.add)
            pst = pp2.tile([128, 128], F16, tag="pst")
            nc.tensor.transpose(pst[:], h1[:], idents[:])
            hT = op.tile([128, 128], F16, tag="hT")
            nc.scalar.copy(hT[:], pst[:])
            nc.sync.dma_start(
                out=agin[:, t * 128:t * 128 + rows], in_=hT[:, 0:rows])

        if "B" in phases:
            edge_phase(0, t0lo, t0hi, adT0, NHID, T0W, GROUPS0, ep0i,
                       post0, None)

        # ---------------- phase C: AllGather + rotation -------------------
        sregs = None
        if "C" in phases:
            nc.gpsimd.collective_compute(
                "AllGather", mybir.AluOpType.bypass,
                replica_groups=[list(range(NCORES))],
                ins=[agin[:]], outs=[agout[:]])
            pid = nc.partition_id(engines=[mybir.EngineType.SP])
            sregs = [nc.sync.snap(((j + pid) % NCORES) * 128)
                     for j in range(NCORES)]

        # ---------------- phase D: layer-1 tables -------------------------
        if "D" in phases and sregs is not None:
            with ExitStack() as pd:
                ngt = min(gt, (N + 127) // 128)
                spans = [(a, min(a + 8, ngt)) for a in range(0, ngt, 8)]
                xp1 = pd.enter_context(tc.tile_pool(name="pd_x", bufs=3))
                pp1 = pd.enter_context(
                    tc.tile_pool(name="pd_ps", bufs=2, space="PSUM"))
                ppA = pd.enter_context(
                    tc.tile_pool(name="pd_pa", bufs=1, space="PSUM"))
                rp1 = pd.enter_context(tc.tile_pool(name="pd_row", bufs=4))
                for si, (sa, sb) in enumerate(spans):
                    hx = xp1.tile([128, 8, 128], F16, tag="hx")
                    r0, r1 = sa * 128, min(sb * 128, N)
                    hxf = hx[:].rearrange("p g j -> p (g j)")
                    w0 = 0
                    r = r0
                    heng = nc.sync
                    while r < r1:
                        j = r // NLOC
                        seg = min(r1, (j + 1) * NLOC) - r
                        heng.dma_start(
                            out=hxf[:, w0:w0 + seg],
                            in_=agout[bass.ds(sregs[j % NCORES], 128),
                                      r - j * NLOC:r - j * NLOC + seg])
                        w0 += seg
                        r += seg
                    for ga in range(sa, sb, 2):
                        gb = min(ga + 2, sb)
                        nsub = gb - ga
                        row = rp1.tile([128, 2, T1W], F16, tag="row")
                        psH = pp1.tile([128, 2, 512], F32, tag="psH")
                        psA = ppA.tile([128, 2, 512], F32, tag="psA")
                        for g2 in range(nsub):
                            nc.tensor.matmul(psH[:, g2, :],
                                             hx[:, ga - sa + g2, :],
                                             W1s[:, 0:512],
                                             start=True, stop=True)
                            nc.tensor.matmul(psA[:, g2, 0:16],
                                             hx[:, ga - sa + g2, :],
                                             W1s[:, 512:528],
                                             start=True, stop=True)
                        nc.scalar.copy(row[:, 0:nsub, 0:176],
                                       psH[:, 0:nsub, 0:176])
                        nc.vector.tensor_copy(row[:, 0:nsub, 176:512],
                                              psH[:, 0:nsub, 176:512])
                        nc.vector.tensor_copy(row[:, 0:nsub, 512:528],
                                              psA[:, 0:nsub, 0:16])
                        eng = nc.scalar if ga % 4 else nc.sync
                        g0 = ga * 128
                        if nsub == 2 and g0 + 256 <= SPLIT:
                            eng.dma_start(
                                out=t1lo[g0:g0 + 256, 0:528]
                                .rearrange("(g p) w -> p g w", p=128),
                                in_=row[:, :, 0:528])
                        elif nsub == 2:
                            o = g0 - SPLIT
                            eng.dma_start(
                                out=t1hi[o:o + 256, 0:528]
                                .rearrange("(g p) w -> p g w", p=128),
                                in_=row[:, :, 0:528])
                        else:
                            o = g0 - SPLIT
                            eng.dma_start(out=t1hi[o:o + 128, 0:528],
                                          in_=row[:, 0, 0:528])
                        for g2 in range(nsub):
                            g = ga + g2
                            if g < lt:
                                pt = ppA.tile([8, 128], F16, tag="pt")
                                nc.tensor.transpose(
                                    pt[:], row[:, g2, 520:528], idents[:])
                                adt_store(adT1, pt, g)

        # ---------------- phase E: layer-1 edges + epilogue ---------------
        def post1(t, P1, P2, op, pp2, fin):
            zbig, nmxb, seb = fin
            r8 = op.tile([128, 8], F32, tag="r8")
            nc.vector.tensor_scalar_add(r8[:], P2[:, 0:8], 1e-16)
            nc.vector.reciprocal(r8[:], r8[:])
            nc.vector.tensor_scalar_mul(r8[:], r8[:], 1.0 / HEADS)
            zw = op.tile([128, 512], F32, tag="zw")
            nc.vector.tensor_tensor(
                out=zw[:].rearrange("p (d h) -> p d h", h=HEADS),
                in0=P1[:, 0:512].rearrange("p (d h) -> p d h", h=HEADS),
                in1=r8[:].unsqueeze(1).broadcast_to([128, 64, HEADS]),
                op=mybir.AluOpType.mult)
            z = zbig[:, t * NCLASS:(t + 1) * NCLASS]
            nc.vector.reduce_sum(
                z, zw[:].rearrange("p (d h) -> p d h", h=HEADS),
                axis=mybir.AxisListType.X)
            nc.vector.tensor_tensor(out=z, in0=z, in1=b1s[:],
                                    op=mybir.AluOpType.add)
            nmx = nmxb[:, t:t + 1]
            nc.vector.reduce_max(nmx, z, axis=mybir.AxisListType.X,
                                 negate=True)
            ez = op.tile([128, NCLASS], F32, tag="ez")
            nc.scalar.activation(ez[:], z,
                                 mybir.ActivationFunctionType.Exp,
                                 bias=nmx, accum_out=seb[:, t:t + 1])

        if "E" in phases:
            fpool = stk.enter_context(tc.tile_pool(name="fin", bufs=1))
            zbig = fpool.tile([128, lt * NCLASS], F32)
            nmxb = fpool.tile([128, lt], F32)
            seb = fpool.tile([128, lt], F32)
            edge_phase(1, t1lo, t1hi, adT1, 512, T1W, GROUPS1, ep1i,
                       post1, (zbig, nmxb, seb))
            # batched log-softmax tail: one Ln + two broadcast ops + 2 DMAs
            nc.scalar.activation(seb[:], seb[:],
                                 mybir.ActivationFunctionType.Ln)
            nc.vector.tensor_tensor(
                out=zbig[:].rearrange("p (t c) -> p t c", c=NCLASS),
                in0=zbig[:].rearrange("p (t c) -> p t c", c=NCLASS),
                in1=nmxb[:].unsqueeze(-1).broadcast_to([128, lt, NCLASS]),
                op=mybir.AluOpType.add)
            nc.vector.tensor_tensor(
                out=zbig[:].rearrange("p (t c) -> p t c", c=NCLASS),
                in0=zbig[:].rearrange("p (t c) -> p t c", c=NCLASS),
                in1=seb[:].unsqueeze(-1).broadcast_to([128, lt, NCLASS]),
                op=mybir.AluOpType.subtract)
            nfull = (lt - 1) * 128
            rlast = LAST_ROWS if lt == LT else 128
            nc.sync.dma_start(
                out=out[0:nfull, :].rearrange("(t p) c -> p t c", p=128),
                in_=zbig[:].rearrange("p (t c) -> p t c", c=NCLASS)
                [:, 0:lt - 1, :])
            nc.sync.dma_start(
                out=out[nfull:nfull + rlast, :],
                in_=zbig[0:rlast, (lt - 1) * NCLASS:lt * NCLASS])

    nc.compile()
    return nc


# --------------------------------------------------------------------------
# entry point
# --------------------------------------------------------------------------

def kernel(**inputs) -> np.ndarray:
    NLk, NHk, in_maps = _prep_inputs(**inputs)
    key = (NLk, NHk)
    if key not in _cache:
        _cache[key] = build(NLk, NHk)
    nc = _cache[key]
    res = run_bass_kernel_spmd(nc, in_maps, list(range(NCORES)))
    return np.concatenate([res.results[c]["out"] for c in range(NCORES)], 0)


# revision 25
# speedup vs baseline: 1.1106x; 1.0410x over previous
"""2-layer GAT (nn_GAT_31490700214331) on 8 Trainium2 NeuronCores.

Strategy (dst-sharded, SPMD, per-core-rotated node layout) — v2:
  - Nodes block-partitioned: core c owns nodes [c*6250, (c+1)*6250), every
    table uses the rotated row order (n - c*6250) mod 50000 so the SPMD
    program has no core-dependent offsets.
  - Layer-0 node features (h0|as0|ad0 = x @ W0cat) are computed replicated
    into a rotated DRAM table with 512B rows; layer-1 rows (h1|as1|ad1)
    are 1280B with 1056B content.  Edges are grouped by 128-dst tiles and
    source rows fetched with dma_gather through lo/hi table views (int16
    indices), batched over groups of tiles to amortize SWDGE overhead.
  - alpha_dst values are NOT dma-gathered per edge (256B/slot minimum).
    Instead each core keeps a transposed local table adT[16, 6272] (f32,
    built with PE transposes during phases A/D) and the per-slot lookup
    runs on gpsimd via ap_gather (indices in epack), then 16-wide PE
    transposes bring the result back slot-major through PSUM.
  - Edge softmax (safe without segment-max: |e| small) and the weighted
    aggregation are fused into per-chunk 128x128 incidence matmuls; the
    exp(e) values ride as 8 extra columns of the moving operand so a
    single PSUM accumulation produces numerator and denominator.
  - Between layers the ELU'd hidden state is AllGather'd (feature-major)
    and rotated into per-core order with partition-id-offset DMA reads.
  - alpha projections fold into the weight matmuls on the host:
    W0cat=[256,144] gives h0|as0|ad0; W1cat=[128,528] gives h1|as1|ad1.

Self-contained: call kernel(**inputs) with the full-problem arrays.
"""
import numpy as np
from contextlib import ExitStack

import concourse.bacc as bacc
import concourse.bass as bass
import concourse.mybir as mybir
from concourse.tile import TileContext
from concourse.bass_utils import run_bass_kernel_spmd

F16 = mybir.dt.float16
F8 = mybir.dt.float8e4
F32 = mybir.dt.float32
I16 = mybir.dt.int16

N = 50000
NFEAT = 256
NHID = 128
NCLASS = 64
HEADS = 8
SLOPE = 0.2
NCORES = 8
NLOC = N // NCORES           # 6250
LT = (NLOC + 127) // 128     # 49 local dst tiles
LAST_ROWS = NLOC - (LT - 1) * 128   # 106 rows in the last tile
GT = 392                     # global node tiles (392*128 = 50176)
GROWS = GT * 128
SPLIT = 25088                # low/high gather-table split (196 tiles)
SENT = 300.0                 # dst_rel sentinel for padding slots
T0W = 256                    # t0 row: [h0(128)|as0(8)|ad0(8)|junk] f16
T1W = 640                    # t1 row: [h1(512)|as1(8)|ad1(8)|junk] f16

GROUPS0 = [(7 * i, 7) for i in range(7)]                     # L0 tile groups
GROUPS1 = [(2 * i, 2) for i in range(24)] + [(48, 1)]        # L1 tile groups
HALF = 3200                  # adT split: dsts [0,3200) core0, rest core1
ADTE = HALF + 1              # num_elems per half (last col = zero slot)

_cache = {}


# --------------------------------------------------------------------------
# host-side preparation
# --------------------------------------------------------------------------

def _wrap_idx(idx):
    """[n] int -> [128, n//16] int16 wrapped gather-index layout."""
    n = idx.shape[0]
    assert n % 16 == 0
    w = idx.reshape(n // 16, 16).T.astype(np.int16)
    return np.tile(w, (8, 1))


def _wrap16(idx):
    """[n] int -> [16, n//16] int16 (single gpsimd core)."""
    n = idx.shape[0]
    assert n % 16 == 0
    return idx.reshape(n // 16, 16).T.astype(np.int16)


def _epw(gsz, NL, NH):
    CH = NL + NH
    w = gsz * (NL * 8 + NH * 8 + CH + CH * 8)
    return w, ((w + 15) // 16) * 16


def _prep_edges(src, dst):
    cores = []
    for c in range(NCORES):
        m = (dst >= c * NLOC) & (dst < (c + 1) * NLOC)
        s = src[m].astype(np.int64)
        d = dst[m].astype(np.int64) - c * NLOC
        order = np.argsort(d, kind="stable")
        s, d = s[order], d[order]
        s_rot = (s - c * NLOC) % N
        tiles = []
        for t in range(LT):
            sel = (d >= t * 128) & (d < (t + 1) * 128)
            st, dt = s_rot[sel], d[sel] - t * 128
            lo = st < SPLIT
            tiles.append((st[lo], dt[lo], st[~lo] - SPLIT, dt[~lo]))
        cores.append(tiles)
    nl = max(len(t[0]) for tl in cores for t in tl)
    nh = max(len(t[2]) for tl in cores for t in tl)
    NL = max(1, (nl + 127) // 128)
    NH = max(1, (nh + 127) // 128)
    assert NL * 128 <= 1024 and NH * 128 <= 1024, (NL, NH)
    CH = NL + NH

    out = []
    for c in range(NCORES):
        tiles = []
        for t in range(LT):
            sl, dl, sh, dh = cores[c][t]
            il = np.zeros(NL * 128, np.int64)
            il[: len(sl)] = sl
            ih = np.zeros(NH * 128, np.int64)
            ih[: len(sh)] = sh
            axg = np.zeros(CH * 128, np.int64)
            axg[: len(dl)] = t * 128 + dl
            axg[NL * 128: NL * 128 + len(dh)] = t * 128 + dh
            ax0 = np.where(axg < HALF, axg, HALF)          # core-0 indices
            ax1 = np.where(axg >= HALF, axg - HALF, HALF)  # core-1 indices
            rl = np.full(NL * 128, SENT)
            rl[: len(dl)] = dl
            rh = np.full(NH * 128, SENT)
            rh[: len(dh)] = dh
            dr = np.concatenate([rl, rh]).reshape(CH, 128).T  # [128, CH]
            tiles.append((il, ih, ax0, ax1, dr.astype(np.float16)))

        def pack(groups):
            _, epwp = _epw(groups[0][1], NL, NH)
            blks = np.zeros((len(groups), 128, epwp), np.int16)
            for gi, (t0, gsz) in enumerate(groups):
                o_ih = gsz * NL * 8
                o_dr = gsz * (NL + NH) * 8
                o_ax = o_dr + gsz * CH
                il = np.concatenate([tiles[t][0] for t in range(t0, t0 + gsz)])
                ih = np.concatenate([tiles[t][1] for t in range(t0, t0 + gsz)])
                ax0 = np.concatenate(
                    [tiles[t][2] for t in range(t0, t0 + gsz)])
                ax1 = np.concatenate(
                    [tiles[t][3] for t in range(t0, t0 + gsz)])
                dr = np.concatenate([tiles[t][4] for t in range(t0, t0 + gsz)],
                                    axis=1)
                blks[gi][:, 0:o_ih] = _wrap_idx(il)
                blks[gi][:, o_ih:o_dr] = _wrap_idx(ih)
                blks[gi][:, o_dr:o_ax] = dr.view(np.int16)
                blks[gi][0:16, o_ax:o_ax + gsz * CH * 8] = _wrap16(ax0)
                blks[gi][32:48, o_ax:o_ax + gsz * CH * 8] = _wrap16(ax1)
            return blks

        out.append(dict(ep0=np.ascontiguousarray(pack(GROUPS0)),
                        ep1=np.ascontiguousarray(pack(GROUPS1))))
    return NL, NH, out


def _prep_inputs(x, edge_index, W0, a_src0, a_dst0, b0, W1, a_src1, a_dst1,
                 b1):
    src = np.asarray(edge_index[0]).astype(np.int64)
    dst = np.asarray(edge_index[1]).astype(np.int64)
    NL, NH, edata = _prep_edges(src, dst)

    def bd(a):  # [H, D] -> blockdiag [H*D, H]
        a = np.asarray(a, np.float32)
        H, D = a.shape
        m = np.zeros((H * D, H), np.float32)
        for h in range(H):
            m[h * D:(h + 1) * D, h] = a[h]
        return m

    W0 = np.asarray(W0, np.float32)
    W1 = np.asarray(W1, np.float32)
    W0a = np.concatenate([W0 @ bd(a_src0), W0 @ bd(a_dst0)], 1)  # [256, 16]
    # head-innermost feature interleave: new col d*8+h <- old col h*D+d
    perm0 = np.array([(f % 8) * 16 + f // 8 for f in range(128)])
    perm1 = np.array([(f % 8) * 64 + f // 8 for f in range(512)])
    W0cat = np.concatenate([W0[:, perm0], W0a], 1)               # [256, 144]
    W1a = np.concatenate([W1 @ bd(a_src1), W1 @ bd(a_dst1)], 1)  # [128, 16]
    W1cat = np.concatenate([W1[perm0][:, perm1], W1a[perm0]], 1)  # [128, 528]

    x = np.asarray(x, np.float32)
    ident = np.eye(128, dtype=np.float16)
    colio = np.tile(np.arange(128, dtype=np.float16)[None, :], (128, 1))
    b0b = np.tile(np.asarray(b0, np.float32)[None, :], (128, 1))
    b1b = np.tile(np.asarray(b1, np.float32)[None, :], (128, 1))

    in_maps = []
    for c in range(NCORES):
        rot = np.roll(np.arange(N), -c * NLOC)
        xr = np.zeros((GROWS, NFEAT), np.float16)
        xr[:N] = x[rot].astype(np.float16)
        xtt = xr.reshape(GROWS // 128, 128, 2, 128).transpose(0, 3, 2, 1)
        m = dict(
            xT=np.ascontiguousarray(xtt),
            W0=np.ascontiguousarray(
                W0cat.astype(np.float16).reshape(2, 128, NHID + 16)),
            W1cat=np.ascontiguousarray(W1cat.astype(np.float16)),
            b0b=np.ascontiguousarray(b0b[:, perm0]), b1b=b1b,
            ident=ident, colio=colio,
            **edata[c],
        )
        in_maps.append(m)
    return NL, NH, in_maps


# --------------------------------------------------------------------------
# device program
# --------------------------------------------------------------------------

def build(NL, NH, lt=LT, gt=GT, debug=False, phases="ABCDE"):
    CH = NL + NH
    HID16 = NHID + 16

    _, EPW0P = _epw(GROUPS0[0][1], NL, NH)
    _, EPW1P = _epw(GROUPS1[0][1], NL, NH)
    nc = bacc.Bacc("TRN2")
    xT = nc.dram_tensor("xT", [GROWS // 128, 128, 2, 128], F16,
                        kind="ExternalInput")
    W0i = nc.dram_tensor("W0", [2, 128, HID16], F16, kind="ExternalInput")
    W1cati = nc.dram_tensor("W1cat", [NHID, 528], F16, kind="ExternalInput")
    b0bi = nc.dram_tensor("b0b", [128, NHID], F32, kind="ExternalInput")
    b1bi = nc.dram_tensor("b1b", [128, NCLASS], F32, kind="ExternalInput")
    identi = nc.dram_tensor("ident", [128, 128], F16, kind="ExternalInput")
    colioi = nc.dram_tensor("colio", [128, 128], F16, kind="ExternalInput")
    ep0i = nc.dram_tensor("ep0", [len(GROUPS0), 128, EPW0P], I16,
                          kind="ExternalInput")
    ep1i = nc.dram_tensor("ep1", [len(GROUPS1), 128, EPW1P], I16,
                          kind="ExternalInput")
    out = nc.dram_tensor("out", [NLOC, NCLASS], F32, kind="ExternalOutput")

    with TileContext(nc) as tc, ExitStack() as stk:
        regs = {}
        for _, g in GROUPS0 + GROUPS1:
            for n in (g * NL * 128, g * NH * 128):
                if n not in regs:
                    regs[n] = nc.gpsimd.to_reg(n)
        dpool = stk.enter_context(
            tc.tile_pool(name="dram", bufs=1, space="DRAM"))
        t0lo = dpool.tile([SPLIT, T0W], F16, tag="t0lo")
        t0hi = dpool.tile([GROWS - SPLIT, T0W], F16, tag="t0hi")
        t1lo = dpool.tile([SPLIT, T1W], F16, tag="t1lo")
        t1hi = dpool.tile([GROWS - SPLIT, T1W], F16, tag="t1hi")
        agin = dpool.tile([128, NLOC], F8, tag="agin")
        agout = dpool.tile([NCORES * 128, NLOC], F8, tag="agout",
                           addr_space="Shared")

        cpool = stk.enter_context(tc.tile_pool(name="const", bufs=1))
        W0s = cpool.tile([128, 2, HID16], F16)
        nc.sync.dma_start(out=W0s[:], in_=W0i.rearrange("k p n -> p k n"))
        W1s = cpool.tile([128, 528], F16)
        nc.sync.dma_start(out=W1s[:], in_=W1cati[:])
        b0s = cpool.tile([128, NHID], F32)
        nc.sync.dma_start(out=b0s[:], in_=b0bi[:])
        b1s = cpool.tile([128, NCLASS], F32)
        nc.sync.dma_start(out=b1s[:], in_=b1bi[:])
        idents = cpool.tile([128, 128], F16)
        nc.sync.dma_start(out=idents[:], in_=identi[:])
        colios = cpool.tile([128, 128], F16)
        nc.sync.dma_start(out=colios[:], in_=colioi[:])
        ident8s = cpool.tile([128, 128], F8)
        nc.vector.tensor_copy(ident8s[:], idents[:])
        ident64f = cpool.tile([64, 64], F32)
        nc.vector.tensor_copy(ident64f[:], idents[0:64, 0:64])
        adT0 = cpool.tile([64, ADTE], F32)
        nc.vector.memset(adT0[:], 0)
        adT1 = cpool.tile([64, ADTE], F32)
        nc.vector.memset(adT1[:], 0)

        def adt_store(adT, pt, g):
            hf = 1 if g * 128 >= HALF else 0
            nc.vector.tensor_copy(
                adT[32 * hf:32 * hf + 8,
                    g * 128 - HALF * hf:(g + 1) * 128 - HALF * hf], pt[:])

        # ---------------- phase A: layer-0 tables (replicated) ------------
        if "A" in phases:
            with ExitStack() as pa:
                xp = pa.enter_context(tc.tile_pool(name="pa_x", bufs=4))
                pp = pa.enter_context(
                    tc.tile_pool(name="pa_ps", bufs=2, space="PSUM"))
                rp = pa.enter_context(tc.tile_pool(name="pa_row", bufs=4))
                assert gt % 4 == 0
                for gg in range(gt // 4):
                    xa = xp.tile([128, 4, 2, 128], F16, tag="xa")
                    (nc.sync if gg % 2 else nc.gpsimd).dma_start(
                        out=xa[:],
                        in_=xT[4 * gg:4 * gg + 4].rearrange(
                            "g p k j -> p g k j"))
                    row = rp.tile([128, 4, T0W], F16, tag="row")
                    for g2 in range(4):
                        ps = pp.tile([128, HID16], F32, tag="ps")
                        for k in range(2):
                            nc.tensor.matmul(ps[:], xa[:, g2, k, :],
                                             W0s[:, k, :],
                                             start=(k == 0), stop=(k == 1))
                        if g2 % 2:
                            nc.scalar.copy(row[:, g2, 0:HID16], ps[:])
                        else:
                            nc.vector.tensor_copy(row[:, g2, 0:HID16],
                                                  ps[:])
                        g = 4 * gg + g2
                        if g < lt:
                            pt = pp.tile([8, 128], F16, tag="pt")
                            nc.tensor.transpose(
                                pt[:], row[:, g2, NHID + 8:HID16],
                                idents[:])
                            adt_store(adT0, pt, g)
                    eng = nc.scalar if gg % 2 else nc.sync
                    g0 = 4 * gg * 128
                    if g0 + 512 <= SPLIT:
                        eng.dma_start(
                            out=t0lo[g0:g0 + 512, :]
                            .rearrange("(g p) w -> p g w", p=128),
                            in_=row[:])
                    else:
                        o = g0 - SPLIT
                        eng.dma_start(
                            out=t0hi[o:o + 512, :]
                            .rearrange("(g p) w -> p g w", p=128),
                            in_=row[:])

        # ---------------- shared edge phase -------------------------------
        def edge_phase(layer, tbl_lo, tbl_hi, adT, fdim, trow, groups, epi,
                       post_fn, fin):
            fd8 = fdim + 8
            D = fdim // HEADS
            with ExitStack() as pb:
                ip = pb.enter_context(
                    tc.tile_pool(name=f"ix{layer}", bufs=2))
                gp = pb.enter_context(
                    tc.tile_pool(name=f"gg{layer}", bufs=2))
                apd = pb.enter_context(
                    tc.tile_pool(name=f"ga{layer}", bufs=2))
                rp2 = pb.enter_context(
                    tc.tile_pool(name=f"rh{layer}", bufs=3))
                pp2 = pb.enter_context(
                    tc.tile_pool(name=f"ps{layer}", bufs=2, space="PSUM"))
                pp3 = pb.enter_context(
                    tc.tile_pool(name=f"px{layer}", bufs=2, space="PSUM"))
                pp4 = pb.enter_context(
                    tc.tile_pool(name=f"pa{layer}", bufs=1, space="PSUM"))
                op = pb.enter_context(
                    tc.tile_pool(name=f"po{layer}", bufs=3))
                for gi, (t0g, gsz) in enumerate(groups):
                    o_ih = gsz * NL * 8
                    o_dr = gsz * (NL + NH) * 8
                    o_ax = o_dr + gsz * CH
                    nli, nhi = gsz * NL * 128, gsz * NH * 128
                    ep = ip.tile([128, epi.shape[2]], I16, tag="ep")
                    nc.sync.dma_start(out=ep[:], in_=epi[gi])
                    Glo = gp.tile([128, gsz * NL, trow], F16, tag="Glo")
                    nc.gpsimd.dma_gather(Glo[:], tbl_lo[:],
                                         ep[:, 0:o_ih], nli, regs[nli],
                                         trow)
                    Ghi = gp.tile([128, gsz * NH, trow], F16, tag="Ghi")
                    nc.gpsimd.dma_gather(Ghi[:], tbl_hi[:],
                                         ep[:, o_ih:o_dr], nhi, regs[nhi],
                                         trow)
                    for tl in range(gsz):
                        t = t0g + tl
                        admT = apd.tile([64, CH * 128], F32, tag="admT")
                        nc.gpsimd.ap_gather(
                            admT[:].unsqueeze(-1), adT[:].unsqueeze(-1),
                            ep[0:64, o_ax + tl * CH * 8:
                               o_ax + (tl + 1) * CH * 8],
                            channels=64, num_elems=ADTE, d=1,
                            num_idxs=CH * 128)
                        dr = ep[:, o_dr + tl * CH:
                                o_dr + (tl + 1) * CH].bitcast(F16)
                        admP = pp4.tile([128, CH * 64], F32, tag="admP")
                        for mm in range(CH):
                            nc.tensor.transpose(
                                admP[:, mm * 64:(mm + 1) * 64],
                                admT[:, mm * 128:(mm + 1) * 128],
                                ident64f[:])
                        adm = apd.tile([128, CH, 8], F16, tag="adm")
                        adml = apd.tile([128, CH, 8], F16, tag="adml")
                        admPv = admP[:].rearrange("p (c w) -> p c w", w=64)
                        nc.scalar.copy(adml[:], admPv[:, :, 0:8])
                        nc.vector.tensor_tensor(
                            out=adm[:], in0=adml[:],
                            in1=admPv[:, :, 32:40], op=mybir.AluOpType.add)

                        glo = Glo[:, tl * NL:(tl + 1) * NL, :]
                        ghi = Ghi[:, tl * NH:(tl + 1) * NH, :]
                        inc = rp2.tile([128, CH, 128], F16, tag="inc")
                        nc.vector.tensor_tensor(
                            out=inc[:],
                            in0=dr.unsqueeze(-1)
                            .broadcast_to([128, CH, 128]),
                            in1=colios[:].unsqueeze(1)
                            .broadcast_to([128, CH, 128]),
                            op=mybir.AluOpType.is_equal)
                        EX = op.tile([128, CH, 8], F16, tag="EX")
                        nc.vector.tensor_tensor(
                            out=EX[:, 0:NL, :],
                            in0=glo[:, :, fdim:fd8],
                            in1=adm[:, 0:NL, :], op=mybir.AluOpType.add)
                        nc.vector.tensor_tensor(
                            out=EX[:, NL:CH, :],
                            in0=ghi[:, :, fdim:fd8],
                            in1=adm[:, NL:CH, :], op=mybir.AluOpType.add)
                        nc.scalar.activation(
                            EX[:], EX[:],
                            mybir.ActivationFunctionType.Prelu, alpha=SLOPE)
                        nc.scalar.activation(
                            EX[:], EX[:], mybir.ActivationFunctionType.Exp)

                        split = fd8 > 512
                        rw = fdim if split else fd8
                        R2 = rp2.tile([128, CH, rw], F16, tag="R2")
                        if not split:
                            nc.scalar.copy(R2[:, :, fdim:fd8], EX[:])
                        nc.vector.tensor_tensor(
                            out=R2[:, 0:NL, 0:fdim]
                            .rearrange("p c (d h) -> p c d h", h=HEADS),
                            in0=glo[:, :, 0:fdim]
                            .rearrange("p c (d h) -> p c d h", h=HEADS),
                            in1=EX[:, 0:NL, :].unsqueeze(2)
                            .broadcast_to([128, NL, D, HEADS]),
                            op=mybir.AluOpType.mult)
                        nc.vector.tensor_tensor(
                            out=R2[:, NL:CH, 0:fdim]
                            .rearrange("p c (d h) -> p c d h", h=HEADS),
                            in0=ghi[:, :, 0:fdim]
                            .rearrange("p c (d h) -> p c d h", h=HEADS),
                            in1=EX[:, NL:CH, :].unsqueeze(2)
                            .broadcast_to([128, NH, D, HEADS]),
                            op=mybir.AluOpType.mult)

                        P1 = pp3.tile([128, rw], F32, tag="P1")
                        for ch in range(CH):
                            nc.tensor.matmul(P1[:], inc[:, ch, :],
                                             R2[:, ch, :],
                                             start=(ch == 0),
                                             stop=(ch == CH - 1))
                        if split:
                            P2 = pp2.tile([128, 8], F32, tag="P2")
                            for ch in range(CH):
                                nc.tensor.matmul(P2[:], inc[:, ch, :],
                                                 EX[:, ch, :],
                                                 start=(ch == 0),
                                                 stop=(ch == CH - 1))
                            post_fn(t, P1, P2, op, pp2, fin)
                        else:
                            post_fn(t, P1, P1, op, pp2, fin)

        # ---- L0 post: softmax-div, +b0, ELU, transpose, store ------------
        def post0(t, P1, P2, op, pp2, fin):
            rows = 128 if t < lt - 1 else LAST_ROWS
            r8 = op.tile([128, 8], F32, tag="r8")
            nc.vector.tensor_scalar_add(r8[:], P2[:, NHID:NHID + 8], 1e-16)
            nc.vector.reciprocal(r8[:], r8[:])
            z = op.tile([128, NHID], F32, tag="z")
            nc.vector.tensor_tensor(
                out=z[:].rearrange("p (d h) -> p d h", h=HEADS),
                in0=P1[:, 0:NHID].rearrange("p (d h) -> p d h", h=HEADS),
                in1=r8[:].unsqueeze(1).broadcast_to([128, 16, HEADS]),
                op=mybir.AluOpType.mult)
            nc.vector.tensor_tensor(out=z[:], in0=z[:], in1=b0s[:],
                                    op=mybir.AluOpType.add)
            zr = op.tile([128, NHID], F32, tag="zr")
            nc.scalar.activation(zr[:], z[:],
                                 mybir.ActivationFunctionType.Relu)
            zm = op.tile([128, NHID], F32, tag="zm")
            nc.vector.tensor_tensor(out=zm[:], in0=z[:], in1=zr[:],
                                    op=mybir.AluOpType.subtract)
            nc.scalar.activation(zm[:], zm[:],
                                 mybir.ActivationFunctionType.Exp)
            h1 = op.tile([128, NHID], F8, tag="h1")
            nc.vector.scalar_tensor_tensor(
                h1[:], zm[:], -1.0, zr[:],
                op0=mybir.AluOpType.add, op1=mybir.AluOpType.add)
            pst = pp2.tile([128, 128], F8, tag="pst")
            nc.tensor.transpose(pst[:], h1[:], ident8s[:])
            hT = op.tile([128, 128], F8, tag="hT")
            nc.scalar.copy(hT[:], pst[:])
            nc.sync.dma_start(
                out=agin[:, t * 128:t * 128 + rows], in_=hT[:, 0:rows])

        if "B" in phases:
            edge_phase(0, t0lo, t0hi, adT0, NHID, T0W, GROUPS0, ep0i,
                       post0, None)

        # ---------------- phase C: AllGather + rotation -------------------
        sregs = None
        if "C" in phases:
            nc.gpsimd.collective_compute(
                "AllGather", mybir.AluOpType.bypass,
                replica_groups=[list(range(NCORES))],
                ins=[agin[:]], outs=[agout[:]])
            pid = nc.partition_id(engines=[mybir.EngineType.SP])
            sregs = [nc.sync.snap(((j + pid) % NCORES) * 128)
                     for j in range(NCORES)]

        # ---------------- phase D: layer-1 tables -------------------------
        if "D" in phases and sregs is not None:
            with ExitStack() as pd:
                ngt = min(gt, (N + 127) // 128)
                spans = [(a, min(a + 8, ngt)) for a in range(0, ngt, 8)]
                xp1 = pd.enter_context(tc.tile_pool(name="pd_x", bufs=3))
                pp1 = pd.enter_context(
                    tc.tile_pool(name="pd_ps", bufs=2, space="PSUM"))
                ppA = pd.enter_context(
                    tc.tile_pool(name="pd_pa", bufs=1, space="PSUM"))
                rp1 = pd.enter_context(tc.tile_pool(name="pd_row", bufs=4))
                for si, (sa, sb) in enumerate(spans):
                    hx8 = xp1.tile([128, 8, 128], F8, tag="hx8")
                    hx = xp1.tile([128, 8, 128], F16, tag="hx")
                    r0, r1 = sa * 128, min(sb * 128, N)
                    hxf = hx8[:].rearrange("p g j -> p (g j)")
                    w0 = 0
                    r = r0
                    heng = nc.sync
                    while r < r1:
                        j = r // NLOC
                        seg = min(r1, (j + 1) * NLOC) - r
                        heng.dma_start(
                            out=hxf[:, w0:w0 + seg],
                            in_=agout[bass.ds(sregs[j % NCORES], 128),
                                      r - j * NLOC:r - j * NLOC + seg])
                        w0 += seg
                        r += seg
                    nc.scalar.copy(hx[:, 0:4, :], hx8[:, 0:4, :])
                    nc.vector.tensor_copy(hx[:, 4:8, :], hx8[:, 4:8, :])
                    for ga in range(sa, sb, 2):
                        gb = min(ga + 2, sb)
                        nsub = gb - ga
                        row = rp1.tile([128, 2, T1W], F16, tag="row")
                        psH = pp1.tile([128, 2, 512], F32, tag="psH")
                        psA = ppA.tile([128, 2, 512], F32, tag="psA")
                        for g2 in range(nsub):
                            nc.tensor.matmul(psH[:, g2, :],
                                             hx[:, ga - sa + g2, :],
                                             W1s[:, 0:512],
                                             start=True, stop=True)
                            nc.tensor.matmul(psA[:, g2, 0:16],
                                             hx[:, ga - sa + g2, :],
                                             W1s[:, 512:528],
                                             start=True, stop=True)
                        nc.scalar.copy(row[:, 0:nsub, 0:176],
                                       psH[:, 0:nsub, 0:176])
                        nc.vector.tensor_copy(row[:, 0:nsub, 176:512],
                                              psH[:, 0:nsub, 176:512])
                        nc.vector.tensor_copy(row[:, 0:nsub, 512:528],
                                              psA[:, 0:nsub, 0:16])
                        eng = nc.scalar if ga % 4 else nc.sync
                        g0 = ga * 128
                        if nsub == 2 and g0 + 256 <= SPLIT:
                            eng.dma_start(
                                out=t1lo[g0:g0 + 256, 0:528]
                                .rearrange("(g p) w -> p g w", p=128),
                                in_=row[:, :, 0:528])
                        elif nsub == 2:
                            o = g0 - SPLIT
                            eng.dma_start(
                                out=t1hi[o:o + 256, 0:528]
                                .rearrange("(g p) w -> p g w", p=128),
                                in_=row[:, :, 0:528])
                        else:
                            o = g0 - SPLIT
                            eng.dma_start(out=t1hi[o:o + 128, 0:528],
                                          in_=row[:, 0, 0:528])
                        for g2 in range(nsub):
                            g = ga + g2
                            if g < lt:
                                pt = ppA.tile([8, 128], F16, tag="pt")
                                nc.tensor.transpose(
                                    pt[:], row[:, g2, 520:528], idents[:])
                                adt_store(adT1, pt, g)

        # ---------------- phase E: layer-1 edges + epilogue ---------------
        def post1(t, P1, P2, op, pp2, fin):
            zbig, nmxb, seb = fin
            r8 = op.tile([128, 8], F32, tag="r8")
            nc.vector.tensor_scalar_add(r8[:], P2[:, 0:8], 1e-16)
            nc.vector.reciprocal(r8[:], r8[:])
            nc.vector.tensor_scalar_mul(r8[:], r8[:], 1.0 / HEADS)
            zw = op.tile([128, 512], F32, tag="zw")
            nc.vector.tensor_tensor(
                out=zw[:].rearrange("p (d h) -> p d h", h=HEADS),
                in0=P1[:, 0:512].rearrange("p (d h) -> p d h", h=HEADS),
                in1=r8[:].unsqueeze(1).broadcast_to([128, 64, HEADS]),
                op=mybir.AluOpType.mult)
            z = zbig[:, t * NCLASS:(t + 1) * NCLASS]
            nc.vector.reduce_sum(
                z, zw[:].rearrange("p (d h) -> p d h", h=HEADS),
                axis=mybir.AxisListType.X)
            nc.vector.tensor_tensor(out=z, in0=z, in1=b1s[:],
                                    op=mybir.AluOpType.add)
            nmx = nmxb[:, t:t + 1]
            nc.vector.reduce_max(nmx, z, axis=mybir.AxisListType.X,
                                 negate=True)
            ez = op.tile([128, NCLASS], F32, tag="ez")
            nc.scalar.activation(ez[:], z,
                                 mybir.ActivationFunctionType.Exp,
                                 bias=nmx, accum_out=seb[:, t:t + 1])

        if "E" in phases:
            fpool = stk.enter_context(tc.tile_pool(name="fin", bufs=1))
            zbig = fpool.tile([128, lt * NCLASS], F32)
            nmxb = fpool.tile([128, lt], F32)
            seb = fpool.tile([128, lt], F32)
            edge_phase(1, t1lo, t1hi, adT1, 512, T1W, GROUPS1, ep1i,
                       post1, (zbig, nmxb, seb))
            # batched log-softmax tail: one Ln + two broadcast ops + 2 DMAs
            nc.scalar.activation(seb[:], seb[:],
                                 mybir.ActivationFunctionType.Ln)
            nc.vector.tensor_tensor(
                out=zbig[:].rearrange("p (t c) -> p t c", c=NCLASS),
                in0=zbig[:].rearrange("p (t c) -> p t c", c=NCLASS),
                in1=nmxb[:].unsqueeze(-1).broadcast_to([128, lt, NCLASS]),
                op=mybir.AluOpType.add)
            nc.vector.tensor_tensor(
                out=zbig[:].rearrange("p (t c) -> p t c", c=NCLASS),
                in0=zbig[:].rearrange("p (t c) -> p t c", c=NCLASS),
                in1=seb[:].unsqueeze(-1).broadcast_to([128, lt, NCLASS]),
                op=mybir.AluOpType.subtract)
            nfull = (lt - 1) * 128
            rlast = LAST_ROWS if lt == LT else 128
            nc.sync.dma_start(
                out=out[0:nfull, :].rearrange("(t p) c -> p t c", p=128),
                in_=zbig[:].rearrange("p (t c) -> p t c", c=NCLASS)
                [:, 0:lt - 1, :])
            nc.sync.dma_start(
                out=out[nfull:nfull + rlast, :],
                in_=zbig[0:rlast, (lt - 1) * NCLASS:lt * NCLASS])

    nc.compile()
    return nc


# --------------------------------------------------------------------------
# entry point
# --------------------------------------------------------------------------

def kernel(**inputs) -> np.ndarray:
    NLk, NHk, in_maps = _prep_inputs(**inputs)
    key = (NLk, NHk)
    if key not in _cache:
        _cache[key] = build(NLk, NHk)
    nc = _cache[key]
    res = run_bass_kernel_spmd(nc, in_maps, list(range(NCORES)))
    return np.concatenate([res.results[c]["out"] for c in range(NCORES)], 0)


# revision 26
# speedup vs baseline: 1.1689x; 1.0525x over previous
"""2-layer GAT (nn_GAT_31490700214331) on 8 Trainium2 NeuronCores.

Strategy (dst-sharded, SPMD, per-core-rotated node layout) — v2:
  - Nodes block-partitioned: core c owns nodes [c*6250, (c+1)*6250), every
    table uses the rotated row order (n - c*6250) mod 50000 so the SPMD
    program has no core-dependent offsets.
  - Layer-0 node features (h0|as0|ad0 = x @ W0cat) are computed replicated
    into a rotated DRAM table with 512B rows; layer-1 rows (h1|as1|ad1)
    are 1280B with 1056B content.  Edges are grouped by 128-dst tiles and
    source rows fetched with dma_gather through lo/hi table views (int16
    indices), batched over groups of tiles to amortize SWDGE overhead.
  - alpha_dst values are NOT dma-gathered per edge (256B/slot minimum).
    Instead each core keeps a transposed local table adT[16, 6272] (f32,
    built with PE transposes during phases A/D) and the per-slot lookup
    runs on gpsimd via ap_gather (indices in epack), then 16-wide PE
    transposes bring the result back slot-major through PSUM.
  - Edge softmax (safe without segment-max: |e| small) and the weighted
    aggregation are fused into per-chunk 128x128 incidence matmuls; the
    exp(e) values ride as 8 extra columns of the moving operand so a
    single PSUM accumulation produces numerator and denominator.
  - Between layers the ELU'd hidden state is AllGather'd (feature-major)
    and rotated into per-core order with partition-id-offset DMA reads.
  - alpha projections fold into the weight matmuls on the host:
    W0cat=[256,144] gives h0|as0|ad0; W1cat=[128,528] gives h1|as1|ad1.

Self-contained: call kernel(**inputs) with the full-problem arrays.
"""
import numpy as np
from contextlib import ExitStack

import concourse.bacc as bacc
import concourse.bass as bass
import concourse.mybir as mybir
from concourse.tile import TileContext
from concourse.bass_utils import run_bass_kernel_spmd

F16 = mybir.dt.float16
F8 = mybir.dt.float8e4
F32 = mybir.dt.float32
I16 = mybir.dt.int16

N = 50000
NFEAT = 256
NHID = 128
NCLASS = 64
HEADS = 8
SLOPE = 0.2
NCORES = 8
NLOC = N // NCORES           # 6250
LT = (NLOC + 127) // 128     # 49 local dst tiles
LAST_ROWS = NLOC - (LT - 1) * 128   # 106 rows in the last tile
GT = 392                     # global node tiles (392*128 = 50176)
GROWS = GT * 128
SPLIT = 25088                # low/high gather-table split (196 tiles)
SENT = 300.0                 # dst_rel sentinel for padding slots
T0W = 256                    # t0 row: [h0(128)|as0(8)|ad0(8)|junk] f16
T1W = 640                    # t1 row: [h1(512)|as1(8)|ad1(8)|junk] f16

GROUPS0 = [(7 * i, 7) for i in range(7)]                     # L0 tile groups
GROUPS1 = [(2 * i, 2) for i in range(24)] + [(48, 1)]        # L1 tile groups
HALF = 3200                  # adT split: dsts [0,3200) core0, rest core1
ADTE = HALF + 1              # num_elems per half (last col = zero slot)

_cache = {}


# --------------------------------------------------------------------------
# host-side preparation
# --------------------------------------------------------------------------

def _wrap_idx(idx):
    """[n] int -> [128, n//16] int16 wrapped gather-index layout."""
    n = idx.shape[0]
    assert n % 16 == 0
    w = idx.reshape(n // 16, 16).T.astype(np.int16)
    return np.tile(w, (8, 1))


def _wrap16(idx):
    """[n] int -> [16, n//16] int16 (single gpsimd core)."""
    n = idx.shape[0]
    assert n % 16 == 0
    return idx.reshape(n // 16, 16).T.astype(np.int16)


def _epw(gsz, NL, NH):
    CH = NL + NH
    w = gsz * (NL * 8 + NH * 8 + CH + CH * 8)
    return w, ((w + 15) // 16) * 16


def _prep_edges(src, dst):
    cores = []
    for c in range(NCORES):
        m = (dst >= c * NLOC) & (dst < (c + 1) * NLOC)
        s = src[m].astype(np.int64)
        d = dst[m].astype(np.int64) - c * NLOC
        order = np.argsort(d, kind="stable")
        s, d = s[order], d[order]
        s_rot = (s - c * NLOC) % N
        tiles = []
        for t in range(LT):
            sel = (d >= t * 128) & (d < (t + 1) * 128)
            st, dt = s_rot[sel], d[sel] - t * 128
            lo = st < SPLIT
            tiles.append((st[lo], dt[lo], st[~lo] - SPLIT, dt[~lo]))
        cores.append(tiles)
    nl = max(len(t[0]) for tl in cores for t in tl)
    nh = max(len(t[2]) for tl in cores for t in tl)
    NL = max(1, (nl + 127) // 128)
    NH = max(1, (nh + 127) // 128)
    assert NL * 128 <= 1024 and NH * 128 <= 1024, (NL, NH)
    CH = NL + NH

    out = []
    for c in range(NCORES):
        tiles = []
        for t in range(LT):
            sl, dl, sh, dh = cores[c][t]
            il = np.zeros(NL * 128, np.int64)
            il[: len(sl)] = sl
            ih = np.zeros(NH * 128, np.int64)
            ih[: len(sh)] = sh
            axg = np.zeros(CH * 128, np.int64)
            axg[: len(dl)] = t * 128 + dl
            axg[NL * 128: NL * 128 + len(dh)] = t * 128 + dh
            ax0 = np.where(axg < HALF, axg, HALF)          # core-0 indices
            ax1 = np.where(axg >= HALF, axg - HALF, HALF)  # core-1 indices
            rl = np.full(NL * 128, SENT)
            rl[: len(dl)] = dl
            rh = np.full(NH * 128, SENT)
            rh[: len(dh)] = dh
            dr = np.concatenate([rl, rh]).reshape(CH, 128).T  # [128, CH]
            tiles.append((il, ih, ax0, ax1, dr.astype(np.float16)))

        def pack(groups):
            _, epwp = _epw(groups[0][1], NL, NH)
            blks = np.zeros((len(groups), 128, epwp), np.int16)
            for gi, (t0, gsz) in enumerate(groups):
                o_ih = gsz * NL * 8
                o_dr = gsz * (NL + NH) * 8
                o_ax = o_dr + gsz * CH
                il = np.concatenate([tiles[t][0] for t in range(t0, t0 + gsz)])
                ih = np.concatenate([tiles[t][1] for t in range(t0, t0 + gsz)])
                ax0 = np.concatenate(
                    [tiles[t][2] for t in range(t0, t0 + gsz)])
                ax1 = np.concatenate(
                    [tiles[t][3] for t in range(t0, t0 + gsz)])
                dr = np.concatenate([tiles[t][4] for t in range(t0, t0 + gsz)],
                                    axis=1)
                blks[gi][:, 0:o_ih] = _wrap_idx(il)
                blks[gi][:, o_ih:o_dr] = _wrap_idx(ih)
                blks[gi][:, o_dr:o_ax] = dr.view(np.int16)
                blks[gi][0:16, o_ax:o_ax + gsz * CH * 8] = _wrap16(ax0)
                blks[gi][32:48, o_ax:o_ax + gsz * CH * 8] = _wrap16(ax1)
            return blks

        out.append(dict(ep0=np.ascontiguousarray(pack(GROUPS0)),
                        ep1=np.ascontiguousarray(pack(GROUPS1))))
    return NL, NH, out


def _prep_inputs(x, edge_index, W0, a_src0, a_dst0, b0, W1, a_src1, a_dst1,
                 b1):
    src = np.asarray(edge_index[0]).astype(np.int64)
    dst = np.asarray(edge_index[1]).astype(np.int64)
    NL, NH, edata = _prep_edges(src, dst)

    def bd(a):  # [H, D] -> blockdiag [H*D, H]
        a = np.asarray(a, np.float32)
        H, D = a.shape
        m = np.zeros((H * D, H), np.float32)
        for h in range(H):
            m[h * D:(h + 1) * D, h] = a[h]
        return m

    W0 = np.asarray(W0, np.float32)
    W1 = np.asarray(W1, np.float32)
    W0a = np.concatenate([W0 @ bd(a_src0), W0 @ bd(a_dst0)], 1)  # [256, 16]
    # head-innermost feature interleave: new col d*8+h <- old col h*D+d
    perm0 = np.array([(f % 8) * 16 + f // 8 for f in range(128)])
    perm1 = np.array([(f % 8) * 64 + f // 8 for f in range(512)])
    W0cat = np.concatenate([W0[:, perm0], W0a], 1)               # [256, 144]
    W1a = np.concatenate([W1 @ bd(a_src1), W1 @ bd(a_dst1)], 1)  # [128, 16]
    W1cat = np.concatenate([W1[perm0][:, perm1], W1a[perm0]], 1)  # [128, 528]

    x = np.asarray(x, np.float32)
    ident = np.eye(128, dtype=np.float16)
    colio = np.tile(np.arange(128, dtype=np.float16)[None, :], (128, 1))
    b0b = np.tile(np.asarray(b0, np.float32)[None, :], (128, 1))
    b1b = np.tile(np.asarray(b1, np.float32)[None, :], (128, 1))

    in_maps = []
    for c in range(NCORES):
        rot = np.roll(np.arange(N), -c * NLOC)
        xr = np.zeros((GROWS, NFEAT), np.float16)
        xr[:N] = x[rot].astype(np.float16)
        xtt = xr.reshape(GROWS // 128, 128, 2, 128).transpose(0, 3, 2, 1)
        m = dict(
            xT=np.ascontiguousarray(xtt),
            W0=np.ascontiguousarray(
                W0cat.astype(np.float16).reshape(2, 128, NHID + 16)),
            W1cat=np.ascontiguousarray(W1cat.astype(np.float16)),
            b0b=np.ascontiguousarray(b0b[:, perm0]), b1b=b1b,
            ident=ident, colio=colio,
            **edata[c],
        )
        in_maps.append(m)
    return NL, NH, in_maps


# --------------------------------------------------------------------------
# device program
# --------------------------------------------------------------------------

def build(NL, NH, lt=LT, gt=GT, debug=False, phases="ABCDE"):
    CH = NL + NH
    HID16 = NHID + 16

    _, EPW0P = _epw(GROUPS0[0][1], NL, NH)
    _, EPW1P = _epw(GROUPS1[0][1], NL, NH)
    nc = bacc.Bacc("TRN2")
    xT = nc.dram_tensor("xT", [GROWS // 128, 128, 2, 128], F16,
                        kind="ExternalInput")
    W0i = nc.dram_tensor("W0", [2, 128, HID16], F16, kind="ExternalInput")
    W1cati = nc.dram_tensor("W1cat", [NHID, 528], F16, kind="ExternalInput")
    b0bi = nc.dram_tensor("b0b", [128, NHID], F32, kind="ExternalInput")
    b1bi = nc.dram_tensor("b1b", [128, NCLASS], F32, kind="ExternalInput")
    identi = nc.dram_tensor("ident", [128, 128], F16, kind="ExternalInput")
    colioi = nc.dram_tensor("colio", [128, 128], F16, kind="ExternalInput")
    ep0i = nc.dram_tensor("ep0", [len(GROUPS0), 128, EPW0P], I16,
                          kind="ExternalInput")
    ep1i = nc.dram_tensor("ep1", [len(GROUPS1), 128, EPW1P], I16,
                          kind="ExternalInput")
    out = nc.dram_tensor("out", [NLOC, NCLASS], F32, kind="ExternalOutput")

    with TileContext(nc) as tc, ExitStack() as stk:
        regs = {}
        for _, g in GROUPS0 + GROUPS1:
            for n in (g * NL * 128, g * NH * 128):
                if n not in regs:
                    regs[n] = nc.gpsimd.to_reg(n)
        dpool = stk.enter_context(
            tc.tile_pool(name="dram", bufs=1, space="DRAM"))
        t0lo = dpool.tile([SPLIT, T0W], F16, tag="t0lo")
        t0hi = dpool.tile([GROWS - SPLIT, T0W], F16, tag="t0hi")
        t1lo = dpool.tile([SPLIT, T1W], F16, tag="t1lo")
        t1hi = dpool.tile([GROWS - SPLIT, T1W], F16, tag="t1hi")
        agin = dpool.tile([128, NLOC], F8, tag="agin")
        agout = dpool.tile([NCORES * 128, NLOC], F8, tag="agout",
                           addr_space="Shared")

        cpool = stk.enter_context(tc.tile_pool(name="const", bufs=1))
        W0s = cpool.tile([128, 2, HID16], F16)
        nc.sync.dma_start(out=W0s[:], in_=W0i.rearrange("k p n -> p k n"))
        W1s = cpool.tile([128, 528], F16)
        nc.sync.dma_start(out=W1s[:], in_=W1cati[:])
        b0s = cpool.tile([128, NHID], F32)
        nc.sync.dma_start(out=b0s[:], in_=b0bi[:])
        b1s = cpool.tile([128, NCLASS], F32)
        nc.sync.dma_start(out=b1s[:], in_=b1bi[:])
        idents = cpool.tile([128, 128], F16)
        nc.sync.dma_start(out=idents[:], in_=identi[:])
        colios = cpool.tile([128, 128], F16)
        nc.sync.dma_start(out=colios[:], in_=colioi[:])
        ident64f = cpool.tile([64, 64], F32)
        nc.vector.tensor_copy(ident64f[:], idents[0:64, 0:64])
        adT0 = cpool.tile([64, ADTE], F32)
        nc.vector.memset(adT0[:], 0)
        adT1 = cpool.tile([64, ADTE], F32)
        nc.vector.memset(adT1[:], 0)

        def adt_store(adT, pt, g):
            hf = 1 if g * 128 >= HALF else 0
            nc.vector.tensor_copy(
                adT[32 * hf:32 * hf + 8,
                    g * 128 - HALF * hf:(g + 1) * 128 - HALF * hf], pt[:])

        # ---------------- phase A: layer-0 tables (replicated) ------------
        if "A" in phases:
            with ExitStack() as pa:
                xp = pa.enter_context(tc.tile_pool(name="pa_x", bufs=4))
                pp = pa.enter_context(
                    tc.tile_pool(name="pa_ps", bufs=2, space="PSUM"))
                rp = pa.enter_context(tc.tile_pool(name="pa_row", bufs=4))
                assert gt % 4 == 0
                for gg in range(gt // 4):
                    xa = xp.tile([128, 4, 2, 128], F16, tag="xa")
                    (nc.sync if gg % 2 else nc.gpsimd).dma_start(
                        out=xa[:],
                        in_=xT[4 * gg:4 * gg + 4].rearrange(
                            "g p k j -> p g k j"))
                    row = rp.tile([128, 4, T0W], F16, tag="row")
                    for g2 in range(4):
                        ps = pp.tile([128, HID16], F32, tag="ps")
                        for k in range(2):
                            nc.tensor.matmul(ps[:], xa[:, g2, k, :],
                                             W0s[:, k, :],
                                             start=(k == 0), stop=(k == 1))
                        if g2 % 2:
                            nc.scalar.copy(row[:, g2, 0:HID16], ps[:])
                        else:
                            nc.vector.tensor_copy(row[:, g2, 0:HID16],
                                                  ps[:])
                        g = 4 * gg + g2
                        if g < lt:
                            pt = pp.tile([8, 128], F16, tag="pt")
                            nc.tensor.transpose(
                                pt[:], row[:, g2, NHID + 8:HID16],
                                idents[:])
                            adt_store(adT0, pt, g)
                    eng = nc.scalar if gg % 2 else nc.sync
                    g0 = 4 * gg * 128
                    if g0 + 512 <= SPLIT:
                        eng.dma_start(
                            out=t0lo[g0:g0 + 512, :]
                            .rearrange("(g p) w -> p g w", p=128),
                            in_=row[:])
                    else:
                        o = g0 - SPLIT
                        eng.dma_start(
                            out=t0hi[o:o + 512, :]
                            .rearrange("(g p) w -> p g w", p=128),
                            in_=row[:])

        # ---------------- shared edge phase -------------------------------
        def edge_phase(layer, tbl_lo, tbl_hi, adT, fdim, trow, groups, epi,
                       post_fn, fin):
            fd8 = fdim + 8
            D = fdim // HEADS
            with ExitStack() as pb:
                ip = pb.enter_context(
                    tc.tile_pool(name=f"ix{layer}", bufs=2))
                gp = pb.enter_context(
                    tc.tile_pool(name=f"gg{layer}", bufs=2))
                apd = pb.enter_context(
                    tc.tile_pool(name=f"ga{layer}", bufs=2))
                rp2 = pb.enter_context(
                    tc.tile_pool(name=f"rh{layer}", bufs=3))
                pp2 = pb.enter_context(
                    tc.tile_pool(name=f"ps{layer}", bufs=2, space="PSUM"))
                pp3 = pb.enter_context(
                    tc.tile_pool(name=f"px{layer}", bufs=2, space="PSUM"))
                pp4 = pb.enter_context(
                    tc.tile_pool(name=f"pa{layer}", bufs=1, space="PSUM"))
                op = pb.enter_context(
                    tc.tile_pool(name=f"po{layer}", bufs=3))
                for gi, (t0g, gsz) in enumerate(groups):
                    o_ih = gsz * NL * 8
                    o_dr = gsz * (NL + NH) * 8
                    o_ax = o_dr + gsz * CH
                    nli, nhi = gsz * NL * 128, gsz * NH * 128
                    ep = ip.tile([128, epi.shape[2]], I16, tag="ep")
                    nc.sync.dma_start(out=ep[:], in_=epi[gi])
                    Glo = gp.tile([128, gsz * NL, trow], F16, tag="Glo")
                    nc.gpsimd.dma_gather(Glo[:], tbl_lo[:],
                                         ep[:, 0:o_ih], nli, regs[nli],
                                         trow)
                    Ghi = gp.tile([128, gsz * NH, trow], F16, tag="Ghi")
                    nc.gpsimd.dma_gather(Ghi[:], tbl_hi[:],
                                         ep[:, o_ih:o_dr], nhi, regs[nhi],
                                         trow)
                    for tl in range(gsz):
                        t = t0g + tl
                        admT = apd.tile([64, CH * 128], F32, tag="admT")
                        nc.gpsimd.ap_gather(
                            admT[:].unsqueeze(-1), adT[:].unsqueeze(-1),
                            ep[0:64, o_ax + tl * CH * 8:
                               o_ax + (tl + 1) * CH * 8],
                            channels=64, num_elems=ADTE, d=1,
                            num_idxs=CH * 128)
                        dr = ep[:, o_dr + tl * CH:
                                o_dr + (tl + 1) * CH].bitcast(F16)
                        admP = pp4.tile([128, CH * 64], F32, tag="admP")
                        for mm in range(CH):
                            nc.tensor.transpose(
                                admP[:, mm * 64:(mm + 1) * 64],
                                admT[:, mm * 128:(mm + 1) * 128],
                                ident64f[:])
                        adm = apd.tile([128, CH, 8], F16, tag="adm")
                        adml = apd.tile([128, CH, 8], F16, tag="adml")
                        admPv = admP[:].rearrange("p (c w) -> p c w", w=64)
                        nc.scalar.copy(adml[:], admPv[:, :, 0:8])
                        nc.vector.tensor_tensor(
                            out=adm[:], in0=adml[:],
                            in1=admPv[:, :, 32:40], op=mybir.AluOpType.add)

                        glo = Glo[:, tl * NL:(tl + 1) * NL, :]
                        ghi = Ghi[:, tl * NH:(tl + 1) * NH, :]
                        inc = rp2.tile([128, CH, 128], F16, tag="inc")
                        nc.vector.tensor_tensor(
                            out=inc[:],
                            in0=dr.unsqueeze(-1)
                            .broadcast_to([128, CH, 128]),
                            in1=colios[:].unsqueeze(1)
                            .broadcast_to([128, CH, 128]),
                            op=mybir.AluOpType.is_equal)
                        EX = op.tile([128, CH, 8], F16, tag="EX")
                        nc.vector.tensor_tensor(
                            out=EX[:, 0:NL, :],
                            in0=glo[:, :, fdim:fd8],
                            in1=adm[:, 0:NL, :], op=mybir.AluOpType.add)
                        nc.vector.tensor_tensor(
                            out=EX[:, NL:CH, :],
                            in0=ghi[:, :, fdim:fd8],
                            in1=adm[:, NL:CH, :], op=mybir.AluOpType.add)
                        nc.scalar.activation(
                            EX[:], EX[:],
                            mybir.ActivationFunctionType.Prelu, alpha=SLOPE)
                        nc.scalar.activation(
                            EX[:], EX[:], mybir.ActivationFunctionType.Exp)

                        split = fd8 > 512
                        rw = fdim if split else fd8
                        R2 = rp2.tile([128, CH, rw], F16, tag="R2")
                        if not split:
                            nc.scalar.copy(R2[:, :, fdim:fd8], EX[:])
                        nc.vector.tensor_tensor(
                            out=R2[:, 0:NL, 0:fdim]
                            .rearrange("p c (d h) -> p c d h", h=HEADS),
                            in0=glo[:, :, 0:fdim]
                            .rearrange("p c (d h) -> p c d h", h=HEADS),
                            in1=EX[:, 0:NL, :].unsqueeze(2)
                            .broadcast_to([128, NL, D, HEADS]),
                            op=mybir.AluOpType.mult)
                        nc.vector.tensor_tensor(
                            out=R2[:, NL:CH, 0:fdim]
                            .rearrange("p c (d h) -> p c d h", h=HEADS),
                            in0=ghi[:, :, 0:fdim]
                            .rearrange("p c (d h) -> p c d h", h=HEADS),
                            in1=EX[:, NL:CH, :].unsqueeze(2)
                            .broadcast_to([128, NH, D, HEADS]),
                            op=mybir.AluOpType.mult)

                        P1 = pp3.tile([128, rw], F32, tag="P1")
                        for ch in range(CH):
                            nc.tensor.matmul(P1[:], inc[:, ch, :],
                                             R2[:, ch, :],
                                             start=(ch == 0),
                                             stop=(ch == CH - 1))
                        if split:
                            P2 = pp2.tile([128, 8], F32, tag="P2")
                            for ch in range(CH):
                                nc.tensor.matmul(P2[:], inc[:, ch, :],
                                                 EX[:, ch, :],
                                                 start=(ch == 0),
                                                 stop=(ch == CH - 1))
                            post_fn(t, P1, P2, op, pp2, fin)
                        else:
                            post_fn(t, P1, P1, op, pp2, fin)

        # ---- L0 post: softmax-div, +b0, ELU, transpose, store ------------
        def post0(t, P1, P2, op, pp2, fin):
            rows = 128 if t < lt - 1 else LAST_ROWS
            r8 = op.tile([128, 8], F32, tag="r8")
            nc.vector.tensor_scalar_add(r8[:], P2[:, NHID:NHID + 8], 1e-16)
            nc.vector.reciprocal(r8[:], r8[:])
            z = op.tile([128, NHID], F32, tag="z")
            nc.vector.tensor_tensor(
                out=z[:].rearrange("p (d h) -> p d h", h=HEADS),
                in0=P1[:, 0:NHID].rearrange("p (d h) -> p d h", h=HEADS),
                in1=r8[:].unsqueeze(1).broadcast_to([128, 16, HEADS]),
                op=mybir.AluOpType.mult)
            nc.vector.tensor_tensor(out=z[:], in0=z[:], in1=b0s[:],
                                    op=mybir.AluOpType.add)
            zr = op.tile([128, NHID], F32, tag="zr")
            nc.scalar.activation(zr[:], z[:],
                                 mybir.ActivationFunctionType.Relu)
            zm = op.tile([128, NHID], F32, tag="zm")
            nc.vector.tensor_tensor(out=zm[:], in0=z[:], in1=zr[:],
                                    op=mybir.AluOpType.subtract)
            nc.scalar.activation(zm[:], zm[:],
                                 mybir.ActivationFunctionType.Exp)
            h1 = op.tile([128, NHID], F16, tag="h1")
            nc.vector.scalar_tensor_tensor(
                h1[:], zm[:], -1.0, zr[:],
                op0=mybir.AluOpType.add, op1=mybir.AluOpType.add)
            pst = pp2.tile([128, 128], F16, tag="pst")
            nc.tensor.transpose(pst[:], h1[:], idents[:])
            hT = op.tile([128, 128], F8, tag="hT")
            nc.scalar.copy(hT[:], pst[:])
            nc.sync.dma_start(
                out=agin[:, t * 128:t * 128 + rows], in_=hT[:, 0:rows])

        if "B" in phases:
            edge_phase(0, t0lo, t0hi, adT0, NHID, T0W, GROUPS0, ep0i,
                       post0, None)

        # ---------------- phase C: AllGather + rotation -------------------
        sregs = None
        if "C" in phases:
            nc.gpsimd.collective_compute(
                "AllGather", mybir.AluOpType.bypass,
                replica_groups=[list(range(NCORES))],
                ins=[agin[:]], outs=[agout[:]])
            pid = nc.partition_id(engines=[mybir.EngineType.SP])
            sregs = [nc.sync.snap(((j + pid) % NCORES) * 128)
                     for j in range(NCORES)]

        # ---------------- phase D: layer-1 tables -------------------------
        if "D" in phases and sregs is not None:
            with ExitStack() as pd:
                ngt = min(gt, (N + 127) // 128)
                spans = [(a, min(a + 8, ngt)) for a in range(0, ngt, 8)]
                xp1 = pd.enter_context(tc.tile_pool(name="pd_x", bufs=3))
                pp1 = pd.enter_context(
                    tc.tile_pool(name="pd_ps", bufs=2, space="PSUM"))
                ppA = pd.enter_context(
                    tc.tile_pool(name="pd_pa", bufs=1, space="PSUM"))
                rp1 = pd.enter_context(tc.tile_pool(name="pd_row", bufs=4))
                for si, (sa, sb) in enumerate(spans):
                    hx8 = xp1.tile([128, 8, 128], F8, tag="hx8")
                    hx = xp1.tile([128, 8, 128], F16, tag="hx")
                    r0, r1 = sa * 128, min(sb * 128, N)
                    hxf = hx8[:].rearrange("p g j -> p (g j)")
                    w0 = 0
                    r = r0
                    heng = nc.sync
                    while r < r1:
                        j = r // NLOC
                        seg = min(r1, (j + 1) * NLOC) - r
                        heng.dma_start(
                            out=hxf[:, w0:w0 + seg],
                            in_=agout[bass.ds(sregs[j % NCORES], 128),
                                      r - j * NLOC:r - j * NLOC + seg])
                        w0 += seg
                        r += seg
                    nc.scalar.copy(hx[:, 0:4, :], hx8[:, 0:4, :])
                    nc.vector.tensor_copy(hx[:, 4:8, :], hx8[:, 4:8, :])
                    for ga in range(sa, sb, 2):
                        gb = min(ga + 2, sb)
                        nsub = gb - ga
                        row = rp1.tile([128, 2, T1W], F16, tag="row")
                        psH = pp1.tile([128, 2, 512], F32, tag="psH")
                        psA = ppA.tile([128, 2, 512], F32, tag="psA")
                        for g2 in range(nsub):
                            nc.tensor.matmul(psH[:, g2, :],
                                             hx[:, ga - sa + g2, :],
                                             W1s[:, 0:512],
                                             start=True, stop=True)
                            nc.tensor.matmul(psA[:, g2, 0:16],
                                             hx[:, ga - sa + g2, :],
                                             W1s[:, 512:528],
                                             start=True, stop=True)
                        nc.scalar.copy(row[:, 0:nsub, 0:176],
                                       psH[:, 0:nsub, 0:176])
                        nc.vector.tensor_copy(row[:, 0:nsub, 176:512],
                                              psH[:, 0:nsub, 176:512])
                        nc.vector.tensor_copy(row[:, 0:nsub, 512:528],
                                              psA[:, 0:nsub, 0:16])
                        eng = nc.scalar if ga % 4 else nc.sync
                        g0 = ga * 128
                        if nsub == 2 and g0 + 256 <= SPLIT:
                            eng.dma_start(
                                out=t1lo[g0:g0 + 256, 0:528]
                                .rearrange("(g p) w -> p g w", p=128),
                                in_=row[:, :, 0:528])
                        elif nsub == 2:
                            o = g0 - SPLIT
                            eng.dma_start(
                                out=t1hi[o:o + 256, 0:528]
                                .rearrange("(g p) w -> p g w", p=128),
                                in_=row[:, :, 0:528])
                        else:
                            o = g0 - SPLIT
                            eng.dma_start(out=t1hi[o:o + 128, 0:528],
                                          in_=row[:, 0, 0:528])
                        for g2 in range(nsub):
                            g = ga + g2
                            if g < lt:
                                pt = ppA.tile([8, 128], F16, tag="pt")
                                nc.tensor.transpose(
                                    pt[:], row[:, g2, 520:528], idents[:])
                                adt_store(adT1, pt, g)

        # ---------------- phase E: layer-1 edges + epilogue ---------------
        def post1(t, P1, P2, op, pp2, fin):
            zbig, nmxb, seb = fin
            r8 = op.tile([128, 8], F32, tag="r8")
            nc.vector.tensor_scalar_add(r8[:], P2[:, 0:8], 1e-16)
            nc.vector.reciprocal(r8[:], r8[:])
            nc.vector.tensor_scalar_mul(r8[:], r8[:], 1.0 / HEADS)
            zw = op.tile([128, 512], F32, tag="zw")
            nc.vector.tensor_tensor(
                out=zw[:].rearrange("p (d h) -> p d h", h=HEADS),
                in0=P1[:, 0:512].rearrange("p (d h) -> p d h", h=HEADS),
                in1=r8[:].unsqueeze(1).broadcast_to([128, 64, HEADS]),
                op=mybir.AluOpType.mult)
            z = zbig[:, t * NCLASS:(t + 1) * NCLASS]
            nc.vector.reduce_sum(
                z, zw[:].rearrange("p (d h) -> p d h", h=HEADS),
                axis=mybir.AxisListType.X)
            nc.vector.tensor_tensor(out=z, in0=z, in1=b1s[:],
                                    op=mybir.AluOpType.add)
            nmx = nmxb[:, t:t + 1]
            nc.vector.reduce_max(nmx, z, axis=mybir.AxisListType.X,
                                 negate=True)
            ez = op.tile([128, NCLASS], F32, tag="ez")
            nc.scalar.activation(ez[:], z,
                                 mybir.ActivationFunctionType.Exp,
                                 bias=nmx, accum_out=seb[:, t:t + 1])

        if "E" in phases:
            fpool = stk.enter_context(tc.tile_pool(name="fin", bufs=1))
            zbig = fpool.tile([128, lt * NCLASS], F32)
            nmxb = fpool.tile([128, lt], F32)
            seb = fpool.tile([128, lt], F32)
            edge_phase(1, t1lo, t1hi, adT1, 512, T1W, GROUPS1, ep1i,
                       post1, (zbig, nmxb, seb))
            # batched log-softmax tail: one Ln + two broadcast ops + 2 DMAs
            nc.scalar.activation(seb[:], seb[:],
                                 mybir.ActivationFunctionType.Ln)
            nc.vector.tensor_tensor(
                out=zbig[:].rearrange("p (t c) -> p t c", c=NCLASS),
                in0=zbig[:].rearrange("p (t c) -> p t c", c=NCLASS),
                in1=nmxb[:].unsqueeze(-1).broadcast_to([128, lt, NCLASS]),
                op=mybir.AluOpType.add)
            nc.vector.tensor_tensor(
                out=zbig[:].rearrange("p (t c) -> p t c", c=NCLASS),
                in0=zbig[:].rearrange("p (t c) -> p t c", c=NCLASS),
                in1=seb[:].unsqueeze(-1).broadcast_to([128, lt, NCLASS]),
                op=mybir.AluOpType.subtract)
            nfull = (lt - 1) * 128
            rlast = LAST_ROWS if lt == LT else 128
            nc.sync.dma_start(
                out=out[0:nfull, :].rearrange("(t p) c -> p t c", p=128),
                in_=zbig[:].rearrange("p (t c) -> p t c", c=NCLASS)
                [:, 0:lt - 1, :])
            nc.sync.dma_start(
                out=out[nfull:nfull + rlast, :],
                in_=zbig[0:rlast, (lt - 1) * NCLASS:lt * NCLASS])

    nc.compile()
    return nc


# --------------------------------------------------------------------------
# entry point
# --------------------------------------------------------------------------

def kernel(**inputs) -> np.ndarray:
    NLk, NHk, in_maps = _prep_inputs(**inputs)
    key = (NLk, NHk)
    if key not in _cache:
        _cache[key] = build(NLk, NHk)
    nc = _cache[key]
    res = run_bass_kernel_spmd(nc, in_maps, list(range(NCORES)))
    return np.concatenate([res.results[c]["out"] for c in range(NCORES)], 0)


# revision 27
# speedup vs baseline: 1.1976x; 1.0245x over previous
"""2-layer GAT (nn_GAT_31490700214331) on 8 Trainium2 NeuronCores.

Strategy (dst-sharded, SPMD, per-core-rotated node layout):
  - Nodes are block-partitioned: core c owns nodes [c*6250, (c+1)*6250).
  - Every table on core c uses a ROTATED row order: node n lives at row
    (n - c*6250) mod 50000, so each core's own nodes are rows 0..6249 and
    the single SPMD program has no core-dependent offsets — the rotation
    lives entirely in host-prepared index/input arrays.
  - Layer-0 features (h0 = x @ W0) + attention alphas are computed
    replicated on every core (cheap) into a rotated DRAM table; edges are
    grouped by dst tile (128 dsts) and their source rows fetched with
    dma_gather (int16 indices -> the table is gathered through two views,
    rows [0, SPLIT) and [SPLIT, ...), keeping indices < 32768).
  - Edge softmax (safe without segment-max: |e| <= ~5) and the weighted
    aggregation are fused into per-chunk 128x128 incidence matmuls
    accumulating in PSUM; denominators ride along as 8 extra columns.
  - Between layers the ELU'd hidden state is AllGather'd (feature-major),
    rotated into per-core order with partition-id-offset DMA copies, and
    layer 1 repeats the scheme with 512-wide features and a head-mean +
    log_softmax epilogue.
  - alpha projections fold into the weight matmuls on the host:
    h @ blockdiag(a) == x @ (W @ blockdiag(a)), so the device gets
    W0a=[256,16] / W1a=[128,16] and computes alphas as 16 extra psum cols.

Self-contained: call kernel(**inputs) with the full-problem arrays.
"""
import numpy as np
from contextlib import ExitStack

import concourse.bacc as bacc
import concourse.bass as bass
import concourse.mybir as mybir
from concourse.tile import TileContext
from concourse.bass_utils import run_bass_kernel_spmd

F16 = mybir.dt.float16
F32 = mybir.dt.float32
F8 = mybir.dt.float8e4
I16 = mybir.dt.int16

N = 50000
NFEAT = 256
NHID = 128
NCLASS = 64
HEADS = 8
SLOPE = 0.2
NCORES = 8
NLOC = N // NCORES           # 6250
LT = (NLOC + 127) // 128     # 49 local dst tiles
LAST_ROWS = NLOC - (LT - 1) * 128   # 106 rows in the last tile
GT = 392                     # global node tiles (392*128 = 50176)
GROWS = GT * 128
SPLIT = 25088                # low/high gather-table split (196 tiles)
SENT = 300.0                 # dst_rel sentinel for padding slots
T0W = 256                    # t0 row: [h0(128)|as0(8)|ad0(8)|junk] f16
T1W = 640                    # t1 row: [h1(512)|as1(8)|ad1(8)|junk] f16

_cache = {}


# --------------------------------------------------------------------------
# host-side preparation
# --------------------------------------------------------------------------

def _wrap_idx(idx):
    """[n] int -> [128, n//16] int16 wrapped gather-index layout."""
    n = idx.shape[0]
    assert n % 16 == 0
    w = idx.reshape(n // 16, 16).T.astype(np.int16)
    return np.tile(w, (8, 1))


def _prep_edges(src, dst):
    cores = []
    for c in range(NCORES):
        m = (dst >= c * NLOC) & (dst < (c + 1) * NLOC)
        s = src[m].astype(np.int64)
        d = dst[m].astype(np.int64) - c * NLOC
        order = np.argsort(d, kind="stable")
        s, d = s[order], d[order]
        s_rot = (s - c * NLOC) % N
        tiles = []
        for t in range(LT):
            sel = (d >= t * 128) & (d < (t + 1) * 128)
            st, dt = s_rot[sel], d[sel] - t * 128
            lo = st < SPLIT
            tiles.append((st[lo], dt[lo], st[~lo] - SPLIT, dt[~lo]))
        cores.append(tiles)
    nl = max(len(t[0]) for tl in cores for t in tl)
    nh = max(len(t[2]) for tl in cores for t in tl)
    NL = max(1, (nl + 127) // 128)
    NH = max(1, (nh + 127) // 128)
    assert NL * 128 <= 1024 and NH * 128 <= 1024, (NL, NH)

    out = []
    for c in range(NCORES):
        eil = np.zeros((LT, 128, NL * 8), np.int16)
        eih = np.zeros((LT, 128, NH * 8), np.int16)
        eal = np.zeros((LT, 128, NL * 8), np.int16)
        eah = np.zeros((LT, 128, NH * 8), np.int16)
        drel = np.full((LT, 128, NL + NH), SENT, np.float16)
        for t in range(LT):
            sl, dl, sh, dh = cores[c][t]
            il = np.zeros(NL * 128, np.int64)
            il[: len(sl)] = sl
            al = np.zeros(NL * 128, np.int64)
            al[: len(dl)] = t * 128 + dl
            ih = np.zeros(NH * 128, np.int64)
            ih[: len(sh)] = sh
            ah = np.zeros(NH * 128, np.int64)
            ah[: len(dh)] = t * 128 + dh
            eil[t] = _wrap_idx(il)
            eih[t] = _wrap_idx(ih)
            eal[t] = _wrap_idx(al)
            eah[t] = _wrap_idx(ah)
            rl = np.full(NL * 128, SENT)
            rl[: len(dl)] = dl
            rh = np.full(NH * 128, SENT)
            rh[: len(dh)] = dh
            r = np.concatenate([rl, rh]).reshape(NL + NH, 128).T
            drel[t] = r.astype(np.float16)
        epack = np.concatenate(
            [eil, eih, eal, eah, drel.view(np.int16)], axis=2)
        out.append(dict(epack=np.ascontiguousarray(epack)))
    return NL, NH, out


def _prep_inputs(x, edge_index, W0, a_src0, a_dst0, b0, W1, a_src1, a_dst1,
                 b1):
    src = np.asarray(edge_index[0]).astype(np.int64)
    dst = np.asarray(edge_index[1]).astype(np.int64)
    NL, NH, edata = _prep_edges(src, dst)

    def bd(a):  # [H, D] -> blockdiag [H*D, H]
        a = np.asarray(a, np.float32)
        H, D = a.shape
        m = np.zeros((H * D, H), np.float32)
        for h in range(H):
            m[h * D:(h + 1) * D, h] = a[h]
        return m

    W0 = np.asarray(W0, np.float32)
    W1 = np.asarray(W1, np.float32)
    W0a = np.concatenate([W0 @ bd(a_src0), W0 @ bd(a_dst0)], 1)  # [256, 16]
    # head-innermost feature interleave: new col d*8+h <- old col h*D+d
    perm0 = np.array([(f % 8) * 16 + f // 8 for f in range(128)])
    perm1 = np.array([(f % 8) * 64 + f // 8 for f in range(512)])
    W0cat = np.concatenate([W0[:, perm0], W0a], 1)               # [256, 144]
    W1a = np.concatenate([W1 @ bd(a_src1), W1 @ bd(a_dst1)], 1)  # [128, 16]

    x = np.asarray(x, np.float32)
    ident = np.eye(128, dtype=np.float16)
    colio = np.tile(np.arange(128, dtype=np.float16)[None, :], (128, 1))
    b0b = np.tile(np.asarray(b0, np.float32)[None, :], (128, 1))
    b1b = np.tile(np.asarray(b1, np.float32)[None, :], (128, 1))

    in_maps = []
    for c in range(NCORES):
        rot = np.roll(np.arange(N), -c * NLOC)
        xr = np.zeros((GROWS, NFEAT), np.float16)
        xr[:N] = x[rot].astype(np.float16)
        xtt = xr.reshape(GROWS // 128, 128, 2, 128).transpose(0, 3, 2, 1)
        m = dict(
            xT=np.ascontiguousarray(xtt),
            W0=np.ascontiguousarray(
                W0cat.astype(np.float16).reshape(2, 128, NHID + 16)),
            W1=np.ascontiguousarray(W1[perm0][:, perm1].astype(np.float16)),
            W1a=np.ascontiguousarray(W1a[perm0].astype(np.float16)),
            b0b=np.ascontiguousarray(b0b[:, perm0]), b1b=b1b,
            ident=ident, colio=colio,
            **edata[c],
        )
        in_maps.append(m)
    return NL, NH, in_maps


# --------------------------------------------------------------------------
# device program
# --------------------------------------------------------------------------

def build(NL, NH, lt=LT, gt=GT, debug=False, phases="ABCDE"):
    CH = NL + NH
    HID16 = NHID + 16
    NLI = NL * 128
    NHI = NH * 128

    EPW = NL * 8 + NH * 8 + NL * 8 + NH * 8 + CH   # packed int16 cols
    nc = bacc.Bacc("TRN2")
    xT = nc.dram_tensor("xT", [GROWS // 128, 128, 2, 128], F16,
                        kind="ExternalInput")
    W0i = nc.dram_tensor("W0", [2, 128, NHID + 16], F16,
                         kind="ExternalInput")
    W1i = nc.dram_tensor("W1", [NHID, 512], F16, kind="ExternalInput")
    W1ai = nc.dram_tensor("W1a", [NHID, 16], F16, kind="ExternalInput")
    b0bi = nc.dram_tensor("b0b", [128, NHID], F32, kind="ExternalInput")
    b1bi = nc.dram_tensor("b1b", [128, NCLASS], F32, kind="ExternalInput")
    identi = nc.dram_tensor("ident", [128, 128], F16, kind="ExternalInput")
    colioi = nc.dram_tensor("colio", [128, 128], F16, kind="ExternalInput")
    epacki = nc.dram_tensor("epack", [lt, 128, EPW], I16,
                            kind="ExternalInput")
    out = nc.dram_tensor("out", [NLOC, NCLASS], F32, kind="ExternalOutput")
    dbg = None
    if debug:
        dbg = nc.dram_tensor("dbg", [gt * 128, T0W], F32,
                             kind="ExternalOutput")

    with TileContext(nc) as tc, ExitStack() as stk:
        reg_l = nc.gpsimd.to_reg(NLI)
        reg_h = nc.gpsimd.to_reg(NHI)
        dpool = stk.enter_context(
            tc.tile_pool(name="dram", bufs=1, space="DRAM"))
        t0lo = dpool.tile([SPLIT, T0W], F16, tag="t0lo")
        t0hi = dpool.tile([GROWS - SPLIT, T0W], F16, tag="t0hi")
        t0ad = dpool.tile([lt * 128, 128], F16, tag="t0ad")
        t1lo = dpool.tile([SPLIT, T1W], F16, tag="t1lo")
        t1hi = dpool.tile([GROWS - SPLIT, T1W], F16, tag="t1hi")
        t1ad = dpool.tile([lt * 128, 128], F16, tag="t1ad")
        agin = dpool.tile([128, NLOC], F8, tag="agin")
        agout = dpool.tile([NCORES * 128, NLOC], F8, tag="agout",
                           addr_space="Shared")

        cpool = stk.enter_context(tc.tile_pool(name="const", bufs=1))
        W0s = cpool.tile([128, 2, NHID + 16], F16)
        nc.sync.dma_start(out=W0s[:], in_=W0i.rearrange("k p n -> p k n"))
        W1s = cpool.tile([128, 512], F16)
        nc.sync.dma_start(out=W1s[:], in_=W1i[:])
        W1as = cpool.tile([128, 16], F16)
        nc.sync.dma_start(out=W1as[:], in_=W1ai[:])
        b0s = cpool.tile([128, NHID], F32)
        nc.sync.dma_start(out=b0s[:], in_=b0bi[:])
        b1s = cpool.tile([128, NCLASS], F32)
        nc.sync.dma_start(out=b1s[:], in_=b1bi[:])
        idents = cpool.tile([128, 128], F16)
        nc.sync.dma_start(out=idents[:], in_=identi[:])
        colios = cpool.tile([128, 128], F16)
        nc.sync.dma_start(out=colios[:], in_=colioi[:])
        zeros = cpool.tile([128, 128], F16)
        nc.vector.memset(zeros[:], 0)

        # ---------------- phase A: layer-0 tables (replicated) ------------
        with ExitStack() as pa:
            xp = pa.enter_context(tc.tile_pool(name="pa_x", bufs=4))
            pp = pa.enter_context(
                tc.tile_pool(name="pa_ps", bufs=2, space="PSUM"))
            rp = pa.enter_context(tc.tile_pool(name="pa_row", bufs=4))
            assert gt % 4 == 0
            for gg in range(gt // 4):
                xa = xp.tile([128, 4, 2, 128], F16, tag="xa")
                (nc.sync if gg % 2 else nc.scalar).dma_start(
                    out=xa[:],
                    in_=xT[4 * gg:4 * gg + 4].rearrange(
                        "g p k j -> p g k j"))
                row = rp.tile([128, 4, T0W], F16, tag="row")
                for g2 in range(4):
                    ps = pp.tile([128, HID16], F32, tag="ps")
                    for k in range(2):
                        nc.tensor.matmul(ps[:], xa[:, g2, k, :],
                                         W0s[:, k, :],
                                         start=(k == 0), stop=(k == 1))
                    if g2 % 2:
                        nc.scalar.copy(row[:, g2, 0:HID16], ps[:])
                    else:
                        nc.vector.tensor_copy(row[:, g2, 0:HID16], ps[:])
                    g = 4 * gg + g2
                    if g < lt:
                        adr = rp.tile([128, 8], F16, tag="adr")
                        nc.vector.tensor_copy(
                            adr[:], ps[:, NHID + 8:HID16])
                        nc.sync.dma_start(
                            out=t0ad[g * 128:(g + 1) * 128, 0:8],
                            in_=adr[:])
                eng = nc.scalar if gg % 2 else nc.sync
                g0 = 4 * gg * 128
                if g0 + 512 <= SPLIT:
                    eng.dma_start(
                        out=t0lo[g0:g0 + 512, :]
                        .rearrange("(g p) w -> p g w", p=128),
                        in_=row[:])
                else:
                    o = g0 - SPLIT
                    eng.dma_start(
                        out=t0hi[o:o + 512, :]
                        .rearrange("(g p) w -> p g w", p=128),
                        in_=row[:])

        # ---------------- shared edge phase -------------------------------
        def edge_phase(layer, tbl_lo, tbl_hi, tblad, fdim, trow, rw,
                       post_fn, fin):
            o_il, o_ih = 0, NL * 8
            o_al, o_ah = NL * 16, NL * 16 + NH * 8
            o_dr = NL * 16 + NH * 16
            with ExitStack() as pb:
                ip = pb.enter_context(
                    tc.tile_pool(name=f"ix{layer}", bufs=4))
                gp = pb.enter_context(
                    tc.tile_pool(name=f"gg{layer}", bufs=4))
                apl = pb.enter_context(
                    tc.tile_pool(name=f"ga{layer}", bufs=3))
                rp2 = pb.enter_context(
                    tc.tile_pool(name=f"rh{layer}", bufs=3))
                pp2 = pb.enter_context(
                    tc.tile_pool(name=f"ps{layer}", bufs=2, space="PSUM"))
                op = pb.enter_context(
                    tc.tile_pool(name=f"po{layer}", bufs=3))
                for t in range(lt):
                    ep = ip.tile([128, EPW], I16, tag="ep")
                    nc.sync.dma_start(out=ep[:], in_=epacki[t])
                    il = ep[:, o_il:o_il + NL * 8]
                    ih = ep[:, o_ih:o_ih + NH * 8]
                    al = ep[:, o_al:o_al + NL * 8]
                    ah = ep[:, o_ah:o_ah + NH * 8]
                    dr = ep[:, o_dr:o_dr + CH].bitcast(F16)

                    G = gp.tile([128, CH, trow], F16, tag="G")
                    nc.gpsimd.dma_gather(G[:, 0:NL, :], tbl_lo[:], il,
                                         NLI, reg_l, trow)
                    nc.gpsimd.dma_gather(G[:, NL:CH, :], tbl_hi[:],
                                         ih, NHI, reg_h, trow)
                    A = apl.tile([128, CH, 128], F16, tag="A")
                    nc.gpsimd.dma_gather(A[:, 0:NL, :], tblad[:], al,
                                         NLI, reg_l, 128)
                    nc.gpsimd.dma_gather(A[:, NL:CH, :], tblad[:], ah,
                                         NHI, reg_h, 128)

                    inc = rp2.tile([128, CH, 128], F16, tag="inc")
                    nc.vector.tensor_tensor(
                        out=inc[:],
                        in0=dr.unsqueeze(-1).broadcast_to([128, CH, 128]),
                        in1=colios[:].unsqueeze(1)
                        .broadcast_to([128, CH, 128]),
                        op=mybir.AluOpType.is_equal)
                    EX = rp2.tile([128, CH, 8], F16, tag="EX")
                    nc.vector.tensor_tensor(
                        out=EX[:], in0=G[:, :, fdim:fdim + 8],
                        in1=A[:, :, 0:8], op=mybir.AluOpType.add)
                    nc.scalar.activation(
                        EX[:], EX[:], mybir.ActivationFunctionType.Prelu,
                        alpha=SLOPE)
                    nc.scalar.activation(
                        EX[:], EX[:], mybir.ActivationFunctionType.Exp)

                    R = rp2.tile([128, CH, fdim], F16, tag="R")
                    H = HEADS
                    D = fdim // H
                    nc.vector.tensor_tensor(
                        out=R[:, :, 0:fdim]
                        .rearrange("p c (d h) -> p c d h", h=H),
                        in0=G[:, :, 0:fdim]
                        .rearrange("p c (d h) -> p c d h", h=H),
                        in1=EX[:].unsqueeze(2).broadcast_to([128, CH, D, H]),
                        op=mybir.AluOpType.mult)

                    P1 = pp2.tile([128, fdim], F32, tag="P1")
                    P2 = pp2.tile([128, 8], F32, tag="P2")
                    for ch in range(CH):
                        nc.tensor.matmul(P1[:], inc[:, ch, :],
                                         R[:, ch, 0:fdim],
                                         start=(ch == 0),
                                         stop=(ch == CH - 1))
                    for ch in range(CH):
                        nc.tensor.matmul(P2[:], inc[:, ch, :],
                                         EX[:, ch, :],
                                         start=(ch == 0),
                                         stop=(ch == CH - 1))
                    post_fn(t, P1, P2, op, pp2, fin)

        # ---- L0 post: softmax-div, +b0, ELU, transpose, store ------------
        def post0(t, P1, P2, op, pp2, fin):
            rows = 128 if t < lt - 1 else LAST_ROWS
            r8 = op.tile([128, 8], F32, tag="r8")
            nc.vector.tensor_scalar_add(r8[:], P2[:], 1e-16)
            nc.vector.reciprocal(r8[:], r8[:])
            z = op.tile([128, NHID], F32, tag="z")
            nc.vector.tensor_tensor(
                out=z[:].rearrange("p (d h) -> p d h", h=HEADS),
                in0=P1[:].rearrange("p (d h) -> p d h", h=HEADS),
                in1=r8[:].unsqueeze(1).broadcast_to([128, 16, HEADS]),
                op=mybir.AluOpType.mult)
            nc.vector.tensor_tensor(out=z[:], in0=z[:], in1=b0s[:],
                                    op=mybir.AluOpType.add)
            zr = op.tile([128, NHID], F32, tag="zr")
            nc.scalar.activation(zr[:], z[:],
                                 mybir.ActivationFunctionType.Relu)
            zm = op.tile([128, NHID], F32, tag="zm")
            nc.vector.tensor_tensor(out=zm[:], in0=z[:], in1=zr[:],
                                    op=mybir.AluOpType.subtract)
            nc.scalar.activation(zm[:], zm[:],
                                 mybir.ActivationFunctionType.Exp)
            h1 = op.tile([128, NHID], F16, tag="h1")
            nc.vector.scalar_tensor_tensor(
                h1[:], zm[:], -1.0, zr[:],
                op0=mybir.AluOpType.add, op1=mybir.AluOpType.add)
            pst = pp2.tile([128, 128], F16, tag="pst")
            nc.tensor.transpose(pst[:], h1[:], idents[:])
            hT = op.tile([128, 128], F8, tag="hT")
            nc.scalar.copy(hT[:], pst[:])
            nc.sync.dma_start(
                out=agin[:, t * 128:t * 128 + rows], in_=hT[:, 0:rows])

        if "B" in phases:
            edge_phase(0, t0lo, t0hi, t0ad, NHID, T0W, 8 + NHID, post0, None)

        # ---------------- phase C: AllGather + rotation -------------------
        sregs = None
        if "C" in phases:
            nc.gpsimd.collective_compute(
                "AllGather", mybir.AluOpType.bypass,
                replica_groups=[list(range(NCORES))],
                ins=[agin[:]], outs=[agout[:]])
            pid = nc.partition_id(engines=[mybir.EngineType.SP])
            sregs = [nc.sync.snap(((j + pid) % NCORES) * 128)
                     for j in range(NCORES)]

        # ---------------- phase D: layer-1 tables -------------------------
        with ExitStack() as pd:
            if "D" not in phases:
                pd.enter_context(ExitStack())  # keep structure
            ngt = min(gt, (N + 127) // 128)
            dsup = [(a, min(a + 2, ngt)) for a in range(0, ngt, 2)]
            if "D" not in phases:
                dsup = []
            xp1 = pd.enter_context(tc.tile_pool(name="pd_x", bufs=4))
            pp1 = pd.enter_context(
                tc.tile_pool(name="pd_ps", bufs=2, space="PSUM"))
            rp1 = pd.enter_context(tc.tile_pool(name="pd_row", bufs=4))
            for ga, gb in dsup:
                nsub = gb - ga
                hx8 = xp1.tile([128, 2, 128], F8, tag="hx8")
                hx = xp1.tile([128, 2, 128], F16, tag="hx")
                r0, r1 = ga * 128, min(gb * 128, N)
                hxf = hx8[:].rearrange("p g j -> p (g j)")
                w0 = 0
                r = r0
                while r < r1:
                    j = r // NLOC
                    seg = min(r1, (j + 1) * NLOC) - r
                    nc.sync.dma_start(
                        out=hxf[:, w0:w0 + seg],
                        in_=agout[bass.ds(sregs[j % NCORES], 128),
                                  r - j * NLOC:r - j * NLOC + seg])
                    w0 += seg
                    r += seg
                nc.scalar.copy(hx[:, 0, :], hx8[:, 0, :])
                nc.vector.tensor_copy(hx[:, 1, :], hx8[:, 1, :])
                row = rp1.tile([128, 2, T1W], F16, tag="row")
                psas = []
                for g2 in range(nsub):
                    ps = pp1.tile([128, 512], F32, tag=f"ps{g2}")
                    nc.tensor.matmul(ps[:], hx[:, g2, :], W1s[:],
                                     start=True, stop=True)
                    psa = pp1.tile([128, 16], F32, tag=f"psa{g2}")
                    nc.tensor.matmul(psa[:], hx[:, g2, :], W1as[:],
                                     start=True, stop=True)
                    nc.scalar.copy(row[:, g2, 0:176], ps[:, 0:176])
                    nc.vector.tensor_copy(row[:, g2, 176:512],
                                          ps[:, 176:512])
                    nc.vector.tensor_copy(row[:, g2, 512:528], psa[:])
                    psas.append(psa)
                eng = nc.scalar if ga % 4 else nc.sync
                g0 = ga * 128
                if nsub == 2 and g0 + 256 <= SPLIT:
                    eng.dma_start(
                        out=t1lo[g0:g0 + 256, 0:528]
                        .rearrange("(g p) w -> p g w", p=128),
                        in_=row[:, :, 0:528])
                elif nsub == 2:
                    o = g0 - SPLIT
                    eng.dma_start(
                        out=t1hi[o:o + 256, 0:528]
                        .rearrange("(g p) w -> p g w", p=128),
                        in_=row[:, :, 0:528])
                else:
                    o = g0 - SPLIT
                    eng.dma_start(out=t1hi[o:o + 128, 0:528],
                                  in_=row[:, 0, 0:528])
                for g2 in range(nsub):
                    g = ga + g2
                    if g < lt:
                        adr = rp1.tile([128, 8], F16, tag="adr")
                        nc.vector.tensor_copy(adr[:], psas[g2][:, 8:16])
                        nc.sync.dma_start(
                            out=t1ad[g * 128:(g + 1) * 128, 0:8],
                            in_=adr[:])

        # ---------------- phase E: layer-1 edges + epilogue ---------------
        def post1(t, P1, P2, op, pp2, fin):
            zbig, nmxb, seb = fin
            r8 = op.tile([128, 8], F32, tag="r8")
            nc.vector.tensor_scalar_add(r8[:], P2[:], 1e-16)
            nc.vector.reciprocal(r8[:], r8[:])
            nc.vector.tensor_scalar_mul(r8[:], r8[:], 1.0 / HEADS)
            zw = op.tile([128, 512], F32, tag="zw")
            nc.vector.tensor_tensor(
                out=zw[:].rearrange("p (d h) -> p d h", h=HEADS),
                in0=P1[:].rearrange("p (d h) -> p d h", h=HEADS),
                in1=r8[:].unsqueeze(1).broadcast_to([128, 64, HEADS]),
                op=mybir.AluOpType.mult)
            z = zbig[:, t * NCLASS:(t + 1) * NCLASS]
            nc.vector.reduce_sum(
                z, zw[:].rearrange("p (d h) -> p d h", h=HEADS),
                axis=mybir.AxisListType.X)
            nc.vector.tensor_tensor(out=z, in0=z, in1=b1s[:],
                                    op=mybir.AluOpType.add)
            nmx = nmxb[:, t:t + 1]
            nc.vector.reduce_max(nmx, z, axis=mybir.AxisListType.X,
                                 negate=True)
            ez = op.tile([128, NCLASS], F32, tag="ez")
            nc.scalar.activation(ez[:], z,
                                 mybir.ActivationFunctionType.Exp,
                                 bias=nmx, accum_out=seb[:, t:t + 1])

        if "E" in phases:
            fpool = stk.enter_context(tc.tile_pool(name="fin", bufs=1))
            zbig = fpool.tile([128, lt * NCLASS], F32)
            nmxb = fpool.tile([128, lt], F32)
            seb = fpool.tile([128, lt], F32)
            edge_phase(1, t1lo, t1hi, t1ad, 512, T1W, 520, post1,
                       (zbig, nmxb, seb))
            # batched log-softmax tail: one Ln + two broadcast ops + 2 DMAs
            nc.scalar.activation(seb[:], seb[:],
                                 mybir.ActivationFunctionType.Ln)
            nc.vector.tensor_tensor(
                out=zbig[:].rearrange("p (t c) -> p t c", c=NCLASS),
                in0=zbig[:].rearrange("p (t c) -> p t c", c=NCLASS),
                in1=nmxb[:].unsqueeze(-1).broadcast_to([128, lt, NCLASS]),
                op=mybir.AluOpType.add)
            nc.vector.tensor_tensor(
                out=zbig[:].rearrange("p (t c) -> p t c", c=NCLASS),
                in0=zbig[:].rearrange("p (t c) -> p t c", c=NCLASS),
                in1=seb[:].unsqueeze(-1).broadcast_to([128, lt, NCLASS]),
                op=mybir.AluOpType.subtract)
            nfull = (lt - 1) * 128
            rlast = LAST_ROWS if lt == LT else 128
            nc.sync.dma_start(
                out=out[0:nfull, :].rearrange("(t p) c -> p t c", p=128),
                in_=zbig[:].rearrange("p (t c) -> p t c", c=NCLASS)
                [:, 0:lt - 1, :])
            nc.sync.dma_start(
                out=out[nfull:nfull + rlast, :],
                in_=zbig[0:rlast, (lt - 1) * NCLASS:lt * NCLASS])

    nc.compile()
    return nc


# --------------------------------------------------------------------------
# entry point
# --------------------------------------------------------------------------

def kernel(**inputs) -> np.ndarray:
    NLk, NHk, in_maps = _prep_inputs(**inputs)
    key = (NLk, NHk)
    if key not in _cache:
        _cache[key] = build(NLk, NHk)
    nc = _cache[key]
    res = run_bass_kernel_spmd(nc, in_maps, list(range(NCORES)))
    return np.concatenate([res.results[c]["out"] for c in range(NCORES)], 0)



# revision 30
# speedup vs baseline: 1.2086x; 1.0092x over previous
"""2-layer GAT (nn_GAT_31490700214331) on 8 Trainium2 NeuronCores.

Strategy (dst-sharded, SPMD, per-core-rotated node layout):
  - Nodes are block-partitioned: core c owns nodes [c*6250, (c+1)*6250).
  - Every table on core c uses a ROTATED row order: node n lives at row
    (n - c*6250) mod 50000, so each core's own nodes are rows 0..6249 and
    the single SPMD program has no core-dependent offsets — the rotation
    lives entirely in host-prepared index/input arrays.
  - Layer-0 features (h0 = x @ W0) + attention alphas are computed
    replicated on every core (cheap) into a rotated DRAM table; edges are
    grouped by dst tile (128 dsts) and their source rows fetched with
    dma_gather (int16 indices -> the table is gathered through two views,
    rows [0, SPLIT) and [SPLIT, ...), keeping indices < 32768).
  - Edge softmax (safe without segment-max: |e| <= ~5) and the weighted
    aggregation are fused into per-chunk 128x128 incidence matmuls
    accumulating in PSUM; denominators ride along as 8 extra columns.
  - Between layers the ELU'd hidden state is AllGather'd (feature-major),
    rotated into per-core order with partition-id-offset DMA copies, and
    layer 1 repeats the scheme with 512-wide features and a head-mean +
    log_softmax epilogue.
  - alpha projections fold into the weight matmuls on the host:
    h @ blockdiag(a) == x @ (W @ blockdiag(a)), so the device gets
    W0a=[256,16] / W1a=[128,16] and computes alphas as 16 extra psum cols.

Self-contained: call kernel(**inputs) with the full-problem arrays.
"""
import numpy as np
from contextlib import ExitStack

import concourse.bacc as bacc
import concourse.bass as bass
import concourse.mybir as mybir
from concourse.tile import TileContext
from concourse.bass_utils import run_bass_kernel_spmd

F16 = mybir.dt.float16
F32 = mybir.dt.float32
F8 = mybir.dt.float8e4
I16 = mybir.dt.int16

N = 50000
NFEAT = 256
NHID = 128
NCLASS = 64
HEADS = 8
SLOPE = 0.2
NCORES = 8
NLOC = N // NCORES           # 6250
LT = (NLOC + 127) // 128     # 49 local dst tiles
LAST_ROWS = NLOC - (LT - 1) * 128   # 106 rows in the last tile
GT = 392                     # global node tiles (392*128 = 50176)
GROWS = GT * 128
SPLIT = 25088                # low/high gather-table split (196 tiles)
SENT = 300.0                 # dst_rel sentinel for padding slots
T0W = 256                    # t0 row: [h0(128)|as0(8)|ad0(8)|junk] f16
T1W = 640                    # t1 row: [h1(512)|as1(8)|ad1(8)|junk] f16

_cache = {}


# --------------------------------------------------------------------------
# host-side preparation
# --------------------------------------------------------------------------

def _wrap_idx(idx):
    """[n] int -> [128, n//16] int16 wrapped gather-index layout."""
    n = idx.shape[0]
    assert n % 16 == 0
    w = idx.reshape(n // 16, 16).T.astype(np.int16)
    return np.tile(w, (8, 1))


def _prep_edges(src, dst):
    cores = []
    for c in range(NCORES):
        m = (dst >= c * NLOC) & (dst < (c + 1) * NLOC)
        s = src[m].astype(np.int64)
        d = dst[m].astype(np.int64) - c * NLOC
        order = np.argsort(d, kind="stable")
        s, d = s[order], d[order]
        s_rot = (s - c * NLOC) % N
        tiles = []
        for t in range(LT):
            sel = (d >= t * 128) & (d < (t + 1) * 128)
            st, dt = s_rot[sel], d[sel] - t * 128
            lo = st < SPLIT
            tiles.append((st[lo], dt[lo], st[~lo] - SPLIT, dt[~lo]))
        cores.append(tiles)
    nl = max(len(t[0]) for tl in cores for t in tl)
    nh = max(len(t[2]) for tl in cores for t in tl)
    NL = max(1, (nl + 127) // 128)
    NH = max(1, (nh + 127) // 128)
    assert NL * 128 <= 1024 and NH * 128 <= 1024, (NL, NH)

    out = []
    for c in range(NCORES):
        eil = np.zeros((LT, 128, NL * 8), np.int16)
        eih = np.zeros((LT, 128, NH * 8), np.int16)
        eal = np.zeros((LT, 128, NL * 8), np.int16)
        eah = np.zeros((LT, 128, NH * 8), np.int16)
        drel = np.full((LT, 128, NL + NH), SENT, np.float16)
        for t in range(LT):
            sl, dl, sh, dh = cores[c][t]
            il = np.zeros(NL * 128, np.int64)
            il[: len(sl)] = sl
            al = np.zeros(NL * 128, np.int64)
            al[: len(dl)] = t * 128 + dl
            ih = np.zeros(NH * 128, np.int64)
            ih[: len(sh)] = sh
            ah = np.zeros(NH * 128, np.int64)
            ah[: len(dh)] = t * 128 + dh
            eil[t] = _wrap_idx(il)
            eih[t] = _wrap_idx(ih)
            eal[t] = _wrap_idx(al)
            eah[t] = _wrap_idx(ah)
            rl = np.full(NL * 128, SENT)
            rl[: len(dl)] = dl
            rh = np.full(NH * 128, SENT)
            rh[: len(dh)] = dh
            r = np.concatenate([rl, rh]).reshape(NL + NH, 128).T
            drel[t] = r.astype(np.float16)
        epack = np.concatenate(
            [eil, eih, eal, eah, drel.view(np.int16)], axis=2)
        out.append(dict(epack=np.ascontiguousarray(epack)))
    return NL, NH, out


def _prep_inputs(x, edge_index, W0, a_src0, a_dst0, b0, W1, a_src1, a_dst1,
                 b1):
    src = np.asarray(edge_index[0]).astype(np.int64)
    dst = np.asarray(edge_index[1]).astype(np.int64)
    NL, NH, edata = _prep_edges(src, dst)

    def bd(a):  # [H, D] -> blockdiag [H*D, H]
        a = np.asarray(a, np.float32)
        H, D = a.shape
        m = np.zeros((H * D, H), np.float32)
        for h in range(H):
            m[h * D:(h + 1) * D, h] = a[h]
        return m

    W0 = np.asarray(W0, np.float32)
    W1 = np.asarray(W1, np.float32)
    W0a = np.concatenate([W0 @ bd(a_src0), W0 @ bd(a_dst0)], 1)  # [256, 16]
    # head-innermost feature interleave: new col d*8+h <- old col h*D+d
    perm0 = np.array([(f % 8) * 16 + f // 8 for f in range(128)])
    perm1 = np.array([(f % 8) * 64 + f // 8 for f in range(512)])
    W0cat = np.concatenate([W0[:, perm0], W0a], 1)               # [256, 144]
    W1a = np.concatenate([W1 @ bd(a_src1), W1 @ bd(a_dst1)], 1)  # [128, 16]

    x = np.asarray(x, np.float32)
    ident = np.eye(128, dtype=np.float16)
    colio = np.tile(np.arange(128, dtype=np.float16)[None, :], (128, 1))
    b0b = np.tile(np.asarray(b0, np.float32)[None, :], (128, 1))
    b1b = np.tile(np.asarray(b1, np.float32)[None, :], (128, 1))

    in_maps = []
    for c in range(NCORES):
        rot = np.roll(np.arange(N), -c * NLOC)
        xr = np.zeros((GROWS, NFEAT), np.float16)
        xr[:N] = x[rot].astype(np.float16)
        xtt = xr.reshape(GROWS // 128, 128, 2, 128).transpose(0, 3, 2, 1)
        m = dict(
            xT=np.ascontiguousarray(xtt),
            W0=np.ascontiguousarray(
                W0cat.astype(np.float16).reshape(2, 128, NHID + 16)),
            W1=np.ascontiguousarray(W1[perm0][:, perm1].astype(np.float16)),
            W1a=np.ascontiguousarray(W1a[perm0].astype(np.float16)),
            b0b=np.ascontiguousarray(b0b[:, perm0]), b1b=b1b,
            ident=ident, colio=colio,
            **edata[c],
        )
        in_maps.append(m)
    return NL, NH, in_maps


# --------------------------------------------------------------------------
# device program
# --------------------------------------------------------------------------

def build(NL, NH, lt=LT, gt=GT, debug=False, phases="ABCDE"):
    CH = NL + NH
    HID16 = NHID + 16
    NLI = NL * 128
    NHI = NH * 128

    EPW = NL * 8 + NH * 8 + NL * 8 + NH * 8 + CH   # packed int16 cols
    nc = bacc.Bacc("TRN2")
    xT = nc.dram_tensor("xT", [GROWS // 128, 128, 2, 128], F16,
                        kind="ExternalInput")
    W0i = nc.dram_tensor("W0", [2, 128, NHID + 16], F16,
                         kind="ExternalInput")
    W1i = nc.dram_tensor("W1", [NHID, 512], F16, kind="ExternalInput")
    W1ai = nc.dram_tensor("W1a", [NHID, 16], F16, kind="ExternalInput")
    b0bi = nc.dram_tensor("b0b", [128, NHID], F32, kind="ExternalInput")
    b1bi = nc.dram_tensor("b1b", [128, NCLASS], F32, kind="ExternalInput")
    identi = nc.dram_tensor("ident", [128, 128], F16, kind="ExternalInput")
    colioi = nc.dram_tensor("colio", [128, 128], F16, kind="ExternalInput")
    epacki = nc.dram_tensor("epack", [lt, 128, EPW], I16,
                            kind="ExternalInput")
    out = nc.dram_tensor("out", [NLOC, NCLASS], F32, kind="ExternalOutput")
    dbg = None
    if debug:
        dbg = nc.dram_tensor("dbg", [gt * 128, T0W], F32,
                             kind="ExternalOutput")

    with TileContext(nc) as tc, ExitStack() as stk:
        reg_l = nc.gpsimd.to_reg(NLI)
        reg_h = nc.gpsimd.to_reg(NHI)
        dpool = stk.enter_context(
            tc.tile_pool(name="dram", bufs=1, space="DRAM"))
        t0lo = dpool.tile([SPLIT, T0W], F16, tag="t0lo")
        t0hi = dpool.tile([GROWS - SPLIT, T0W], F16, tag="t0hi")
        t0ad = dpool.tile([lt * 128, 128], F16, tag="t0ad")
        t1lo = dpool.tile([SPLIT, T1W], F16, tag="t1lo")
        t1hi = dpool.tile([GROWS - SPLIT, T1W], F16, tag="t1hi")
        t1ad = dpool.tile([lt * 128, 128], F16, tag="t1ad")
        agin = dpool.tile([128, NLOC], F8, tag="agin")
        agout = dpool.tile([NCORES * 128, NLOC], F8, tag="agout",
                           addr_space="Shared")

        cpool = stk.enter_context(tc.tile_pool(name="const", bufs=1))
        W0s = cpool.tile([128, 2, NHID + 16], F16)
        nc.sync.dma_start(out=W0s[:], in_=W0i.rearrange("k p n -> p k n"))
        W1s = cpool.tile([128, 512], F16)
        nc.sync.dma_start(out=W1s[:], in_=W1i[:])
        W1as = cpool.tile([128, 16], F16)
        nc.sync.dma_start(out=W1as[:], in_=W1ai[:])
        b0s = cpool.tile([128, NHID], F32)
        nc.sync.dma_start(out=b0s[:], in_=b0bi[:])
        b1s = cpool.tile([128, NCLASS], F32)
        nc.sync.dma_start(out=b1s[:], in_=b1bi[:])
        idents = cpool.tile([128, 128], F16)
        nc.sync.dma_start(out=idents[:], in_=identi[:])
        colios = cpool.tile([128, 128], F16)
        nc.sync.dma_start(out=colios[:], in_=colioi[:])
        zeros = cpool.tile([128, 128], F16)
        nc.vector.memset(zeros[:], 0)

        # ---------------- phase A: layer-0 tables (replicated) ------------
        with ExitStack() as pa:
            xp = pa.enter_context(tc.tile_pool(name="pa_x", bufs=4))
            pp = pa.enter_context(
                tc.tile_pool(name="pa_ps", bufs=2, space="PSUM"))
            rp = pa.enter_context(tc.tile_pool(name="pa_row", bufs=4))
            assert gt % 4 == 0
            for gg in range(gt // 4):
                xa = xp.tile([128, 4, 2, 128], F16, tag="xa")
                (nc.sync if gg % 2 else nc.scalar).dma_start(
                    out=xa[:],
                    in_=xT[4 * gg:4 * gg + 4].rearrange(
                        "g p k j -> p g k j"))
                row = rp.tile([128, 4, T0W], F16, tag="row")
                for g2 in range(4):
                    ps = pp.tile([128, HID16], F32, tag="ps")
                    for k in range(2):
                        nc.tensor.matmul(ps[:], xa[:, g2, k, :],
                                         W0s[:, k, :],
                                         start=(k == 0), stop=(k == 1))
                    if g2 % 2:
                        nc.scalar.copy(row[:, g2, 0:HID16], ps[:])
                    else:
                        nc.vector.tensor_copy(row[:, g2, 0:HID16], ps[:])
                    g = 4 * gg + g2
                    if g < lt:
                        adr = rp.tile([128, 8], F16, tag="adr")
                        nc.vector.tensor_copy(
                            adr[:], ps[:, NHID + 8:HID16])
                        nc.sync.dma_start(
                            out=t0ad[g * 128:(g + 1) * 128, 0:8],
                            in_=adr[:])
                eng = nc.scalar if gg % 2 else nc.sync
                g0 = 4 * gg * 128
                if g0 + 512 <= SPLIT:
                    eng.dma_start(
                        out=t0lo[g0:g0 + 512, :]
                        .rearrange("(g p) w -> p g w", p=128),
                        in_=row[:])
                else:
                    o = g0 - SPLIT
                    eng.dma_start(
                        out=t0hi[o:o + 512, :]
                        .rearrange("(g p) w -> p g w", p=128),
                        in_=row[:])

        # ---------------- shared edge phase -------------------------------
        def edge_phase(layer, tbl_lo, tbl_hi, tblad, fdim, trow, rw,
                       post_fn, fin):
            o_il, o_ih = 0, NL * 8
            o_al, o_ah = NL * 16, NL * 16 + NH * 8
            o_dr = NL * 16 + NH * 16
            with ExitStack() as pb:
                ip = pb.enter_context(
                    tc.tile_pool(name=f"ix{layer}", bufs=4))
                gp = pb.enter_context(
                    tc.tile_pool(name=f"gg{layer}", bufs=4))
                apl = pb.enter_context(
                    tc.tile_pool(name=f"ga{layer}", bufs=3))
                rp2 = pb.enter_context(
                    tc.tile_pool(name=f"rh{layer}", bufs=3))
                pp2 = pb.enter_context(
                    tc.tile_pool(name=f"ps{layer}", bufs=2, space="PSUM"))
                op = pb.enter_context(
                    tc.tile_pool(name=f"po{layer}", bufs=3))
                for t in range(lt):
                    ep = ip.tile([128, EPW], I16, tag="ep")
                    nc.sync.dma_start(out=ep[:], in_=epacki[t])
                    il = ep[:, o_il:o_il + NL * 8]
                    ih = ep[:, o_ih:o_ih + NH * 8]
                    al = ep[:, o_al:o_al + NL * 8]
                    ah = ep[:, o_ah:o_ah + NH * 8]
                    dr = ep[:, o_dr:o_dr + CH].bitcast(F16)

                    G = gp.tile([128, CH, trow], F16, tag="G")
                    nc.gpsimd.dma_gather(G[:, 0:NL, :], tbl_lo[:], il,
                                         NLI, reg_l, trow)
                    nc.gpsimd.dma_gather(G[:, NL:CH, :], tbl_hi[:],
                                         ih, NHI, reg_h, trow)
                    A = apl.tile([128, CH, 128], F16, tag="A")
                    nc.gpsimd.dma_gather(A[:, 0:NL, :], tblad[:], al,
                                         NLI, reg_l, 128)
                    nc.gpsimd.dma_gather(A[:, NL:CH, :], tblad[:], ah,
                                         NHI, reg_h, 128)

                    inc = rp2.tile([128, CH, 128], F16, tag="inc")
                    nc.vector.tensor_tensor(
                        out=inc[:],
                        in0=dr.unsqueeze(-1).broadcast_to([128, CH, 128]),
                        in1=colios[:].unsqueeze(1)
                        .broadcast_to([128, CH, 128]),
                        op=mybir.AluOpType.is_equal)
                    EX = rp2.tile([128, CH, 8], F16, tag="EX")
                    nc.vector.tensor_tensor(
                        out=EX[:], in0=G[:, :, fdim:fdim + 8],
                        in1=A[:, :, 0:8], op=mybir.AluOpType.add)
                    nc.scalar.activation(
                        EX[:], EX[:], mybir.ActivationFunctionType.Prelu,
                        alpha=SLOPE)
                    nc.scalar.activation(
                        EX[:], EX[:], mybir.ActivationFunctionType.Exp)

                    R = rp2.tile([128, CH, fdim], F16, tag="R")
                    H = HEADS
                    D = fdim // H
                    nc.vector.tensor_tensor(
                        out=R[:, :, 0:fdim]
                        .rearrange("p c (d h) -> p c d h", h=H),
                        in0=G[:, :, 0:fdim]
                        .rearrange("p c (d h) -> p c d h", h=H),
                        in1=EX[:].unsqueeze(2).broadcast_to([128, CH, D, H]),
                        op=mybir.AluOpType.mult)

                    P1 = pp2.tile([128, fdim], F32, tag="P1")
                    P2 = pp2.tile([128, 8], F32, tag="P2")
                    for ch in range(CH):
                        nc.tensor.matmul(P1[:], inc[:, ch, :],
                                         R[:, ch, 0:fdim],
                                         start=(ch == 0),
                                         stop=(ch == CH - 1))
                    for ch in range(CH):
                        nc.tensor.matmul(P2[:], inc[:, ch, :],
                                         EX[:, ch, :],
                                         start=(ch == 0),
                                         stop=(ch == CH - 1))
                    post_fn(t, P1, P2, op, pp2, fin)

        # ---- L0 post: softmax-div, +b0, ELU, transpose, store ------------
        def post0(t, P1, P2, op, pp2, fin):
            rows = 128 if t < lt - 1 else LAST_ROWS
            r8 = op.tile([128, 8], F32, tag="r8")
            nc.vector.tensor_scalar_add(r8[:], P2[:], 1e-16)
            nc.vector.reciprocal(r8[:], r8[:])
            z = op.tile([128, NHID], F32, tag="z")
            nc.vector.tensor_tensor(
                out=z[:].rearrange("p (d h) -> p d h", h=HEADS),
                in0=P1[:].rearrange("p (d h) -> p d h", h=HEADS),
                in1=r8[:].unsqueeze(1).broadcast_to([128, 16, HEADS]),
                op=mybir.AluOpType.mult)
            nc.vector.tensor_tensor(out=z[:], in0=z[:], in1=b0s[:],
                                    op=mybir.AluOpType.add)
            zr = op.tile([128, NHID], F32, tag="zr")
            nc.scalar.activation(zr[:], z[:],
                                 mybir.ActivationFunctionType.Relu)
            zm = op.tile([128, NHID], F32, tag="zm")
            nc.vector.tensor_tensor(out=zm[:], in0=z[:], in1=zr[:],
                                    op=mybir.AluOpType.subtract)
            nc.scalar.activation(zm[:], zm[:],
                                 mybir.ActivationFunctionType.Exp)
            h1 = op.tile([128, NHID], F16, tag="h1")
            nc.vector.scalar_tensor_tensor(
                h1[:], zm[:], -1.0, zr[:],
                op0=mybir.AluOpType.add, op1=mybir.AluOpType.add)
            pst = pp2.tile([128, 128], F16, tag="pst")
            nc.tensor.transpose(pst[:], h1[:], idents[:])
            hT = op.tile([128, 128], F8, tag="hT")
            nc.scalar.copy(hT[:], pst[:])
            nc.sync.dma_start(
                out=agin[:, t * 128:t * 128 + rows], in_=hT[:, 0:rows])

        if "B" in phases:
            edge_phase(0, t0lo, t0hi, t0ad, NHID, T0W, 8 + NHID, post0, None)

        # ---------------- phase C: AllGather + rotation -------------------
        sregs = None
        if "C" in phases:
            nc.gpsimd.collective_compute(
                "AllGather", mybir.AluOpType.bypass,
                replica_groups=[list(range(NCORES))],
                ins=[agin[:]], outs=[agout[:]])
            pid = nc.partition_id(engines=[mybir.EngineType.SP])
            sregs = [nc.sync.snap(((j + pid) % NCORES) * 128)
                     for j in range(NCORES)]

        # ---------------- phase D: layer-1 tables -------------------------
        with ExitStack() as pd:
            if "D" not in phases:
                pd.enter_context(ExitStack())  # keep structure
            ngt = min(gt, (N + 127) // 128)
            dsup = [(a, min(a + 2, ngt)) for a in range(0, ngt, 2)]
            if "D" not in phases:
                dsup = []
            xp1 = pd.enter_context(tc.tile_pool(name="pd_x", bufs=4))
            pp1 = pd.enter_context(
                tc.tile_pool(name="pd_ps", bufs=2, space="PSUM"))
            rp1 = pd.enter_context(tc.tile_pool(name="pd_row", bufs=4))
            for ga, gb in dsup:
                nsub = gb - ga
                hx8 = xp1.tile([128, 2, 128], F8, tag="hx8")
                hx = xp1.tile([128, 2, 128], F16, tag="hx")
                r0, r1 = ga * 128, min(gb * 128, N)
                hxf = hx8[:].rearrange("p g j -> p (g j)")
                w0 = 0
                r = r0
                while r < r1:
                    j = r // NLOC
                    seg = min(r1, (j + 1) * NLOC) - r
                    nc.sync.dma_start(
                        out=hxf[:, w0:w0 + seg],
                        in_=agout[bass.ds(sregs[j % NCORES], 128),
                                  r - j * NLOC:r - j * NLOC + seg])
                    w0 += seg
                    r += seg
                nc.scalar.copy(hx[:, 0, :], hx8[:, 0, :])
                nc.vector.tensor_copy(hx[:, 1, :], hx8[:, 1, :])
                row = rp1.tile([128, 2, T1W], F16, tag="row")
                psas = []
                for g2 in range(nsub):
                    ps = pp1.tile([128, 512], F32, tag=f"ps{g2}")
                    nc.tensor.matmul(ps[:], hx[:, g2, :], W1s[:],
                                     start=True, stop=True)
                    psa = pp1.tile([128, 16], F32, tag=f"psa{g2}")
                    nc.tensor.matmul(psa[:], hx[:, g2, :], W1as[:],
                                     start=True, stop=True)
                    nc.scalar.copy(row[:, g2, 0:176], ps[:, 0:176])
                    nc.vector.tensor_copy(row[:, g2, 176:512],
                                          ps[:, 176:512])
                    nc.vector.tensor_copy(row[:, g2, 512:528], psa[:])
                    psas.append(psa)
                eng = nc.scalar if ga % 4 else nc.sync
                g0 = ga * 128
                if nsub == 2 and g0 + 256 <= SPLIT:
                    eng.dma_start(
                        out=t1lo[g0:g0 + 256, 0:528]
                        .rearrange("(g p) w -> p g w", p=128),
                        in_=row[:, :, 0:528])
                elif nsub == 2:
                    o = g0 - SPLIT
                    eng.dma_start(
                        out=t1hi[o:o + 256, 0:528]
                        .rearrange("(g p) w -> p g w", p=128),
                        in_=row[:, :, 0:528])
                else:
                    o = g0 - SPLIT
                    eng.dma_start(out=t1hi[o:o + 128, 0:528],
                                  in_=row[:, 0, 0:528])
                for g2 in range(nsub):
                    g = ga + g2
                    if g < lt:
                        adr = rp1.tile([128, 8], F16, tag="adr")
                        nc.vector.tensor_copy(adr[:], psas[g2][:, 8:16])
                        nc.sync.dma_start(
                            out=t1ad[g * 128:(g + 1) * 128, 0:8],
                            in_=adr[:])

        # ---------------- phase E: layer-1 edges + epilogue ---------------
        def post1(t, P1, P2, op, pp2, fin):
            zbig, nmxb, seb = fin
            r8 = op.tile([128, 8], F32, tag="r8")
            nc.vector.tensor_scalar_add(r8[:], P2[:], 1e-16)
            nc.vector.reciprocal(r8[:], r8[:])
            nc.vector.tensor_scalar_mul(r8[:], r8[:], 1.0 / HEADS)
            zw = op.tile([128, 512], F32, tag="zw")
            nc.vector.tensor_tensor(
                out=zw[:].rearrange("p (d h) -> p d h", h=HEADS),
                in0=P1[:].rearrange("p (d h) -> p d h", h=HEADS),
                in1=r8[:].unsqueeze(1).broadcast_to([128, 64, HEADS]),
                op=mybir.AluOpType.mult)
            z = zbig[:, t * NCLASS:(t + 1) * NCLASS]
            nc.vector.reduce_sum(
                z, zw[:].rearrange("p (d h) -> p d h", h=HEADS),
                axis=mybir.AxisListType.X)
            nc.vector.tensor_tensor(out=z, in0=z, in1=b1s[:],
                                    op=mybir.AluOpType.add)
            nmx = nmxb[:, t:t + 1]
            nc.vector.reduce_max(nmx, z, axis=mybir.AxisListType.X,
                                 negate=True)
            ez = op.tile([128, NCLASS], F32, tag="ez")
            nc.scalar.activation(ez[:], z,
                                 mybir.ActivationFunctionType.Exp,
                                 bias=nmx, accum_out=seb[:, t:t + 1])

        if "E" in phases:
            fpool = stk.enter_context(tc.tile_pool(name="fin", bufs=1))
            zbig = fpool.tile([128, lt * NCLASS], F32)
            nmxb = fpool.tile([128, lt], F32)
            seb = fpool.tile([128, lt], F32)
            edge_phase(1, t1lo, t1hi, t1ad, 512, T1W, 520, post1,
                       (zbig, nmxb, seb))
            # batched log-softmax tail: one Ln + two broadcast ops + 2 DMAs
            nc.scalar.activation(seb[:], seb[:],
                                 mybir.ActivationFunctionType.Ln)
            nc.vector.tensor_tensor(
                out=zbig[:].rearrange("p (t c) -> p t c", c=NCLASS),
                in0=zbig[:].rearrange("p (t c) -> p t c", c=NCLASS),
                in1=nmxb[:].unsqueeze(-1).broadcast_to([128, lt, NCLASS]),
                op=mybir.AluOpType.add)
            nc.vector.tensor_tensor(
                out=zbig[:].rearrange("p (t c) -> p t c", c=NCLASS),
                in0=zbig[:].rearrange("p (t c) -> p t c", c=NCLASS),
                in1=seb[:].unsqueeze(-1).broadcast_to([128, lt, NCLASS]),
                op=mybir.AluOpType.subtract)
            nfull = (lt - 1) * 128
            rlast = LAST_ROWS if lt == LT else 128
            nc.sync.dma_start(
                out=out[0:nfull, :].rearrange("(t p) c -> p t c", p=128),
                in_=zbig[:].rearrange("p (t c) -> p t c", c=NCLASS)
                [:, 0:lt - 1, :])
            nc.sync.dma_start(
                out=out[nfull:nfull + rlast, :],
                in_=zbig[0:rlast, (lt - 1) * NCLASS:lt * NCLASS])

    nc.compile()
    return nc


# --------------------------------------------------------------------------
# entry point
# --------------------------------------------------------------------------

def kernel(**inputs) -> np.ndarray:
    NLk, NHk, in_maps = _prep_inputs(**inputs)
    key = (NLk, NHk)
    if key not in _cache:
        _cache[key] = build(NLk, NHk)
    nc = _cache[key]
    res = run_bass_kernel_spmd(nc, in_maps, list(range(NCORES)))
    return np.concatenate([res.results[c]["out"] for c in range(NCORES)], 0)



# revision 31
# speedup vs baseline: 1.2603x; 1.0428x over previous
"""2-layer GAT (nn_GAT_31490700214331) on 8 Trainium2 NeuronCores.

Strategy (dst-sharded, SPMD, per-core-rotated node layout):
  - Nodes are block-partitioned: core c owns nodes [c*6250, (c+1)*6250).
  - Every table on core c uses a ROTATED row order: node n lives at row
    (n - c*6250) mod 50000, so each core's own nodes are rows 0..6249 and
    the single SPMD program has no core-dependent offsets — the rotation
    lives entirely in host-prepared index/input arrays.
  - Layer-0 features (h0 = x @ W0) + attention alphas are computed
    replicated on every core (cheap) into a rotated DRAM table; edges are
    grouped by dst tile (128 dsts) and their source rows fetched with
    dma_gather (int16 indices -> the table is gathered through two views,
    rows [0, SPLIT) and [SPLIT, ...), keeping indices < 32768).
  - Edge softmax (safe without segment-max: |e| <= ~5) and the weighted
    aggregation are fused into per-chunk 128x128 incidence matmuls
    accumulating in PSUM; denominators ride along as 8 extra columns.
  - Between layers the ELU'd hidden state is AllGather'd (feature-major),
    rotated into per-core order with partition-id-offset DMA copies, and
    layer 1 repeats the scheme with 512-wide features and a head-mean +
    log_softmax epilogue.
  - alpha projections fold into the weight matmuls on the host:
    h @ blockdiag(a) == x @ (W @ blockdiag(a)), so the device gets
    W0a=[256,16] / W1a=[128,16] and computes alphas as 16 extra psum cols.

Self-contained: call kernel(**inputs) with the full-problem arrays.
"""
import numpy as np
from contextlib import ExitStack

import concourse.bacc as bacc
import concourse.bass as bass
import concourse.mybir as mybir
from concourse.tile import TileContext
from concourse.bass_utils import run_bass_kernel_spmd

F16 = mybir.dt.float16
F32 = mybir.dt.float32
F8 = mybir.dt.float8e4
I16 = mybir.dt.int16

N = 50000
NFEAT = 256
NHID = 128
NCLASS = 64
HEADS = 8
SLOPE = 0.2
NCORES = 8
NLOC = N // NCORES           # 6250
LT = (NLOC + 127) // 128     # 49 local dst tiles
LAST_ROWS = NLOC - (LT - 1) * 128   # 106 rows in the last tile
GT = 392                     # global node tiles (392*128 = 50176)
GROWS = GT * 128
SPLIT = 25088                # low/high gather-table split (196 tiles)
SENT = 300.0                 # dst_rel sentinel for padding slots
T0W = 256                    # t0 row: [h0(128)|as0(8)|ad0(8)|junk] f16
T1W = 640                    # t1 row: [h1(512)|as1(8)|ad1(8)|junk] f16

_cache = {}


# --------------------------------------------------------------------------
# host-side preparation
# --------------------------------------------------------------------------

def _wrap_idx(idx):
    """[n] int -> [128, n//16] int16 wrapped gather-index layout."""
    n = idx.shape[0]
    assert n % 16 == 0
    w = idx.reshape(n // 16, 16).T.astype(np.int16)
    return np.tile(w, (8, 1))


def _prep_edges(src, dst):
    cores = []
    for c in range(NCORES):
        m = (dst >= c * NLOC) & (dst < (c + 1) * NLOC)
        s = src[m].astype(np.int64)
        d = dst[m].astype(np.int64) - c * NLOC
        order = np.argsort(d, kind="stable")
        s, d = s[order], d[order]
        s_rot = (s - c * NLOC) % N
        tiles = []
        for t in range(LT):
            sel = (d >= t * 128) & (d < (t + 1) * 128)
            st, dt = s_rot[sel], d[sel] - t * 128
            lo = st < SPLIT
            tiles.append((st[lo], dt[lo], st[~lo] - SPLIT, dt[~lo]))
        cores.append(tiles)
    nl = max(len(t[0]) for tl in cores for t in tl)
    nh = max(len(t[2]) for tl in cores for t in tl)
    NL = max(1, (nl + 127) // 128)
    NH = max(1, (nh + 127) // 128)
    assert NL * 128 <= 1024 and NH * 128 <= 1024, (NL, NH)

    out = []
    for c in range(NCORES):
        eil = np.zeros((LT, 128, NL * 8), np.int16)
        eih = np.zeros((LT, 128, NH * 8), np.int16)
        eal = np.zeros((LT, 128, NL * 8), np.int16)
        eah = np.zeros((LT, 128, NH * 8), np.int16)
        drel = np.full((LT, 128, NL + NH), SENT, np.float16)
        for t in range(LT):
            sl, dl, sh, dh = cores[c][t]
            il = np.zeros(NL * 128, np.int64)
            il[: len(sl)] = sl
            al = np.zeros(NL * 128, np.int64)
            al[: len(dl)] = t * 128 + dl
            ih = np.zeros(NH * 128, np.int64)
            ih[: len(sh)] = sh
            ah = np.zeros(NH * 128, np.int64)
            ah[: len(dh)] = t * 128 + dh
            eil[t] = _wrap_idx(il)
            eih[t] = _wrap_idx(ih)
            eal[t] = _wrap_idx(al)
            eah[t] = _wrap_idx(ah)
            rl = np.full(NL * 128, SENT)
            rl[: len(dl)] = dl
            rh = np.full(NH * 128, SENT)
            rh[: len(dh)] = dh
            r = np.concatenate([rl, rh]).reshape(NL + NH, 128).T
            drel[t] = r.astype(np.float16)
        epack = np.concatenate(
            [eil, eih, eal, eah, drel.view(np.int16)], axis=2)
        out.append(dict(epack=np.ascontiguousarray(epack)))
    return NL, NH, out


def _prep_inputs(x, edge_index, W0, a_src0, a_dst0, b0, W1, a_src1, a_dst1,
                 b1):
    src = np.asarray(edge_index[0]).astype(np.int64)
    dst = np.asarray(edge_index[1]).astype(np.int64)
    NL, NH, edata = _prep_edges(src, dst)

    def bd(a):  # [H, D] -> blockdiag [H*D, H]
        a = np.asarray(a, np.float32)
        H, D = a.shape
        m = np.zeros((H * D, H), np.float32)
        for h in range(H):
            m[h * D:(h + 1) * D, h] = a[h]
        return m

    W0 = np.asarray(W0, np.float32)
    W1 = np.asarray(W1, np.float32)
    W0a = np.concatenate([W0 @ bd(a_src0), W0 @ bd(a_dst0)], 1)  # [256, 16]
    # head-innermost feature interleave: new col d*8+h <- old col h*D+d
    perm0 = np.array([(f % 8) * 16 + f // 8 for f in range(128)])
    perm1 = np.array([(f % 8) * 64 + f // 8 for f in range(512)])
    W0cat = np.concatenate([W0[:, perm0], W0a], 1)               # [256, 144]
    W1a = np.concatenate([W1 @ bd(a_src1), W1 @ bd(a_dst1)], 1)  # [128, 16]

    x = np.asarray(x, np.float32)
    ident = np.eye(128, dtype=np.float16)
    colio = np.tile(np.arange(128, dtype=np.float16)[None, :], (128, 1))
    b0b = np.tile(np.asarray(b0, np.float32)[None, :], (128, 1))
    b1b = np.tile(np.asarray(b1, np.float32)[None, :], (128, 1))

    in_maps = []
    for c in range(NCORES):
        rot = np.roll(np.arange(N), -c * NLOC)
        xr = np.zeros((GROWS, NFEAT), np.float16)
        xr[:N] = x[rot].astype(np.float16)
        xtt = xr.reshape(GROWS // 128, 128, 2, 128).transpose(0, 3, 2, 1)
        m = dict(
            xT=np.ascontiguousarray(xtt),
            W0=np.ascontiguousarray(
                W0cat.astype(np.float16).reshape(2, 128, NHID + 16)),
            W1=np.ascontiguousarray(W1[perm0][:, perm1].astype(np.float16)),
            W1a=np.ascontiguousarray(W1a[perm0].astype(np.float16)),
            b0b=np.ascontiguousarray(b0b[:, perm0]), b1b=b1b,
            ident=ident, colio=colio,
            **edata[c],
        )
        in_maps.append(m)
    return NL, NH, in_maps


# --------------------------------------------------------------------------
# device program
# --------------------------------------------------------------------------

def build(NL, NH, lt=LT, gt=GT, debug=False, phases="ABCDE"):
    CH = NL + NH
    HID16 = NHID + 16
    NLI = NL * 128
    NHI = NH * 128

    EPW = NL * 8 + NH * 8 + NL * 8 + NH * 8 + CH   # packed int16 cols
    nc = bacc.Bacc("TRN2")
    xT = nc.dram_tensor("xT", [GROWS // 128, 128, 2, 128], F16,
                        kind="ExternalInput")
    W0i = nc.dram_tensor("W0", [2, 128, NHID + 16], F16,
                         kind="ExternalInput")
    W1i = nc.dram_tensor("W1", [NHID, 512], F16, kind="ExternalInput")
    W1ai = nc.dram_tensor("W1a", [NHID, 16], F16, kind="ExternalInput")
    b0bi = nc.dram_tensor("b0b", [128, NHID], F32, kind="ExternalInput")
    b1bi = nc.dram_tensor("b1b", [128, NCLASS], F32, kind="ExternalInput")
    identi = nc.dram_tensor("ident", [128, 128], F16, kind="ExternalInput")
    colioi = nc.dram_tensor("colio", [128, 128], F16, kind="ExternalInput")
    epacki = nc.dram_tensor("epack", [lt, 128, EPW], I16,
                            kind="ExternalInput")
    out = nc.dram_tensor("out", [NLOC, NCLASS], F32, kind="ExternalOutput")
    dbg = None
    if debug:
        dbg = nc.dram_tensor("dbg", [gt * 128, T0W], F32,
                             kind="ExternalOutput")

    with TileContext(nc) as tc, ExitStack() as stk:
        reg_l = nc.gpsimd.to_reg(NLI)
        reg_h = nc.gpsimd.to_reg(NHI)
        dpool = stk.enter_context(
            tc.tile_pool(name="dram", bufs=1, space="DRAM"))
        t0lo = dpool.tile([SPLIT, T0W], F16, tag="t0lo")
        t0hi = dpool.tile([GROWS - SPLIT, T0W], F16, tag="t0hi")
        t0ad = dpool.tile([lt * 128, 128], F16, tag="t0ad")
        t1lo = dpool.tile([SPLIT, T1W], F16, tag="t1lo")
        t1hi = dpool.tile([GROWS - SPLIT, T1W], F16, tag="t1hi")
        t1ad = dpool.tile([lt * 128, 128], F16, tag="t1ad")
        agin = dpool.tile([128, NLOC], F8, tag="agin")
        agout = dpool.tile([NCORES * 128, NLOC], F8, tag="agout",
                           addr_space="Shared")

        cpool = stk.enter_context(tc.tile_pool(name="const", bufs=1))
        W0s = cpool.tile([128, 2, NHID + 16], F16)
        nc.sync.dma_start(out=W0s[:], in_=W0i.rearrange("k p n -> p k n"))
        W1s = cpool.tile([128, 512], F16)
        nc.sync.dma_start(out=W1s[:], in_=W1i[:])
        W1as = cpool.tile([128, 16], F16)
        nc.sync.dma_start(out=W1as[:], in_=W1ai[:])
        b0s = cpool.tile([128, NHID], F32)
        nc.sync.dma_start(out=b0s[:], in_=b0bi[:])
        b1s = cpool.tile([128, NCLASS], F32)
        nc.sync.dma_start(out=b1s[:], in_=b1bi[:])
        idents = cpool.tile([128, 128], F16)
        nc.sync.dma_start(out=idents[:], in_=identi[:])
        colios = cpool.tile([128, 128], F16)
        nc.sync.dma_start(out=colios[:], in_=colioi[:])
        zeros = cpool.tile([128, 128], F16)
        nc.vector.memset(zeros[:], 0)

        # ---------------- phase A: layer-0 tables (replicated) ------------
        with ExitStack() as pa:
            xp = pa.enter_context(tc.tile_pool(name="pa_x", bufs=4))
            pp = pa.enter_context(
                tc.tile_pool(name="pa_ps", bufs=2, space="PSUM"))
            rp = pa.enter_context(tc.tile_pool(name="pa_row", bufs=4))
            assert gt % 4 == 0
            for gg in range(gt // 4):
                xa = xp.tile([128, 4, 2, 128], F16, tag="xa")
                (nc.sync if gg % 2 else nc.scalar).dma_start(
                    out=xa[:],
                    in_=xT[4 * gg:4 * gg + 4].rearrange(
                        "g p k j -> p g k j"))
                row = rp.tile([128, 4, T0W], F16, tag="row")
                for g2 in range(4):
                    ps = pp.tile([128, HID16], F32, tag="ps")
                    for k in range(2):
                        nc.tensor.matmul(ps[:], xa[:, g2, k, :],
                                         W0s[:, k, :],
                                         start=(k == 0), stop=(k == 1))
                    if g2 % 2:
                        nc.scalar.copy(row[:, g2, 0:HID16], ps[:])
                    else:
                        nc.vector.tensor_copy(row[:, g2, 0:HID16], ps[:])
                    g = 4 * gg + g2
                    if g < lt:
                        adr = rp.tile([128, 8], F16, tag="adr")
                        nc.vector.tensor_copy(
                            adr[:], ps[:, NHID + 8:HID16])
                        nc.sync.dma_start(
                            out=t0ad[g * 128:(g + 1) * 128, 0:8],
                            in_=adr[:])
                eng = nc.scalar if gg % 2 else nc.sync
                g0 = 4 * gg * 128
                if g0 + 512 <= SPLIT:
                    eng.dma_start(
                        out=t0lo[g0:g0 + 512, :]
                        .rearrange("(g p) w -> p g w", p=128),
                        in_=row[:])
                else:
                    o = g0 - SPLIT
                    eng.dma_start(
                        out=t0hi[o:o + 512, :]
                        .rearrange("(g p) w -> p g w", p=128),
                        in_=row[:])

        # ---------------- shared edge phase -------------------------------
        def edge_phase(layer, tbl_lo, tbl_hi, tblad, fdim, trow, rw,
                       post_fn, fin):
            o_il, o_ih = 0, NL * 8
            o_al, o_ah = NL * 16, NL * 16 + NH * 8
            o_dr = NL * 16 + NH * 16
            with ExitStack() as pb:
                ip = pb.enter_context(
                    tc.tile_pool(name=f"ix{layer}", bufs=4))
                gp = pb.enter_context(
                    tc.tile_pool(name=f"gg{layer}", bufs=4))
                apl = pb.enter_context(
                    tc.tile_pool(name=f"ga{layer}", bufs=3))
                rp2 = pb.enter_context(
                    tc.tile_pool(name=f"rh{layer}", bufs=3))
                pp2 = pb.enter_context(
                    tc.tile_pool(name=f"ps{layer}", bufs=2, space="PSUM"))
                op = pb.enter_context(
                    tc.tile_pool(name=f"po{layer}", bufs=3))
                for t in range(lt):
                    ep = ip.tile([128, EPW], I16, tag="ep")
                    nc.sync.dma_start(out=ep[:], in_=epacki[t])
                    il = ep[:, o_il:o_il + NL * 8]
                    ih = ep[:, o_ih:o_ih + NH * 8]
                    al = ep[:, o_al:o_al + NL * 8]
                    ah = ep[:, o_ah:o_ah + NH * 8]
                    dr = ep[:, o_dr:o_dr + CH].bitcast(F16)

                    G = gp.tile([128, CH, trow], F16, tag="G")
                    nc.gpsimd.dma_gather(G[:, 0:NL, :], tbl_lo[:], il,
                                         NLI, reg_l, trow)
                    nc.gpsimd.dma_gather(G[:, NL:CH, :], tbl_hi[:],
                                         ih, NHI, reg_h, trow)
                    A = apl.tile([128, CH, 128], F16, tag="A")
                    nc.gpsimd.dma_gather(A[:, 0:NL, :], tblad[:], al,
                                         NLI, reg_l, 128)
                    nc.gpsimd.dma_gather(A[:, NL:CH, :], tblad[:], ah,
                                         NHI, reg_h, 128)

                    inc = rp2.tile([128, CH, 128], F16, tag="inc")
                    nc.vector.tensor_tensor(
                        out=inc[:],
                        in0=dr.unsqueeze(-1).broadcast_to([128, CH, 128]),
                        in1=colios[:].unsqueeze(1)
                        .broadcast_to([128, CH, 128]),
                        op=mybir.AluOpType.is_equal)
                    EX = rp2.tile([128, CH, 8], F16, tag="EX")
                    nc.vector.tensor_tensor(
                        out=EX[:], in0=G[:, :, fdim:fdim + 8],
                        in1=A[:, :, 0:8], op=mybir.AluOpType.add)
                    nc.scalar.activation(
                        EX[:], EX[:], mybir.ActivationFunctionType.Prelu,
                        alpha=SLOPE)
                    nc.scalar.activation(
                        EX[:], EX[:], mybir.ActivationFunctionType.Exp)

                    R = rp2.tile([128, CH, fdim], F16, tag="R")
                    H = HEADS
                    D = fdim // H
                    nc.vector.tensor_tensor(
                        out=R[:, :, 0:fdim]
                        .rearrange("p c (d h) -> p c d h", h=H),
                        in0=G[:, :, 0:fdim]
                        .rearrange("p c (d h) -> p c d h", h=H),
                        in1=EX[:].unsqueeze(2).broadcast_to([128, CH, D, H]),
                        op=mybir.AluOpType.mult)

                    P1 = pp2.tile([128, fdim], F32, tag="P1")
                    P2 = pp2.tile([128, 8], F32, tag="P2")
                    for ch in range(CH):
                        nc.tensor.matmul(P1[:], inc[:, ch, :],
                                         R[:, ch, 0:fdim],
                                         start=(ch == 0),
                                         stop=(ch == CH - 1))
                    for ch in range(CH):
                        nc.tensor.matmul(P2[:], inc[:, ch, :],
                                         EX[:, ch, :],
                                         start=(ch == 0),
                                         stop=(ch == CH - 1))
                    post_fn(t, P1, P2, op, pp2, fin)

        # ---- L0 post: softmax-div, +b0, ELU, transpose, store ------------
        def post0(t, P1, P2, op, pp2, fin):
            rows = 128 if t < lt - 1 else LAST_ROWS
            r8 = op.tile([128, 8], F32, tag="r8")
            nc.vector.tensor_scalar_add(r8[:], P2[:], 1e-16)
            nc.vector.reciprocal(r8[:], r8[:])
            z = op.tile([128, NHID], F32, tag="z")
            nc.vector.tensor_tensor(
                out=z[:].rearrange("p (d h) -> p d h", h=HEADS),
                in0=P1[:].rearrange("p (d h) -> p d h", h=HEADS),
                in1=r8[:].unsqueeze(1).broadcast_to([128, 16, HEADS]),
                op=mybir.AluOpType.mult)
            nc.vector.tensor_tensor(out=z[:], in0=z[:], in1=b0s[:],
                                    op=mybir.AluOpType.add)
            zr = op.tile([128, NHID], F32, tag="zr")
            nc.scalar.activation(zr[:], z[:],
                                 mybir.ActivationFunctionType.Relu)
            zm = op.tile([128, NHID], F32, tag="zm")
            nc.vector.tensor_tensor(out=zm[:], in0=z[:], in1=zr[:],
                                    op=mybir.AluOpType.subtract)
            nc.scalar.activation(zm[:], zm[:],
                                 mybir.ActivationFunctionType.Exp)
            h1 = op.tile([128, NHID], F16, tag="h1")
            nc.vector.scalar_tensor_tensor(
                h1[:], zm[:], -1.0, zr[:],
                op0=mybir.AluOpType.add, op1=mybir.AluOpType.add)
            pst = pp2.tile([128, 128], F16, tag="pst")
            nc.tensor.transpose(pst[:], h1[:], idents[:])
            hT = op.tile([128, 128], F8, tag="hT")
            nc.scalar.copy(hT[:], pst[:])
            nc.sync.dma_start(
                out=agin[:, t * 128:t * 128 + rows], in_=hT[:, 0:rows])

        if "B" in phases:
            edge_phase(0, t0lo, t0hi, t0ad, NHID, T0W, 8 + NHID, post0, None)

        # ---------------- phase C: AllGather + rotation -------------------
        sregs = None
        if "C" in phases:
            nc.gpsimd.collective_compute(
                "AllGather", mybir.AluOpType.bypass,
                replica_groups=[list(range(NCORES))],
                ins=[agin[:]], outs=[agout[:]])
            pid = nc.partition_id(engines=[mybir.EngineType.SP])
            sregs = [nc.sync.snap(((j + pid) % NCORES) * 128)
                     for j in range(NCORES)]

        # ---------------- phase D: layer-1 tables -------------------------
        with ExitStack() as pd:
            if "D" not in phases:
                pd.enter_context(ExitStack())  # keep structure
            ngt = min(gt, (N + 127) // 128)
            spans = [(x, min(x + 8, ngt)) for x in range(0, ngt, 8)]
            if "D" not in phases or sregs is None:
                spans = []
            xp1 = pd.enter_context(tc.tile_pool(name="pd_x", bufs=3))
            pp1 = pd.enter_context(
                tc.tile_pool(name="pd_ps", bufs=2, space="PSUM"))
            ppA = pd.enter_context(
                tc.tile_pool(name="pd_pa", bufs=2, space="PSUM"))
            rp1 = pd.enter_context(tc.tile_pool(name="pd_row", bufs=4))
            for si, (sa, sb) in enumerate(spans):
                hx8 = xp1.tile([128, 8, 128], F8, tag="hx8")
                hx = xp1.tile([128, 8, 128], F16, tag="hx")
                r0, r1 = sa * 128, min(sb * 128, N)
                hxf = hx8[:].rearrange("p g j -> p (g j)")
                w0 = 0
                r = r0
                while r < r1:
                    j = r // NLOC
                    seg = min(r1, (j + 1) * NLOC) - r
                    nc.sync.dma_start(
                        out=hxf[:, w0:w0 + seg],
                        in_=agout[bass.ds(sregs[j % NCORES], 128),
                                  r - j * NLOC:r - j * NLOC + seg])
                    w0 += seg
                    r += seg
                nc.scalar.copy(hx[:, 0:4, :], hx8[:, 0:4, :])
                nc.vector.tensor_copy(hx[:, 4:8, :], hx8[:, 4:8, :])
                for ga in range(sa, sb, 2):
                    gb = min(ga + 2, sb)
                    nsub = gb - ga
                    row = rp1.tile([128, 2, T1W], F16, tag="row")
                    psH = pp1.tile([128, 2, 512], F32, tag="psH")
                    psA = ppA.tile([128, 2, 512], F32, tag="psA")
                    for g2 in range(nsub):
                        nc.tensor.matmul(psH[:, g2, :],
                                         hx[:, ga - sa + g2, :], W1s[:],
                                         start=True, stop=True)
                        nc.tensor.matmul(psA[:, g2, 0:16],
                                         hx[:, ga - sa + g2, :], W1as[:],
                                         start=True, stop=True)
                    nc.scalar.copy(row[:, 0:nsub, 0:176],
                                   psH[:, 0:nsub, 0:176])
                    nc.vector.tensor_copy(row[:, 0:nsub, 176:512],
                                          psH[:, 0:nsub, 176:512])
                    nc.vector.tensor_copy(row[:, 0:nsub, 512:528],
                                          psA[:, 0:nsub, 0:16])
                    eng = nc.scalar if ga % 4 else nc.sync
                    g0 = ga * 128
                    if nsub == 2 and g0 + 256 <= SPLIT:
                        eng.dma_start(
                            out=t1lo[g0:g0 + 256, 0:528]
                            .rearrange("(g p) w -> p g w", p=128),
                            in_=row[:, :, 0:528])
                    elif nsub == 2:
                        o = g0 - SPLIT
                        eng.dma_start(
                            out=t1hi[o:o + 256, 0:528]
                            .rearrange("(g p) w -> p g w", p=128),
                            in_=row[:, :, 0:528])
                    else:
                        o = g0 - SPLIT
                        eng.dma_start(out=t1hi[o:o + 128, 0:528],
                                      in_=row[:, 0, 0:528])
                    for g2 in range(nsub):
                        g = ga + g2
                        if g < lt:
                            adr = rp1.tile([128, 8], F16, tag="adr")
                            nc.vector.tensor_copy(adr[:],
                                                  psA[:, g2, 8:16])
                            nc.sync.dma_start(
                                out=t1ad[g * 128:(g + 1) * 128, 0:8],
                                in_=adr[:])

        # ---------------- phase E: layer-1 edges + epilogue ---------------
        def post1(t, P1, P2, op, pp2, fin):
            zbig, nmxb, seb = fin
            r8 = op.tile([128, 8], F32, tag="r8")
            nc.vector.tensor_scalar_add(r8[:], P2[:], 1e-16)
            nc.vector.reciprocal(r8[:], r8[:])
            nc.vector.tensor_scalar_mul(r8[:], r8[:], 1.0 / HEADS)
            zw = op.tile([128, 512], F32, tag="zw")
            nc.vector.tensor_tensor(
                out=zw[:].rearrange("p (d h) -> p d h", h=HEADS),
                in0=P1[:].rearrange("p (d h) -> p d h", h=HEADS),
                in1=r8[:].unsqueeze(1).broadcast_to([128, 64, HEADS]),
                op=mybir.AluOpType.mult)
            z = zbig[:, t * NCLASS:(t + 1) * NCLASS]
            nc.vector.reduce_sum(
                z, zw[:].rearrange("p (d h) -> p d h", h=HEADS),
                axis=mybir.AxisListType.X)
            nc.vector.tensor_tensor(out=z, in0=z, in1=b1s[:],
                                    op=mybir.AluOpType.add)
            nmx = nmxb[:, t:t + 1]
            nc.vector.reduce_max(nmx, z, axis=mybir.AxisListType.X,
                                 negate=True)
            ez = op.tile([128, NCLASS], F32, tag="ez")
            nc.scalar.activation(ez[:], z,
                                 mybir.ActivationFunctionType.Exp,
                                 bias=nmx, accum_out=seb[:, t:t + 1])

        if "E" in phases:
            fpool = stk.enter_context(tc.tile_pool(name="fin", bufs=1))
            zbig = fpool.tile([128, lt * NCLASS], F32)
            nmxb = fpool.tile([128, lt], F32)
            seb = fpool.tile([128, lt], F32)
            edge_phase(1, t1lo, t1hi, t1ad, 512, T1W, 520, post1,
                       (zbig, nmxb, seb))
            # batched log-softmax tail: one Ln + two broadcast ops + 2 DMAs
            nc.scalar.activation(seb[:], seb[:],
                                 mybir.ActivationFunctionType.Ln)
            nc.vector.tensor_tensor(
                out=zbig[:].rearrange("p (t c) -> p t c", c=NCLASS),
                in0=zbig[:].rearrange("p (t c) -> p t c", c=NCLASS),
                in1=nmxb[:].unsqueeze(-1).broadcast_to([128, lt, NCLASS]),
                op=mybir.AluOpType.add)
            nc.vector.tensor_tensor(
                out=zbig[:].rearrange("p (t c) -> p t c", c=NCLASS),
                in0=zbig[:].rearrange("p (t c) -> p t c", c=NCLASS),
                in1=seb[:].unsqueeze(-1).broadcast_to([128, lt, NCLASS]),
                op=mybir.AluOpType.subtract)
            nfull = (lt - 1) * 128
            rlast = LAST_ROWS if lt == LT else 128
            nc.sync.dma_start(
                out=out[0:nfull, :].rearrange("(t p) c -> p t c", p=128),
                in_=zbig[:].rearrange("p (t c) -> p t c", c=NCLASS)
                [:, 0:lt - 1, :])
            nc.sync.dma_start(
                out=out[nfull:nfull + rlast, :],
                in_=zbig[0:rlast, (lt - 1) * NCLASS:lt * NCLASS])

    nc.compile()
    return nc


# --------------------------------------------------------------------------
# entry point
# --------------------------------------------------------------------------

def kernel(**inputs) -> np.ndarray:
    NLk, NHk, in_maps = _prep_inputs(**inputs)
    key = (NLk, NHk)
    if key not in _cache:
        _cache[key] = build(NLk, NHk)
    nc = _cache[key]
    res = run_bass_kernel_spmd(nc, in_maps, list(range(NCORES)))
    return np.concatenate([res.results[c]["out"] for c in range(NCORES)], 0)



# revision 32
# speedup vs baseline: 1.2886x; 1.0225x over previous
"""2-layer GAT (nn_GAT_31490700214331) on 8 Trainium2 NeuronCores.

Strategy (dst-sharded, SPMD, per-core-rotated node layout):
  - Nodes are block-partitioned: core c owns nodes [c*6250, (c+1)*6250).
  - Every table on core c uses a ROTATED row order: node n lives at row
    (n - c*6250) mod 50000, so each core's own nodes are rows 0..6249 and
    the single SPMD program has no core-dependent offsets — the rotation
    lives entirely in host-prepared index/input arrays.
  - Layer-0 features (h0 = x @ W0) + attention alphas are computed
    replicated on every core (cheap) into a rotated DRAM table; edges are
    grouped by dst tile (128 dsts) and their source rows fetched with
    dma_gather (int16 indices -> the table is gathered through two views,
    rows [0, SPLIT) and [SPLIT, ...), keeping indices < 32768).
  - Edge softmax (safe without segment-max: |e| <= ~5) and the weighted
    aggregation are fused into per-chunk 128x128 incidence matmuls
    accumulating in PSUM; denominators ride along as 8 extra columns.
  - Between layers the ELU'd hidden state is AllGather'd (feature-major),
    rotated into per-core order with partition-id-offset DMA copies, and
    layer 1 repeats the scheme with 512-wide features and a head-mean +
    log_softmax epilogue.
  - alpha projections fold into the weight matmuls on the host:
    h @ blockdiag(a) == x @ (W @ blockdiag(a)), so the device gets
    W0a=[256,16] / W1a=[128,16] and computes alphas as 16 extra psum cols.

Self-contained: call kernel(**inputs) with the full-problem arrays.
"""
import numpy as np
from contextlib import ExitStack

import concourse.bacc as bacc
import concourse.bass as bass
import concourse.mybir as mybir
from concourse.tile import TileContext
from concourse.bass_utils import run_bass_kernel_spmd

F16 = mybir.dt.float16
F32 = mybir.dt.float32
F8 = mybir.dt.float8e4
I16 = mybir.dt.int16

N = 50000
NFEAT = 256
NHID = 128
NCLASS = 64
HEADS = 8
SLOPE = 0.2
NCORES = 8
NLOC = N // NCORES           # 6250
LT = (NLOC + 127) // 128     # 49 local dst tiles
LAST_ROWS = NLOC - (LT - 1) * 128   # 106 rows in the last tile
GT = 392                     # global node tiles (392*128 = 50176)
GROWS = GT * 128
SPLIT = 25088                # low/high gather-table split (196 tiles)
SENT = 300.0                 # dst_rel sentinel for padding slots
T0W = 256                    # t0 row: [h0(128)|as0(8)|ad0(8)|junk] f16
T1W = 640                    # t1 row: [h1(512)|as1(8)|ad1(8)|junk] f16

_cache = {}


# --------------------------------------------------------------------------
# host-side preparation
# --------------------------------------------------------------------------

def _wrap_idx(idx):
    """[n] int -> [128, n//16] int16 wrapped gather-index layout."""
    n = idx.shape[0]
    assert n % 16 == 0
    w = idx.reshape(n // 16, 16).T.astype(np.int16)
    return np.tile(w, (8, 1))


def _prep_edges(src, dst):
    cores = []
    for c in range(NCORES):
        m = (dst >= c * NLOC) & (dst < (c + 1) * NLOC)
        s = src[m].astype(np.int64)
        d = dst[m].astype(np.int64) - c * NLOC
        order = np.argsort(d, kind="stable")
        s, d = s[order], d[order]
        s_rot = (s - c * NLOC) % N
        tiles = []
        for t in range(LT):
            sel = (d >= t * 128) & (d < (t + 1) * 128)
            st, dt = s_rot[sel], d[sel] - t * 128
            lo = st < SPLIT
            tiles.append((st[lo], dt[lo], st[~lo] - SPLIT, dt[~lo]))
        cores.append(tiles)
    nl = max(len(t[0]) for tl in cores for t in tl)
    nh = max(len(t[2]) for tl in cores for t in tl)
    NL = max(1, (nl + 127) // 128)
    NH = max(1, (nh + 127) // 128)
    assert NL * 128 <= 1024 and NH * 128 <= 1024, (NL, NH)

    out = []
    for c in range(NCORES):
        eil = np.zeros((LT, 128, NL * 8), np.int16)
        eih = np.zeros((LT, 128, NH * 8), np.int16)
        eal = np.zeros((LT, 128, NL * 8), np.int16)
        eah = np.zeros((LT, 128, NH * 8), np.int16)
        drel = np.full((LT, 128, NL + NH), SENT, np.float16)
        for t in range(LT):
            sl, dl, sh, dh = cores[c][t]
            il = np.zeros(NL * 128, np.int64)
            il[: len(sl)] = sl
            al = np.zeros(NL * 128, np.int64)
            al[: len(dl)] = t * 128 + dl
            ih = np.zeros(NH * 128, np.int64)
            ih[: len(sh)] = sh
            ah = np.zeros(NH * 128, np.int64)
            ah[: len(dh)] = t * 128 + dh
            eil[t] = _wrap_idx(il)
            eih[t] = _wrap_idx(ih)
            eal[t] = _wrap_idx(al)
            eah[t] = _wrap_idx(ah)
            rl = np.full(NL * 128, SENT)
            rl[: len(dl)] = dl
            rh = np.full(NH * 128, SENT)
            rh[: len(dh)] = dh
            r = np.concatenate([rl, rh]).reshape(NL + NH, 128).T
            drel[t] = r.astype(np.float16)
        epack = np.concatenate(
            [eil, eih, eal, eah, drel.view(np.int16)], axis=2)
        out.append(dict(epack=np.ascontiguousarray(epack)))
    return NL, NH, out


def _prep_inputs(x, edge_index, W0, a_src0, a_dst0, b0, W1, a_src1, a_dst1,
                 b1):
    src = np.asarray(edge_index[0]).astype(np.int64)
    dst = np.asarray(edge_index[1]).astype(np.int64)
    NL, NH, edata = _prep_edges(src, dst)

    def bd(a):  # [H, D] -> blockdiag [H*D, H]
        a = np.asarray(a, np.float32)
        H, D = a.shape
        m = np.zeros((H * D, H), np.float32)
        for h in range(H):
            m[h * D:(h + 1) * D, h] = a[h]
        return m

    W0 = np.asarray(W0, np.float32)
    W1 = np.asarray(W1, np.float32)
    W0a = np.concatenate([W0 @ bd(a_src0), W0 @ bd(a_dst0)], 1)  # [256, 16]
    # head-innermost feature interleave: new col d*8+h <- old col h*D+d
    perm0 = np.array([(f % 8) * 16 + f // 8 for f in range(128)])
    perm1 = np.array([(f % 8) * 64 + f // 8 for f in range(512)])
    W0cat = np.concatenate([W0[:, perm0], W0a], 1)               # [256, 144]
    W1a = np.concatenate([W1 @ bd(a_src1), W1 @ bd(a_dst1)], 1)  # [128, 16]

    x = np.asarray(x, np.float32)
    ident = np.eye(128, dtype=np.float16)
    colio = np.tile(np.arange(128, dtype=np.float16)[None, :], (128, 1))
    b0b = np.tile(np.asarray(b0, np.float32)[None, :], (128, 1))
    b1b = np.tile(np.asarray(b1, np.float32)[None, :], (128, 1))

    in_maps = []
    for c in range(NCORES):
        rot = np.roll(np.arange(N), -c * NLOC)
        xr = np.zeros((GROWS, NFEAT), np.float16)
        xr[:N] = x[rot].astype(np.float16)
        xtt = xr.reshape(GROWS // 128, 128, 2, 128).transpose(0, 3, 2, 1)
        m = dict(
            xT=np.ascontiguousarray(xtt),
            W0=np.ascontiguousarray(
                W0cat.astype(np.float16).reshape(2, 128, NHID + 16)),
            W1=np.ascontiguousarray(W1[perm0][:, perm1].astype(np.float16)),
            W1a=np.ascontiguousarray(W1a[perm0].astype(np.float16)),
            b0b=np.ascontiguousarray(b0b[:, perm0]), b1b=b1b,
            ident=ident, colio=colio,
            **edata[c],
        )
        in_maps.append(m)
    return NL, NH, in_maps


# --------------------------------------------------------------------------
# device program
# --------------------------------------------------------------------------

def build(NL, NH, lt=LT, gt=GT, debug=False, phases="ABCDE"):
    CH = NL + NH
    HID16 = NHID + 16
    NLI = NL * 128
    NHI = NH * 128

    EPW = NL * 8 + NH * 8 + NL * 8 + NH * 8 + CH   # packed int16 cols
    nc = bacc.Bacc("TRN2")
    xT = nc.dram_tensor("xT", [GROWS // 128, 128, 2, 128], F16,
                        kind="ExternalInput")
    W0i = nc.dram_tensor("W0", [2, 128, NHID + 16], F16,
                         kind="ExternalInput")
    W1i = nc.dram_tensor("W1", [NHID, 512], F16, kind="ExternalInput")
    W1ai = nc.dram_tensor("W1a", [NHID, 16], F16, kind="ExternalInput")
    b0bi = nc.dram_tensor("b0b", [128, NHID], F32, kind="ExternalInput")
    b1bi = nc.dram_tensor("b1b", [128, NCLASS], F32, kind="ExternalInput")
    identi = nc.dram_tensor("ident", [128, 128], F16, kind="ExternalInput")
    colioi = nc.dram_tensor("colio", [128, 128], F16, kind="ExternalInput")
    epacki = nc.dram_tensor("epack", [lt, 128, EPW], I16,
                            kind="ExternalInput")
    out = nc.dram_tensor("out", [NLOC, NCLASS], F32, kind="ExternalOutput")
    dbg = None
    if debug:
        dbg = nc.dram_tensor("dbg", [gt * 128, T0W], F32,
                             kind="ExternalOutput")

    with TileContext(nc) as tc, ExitStack() as stk:
        reg_l = nc.gpsimd.to_reg(NLI)
        reg_h = nc.gpsimd.to_reg(NHI)
        dpool = stk.enter_context(
            tc.tile_pool(name="dram", bufs=1, space="DRAM"))
        t0lo = dpool.tile([SPLIT, T0W], F16, tag="t0lo")
        t0hi = dpool.tile([GROWS - SPLIT, T0W], F16, tag="t0hi")
        t0ad = dpool.tile([lt * 128, 128], F16, tag="t0ad")
        t1lo = dpool.tile([SPLIT, T1W], F16, tag="t1lo")
        t1hi = dpool.tile([GROWS - SPLIT, T1W], F16, tag="t1hi")
        t1ad = dpool.tile([lt * 128, 128], F16, tag="t1ad")
        HCOL = 3200              # 25 local tiles; 128-aligned AG split
        aginA = dpool.tile([128, HCOL], F8, tag="aginA")
        aginB = dpool.tile([128, NLOC - HCOL], F8, tag="aginB")
        agoutA = dpool.tile([NCORES * 128, HCOL], F8, tag="agoutA",
                            addr_space="Shared")
        agoutB = dpool.tile([NCORES * 128, NLOC - HCOL], F8, tag="agoutB",
                            addr_space="Shared")

        cpool = stk.enter_context(tc.tile_pool(name="const", bufs=1))
        W0s = cpool.tile([128, 2, NHID + 16], F16)
        nc.sync.dma_start(out=W0s[:], in_=W0i.rearrange("k p n -> p k n"))
        W1s = cpool.tile([128, 512], F16)
        nc.sync.dma_start(out=W1s[:], in_=W1i[:])
        W1as = cpool.tile([128, 16], F16)
        nc.sync.dma_start(out=W1as[:], in_=W1ai[:])
        b0s = cpool.tile([128, NHID], F32)
        nc.sync.dma_start(out=b0s[:], in_=b0bi[:])
        b1s = cpool.tile([128, NCLASS], F32)
        nc.sync.dma_start(out=b1s[:], in_=b1bi[:])
        idents = cpool.tile([128, 128], F16)
        nc.sync.dma_start(out=idents[:], in_=identi[:])
        colios = cpool.tile([128, 128], F16)
        nc.sync.dma_start(out=colios[:], in_=colioi[:])
        zeros = cpool.tile([128, 128], F16)
        nc.vector.memset(zeros[:], 0)

        # ---------------- phase A: layer-0 tables (replicated) ------------
        with ExitStack() as pa:
            xp = pa.enter_context(tc.tile_pool(name="pa_x", bufs=4))
            pp = pa.enter_context(
                tc.tile_pool(name="pa_ps", bufs=2, space="PSUM"))
            rp = pa.enter_context(tc.tile_pool(name="pa_row", bufs=4))
            assert gt % 4 == 0
            for gg in range(gt // 4):
                xa = xp.tile([128, 4, 2, 128], F16, tag="xa")
                (nc.sync if gg % 2 else nc.scalar).dma_start(
                    out=xa[:],
                    in_=xT[4 * gg:4 * gg + 4].rearrange(
                        "g p k j -> p g k j"))
                row = rp.tile([128, 4, T0W], F16, tag="row")
                for g2 in range(4):
                    ps = pp.tile([128, HID16], F32, tag="ps")
                    for k in range(2):
                        nc.tensor.matmul(ps[:], xa[:, g2, k, :],
                                         W0s[:, k, :],
                                         start=(k == 0), stop=(k == 1))
                    if g2 % 2:
                        nc.scalar.copy(row[:, g2, 0:HID16], ps[:])
                    else:
                        nc.vector.tensor_copy(row[:, g2, 0:HID16], ps[:])
                    g = 4 * gg + g2
                    if g < lt:
                        adr = rp.tile([128, 8], F16, tag="adr")
                        nc.vector.tensor_copy(
                            adr[:], ps[:, NHID + 8:HID16])
                        nc.sync.dma_start(
                            out=t0ad[g * 128:(g + 1) * 128, 0:8],
                            in_=adr[:])
                eng = nc.scalar if gg % 2 else nc.sync
                g0 = 4 * gg * 128
                if g0 + 512 <= SPLIT:
                    eng.dma_start(
                        out=t0lo[g0:g0 + 512, :]
                        .rearrange("(g p) w -> p g w", p=128),
                        in_=row[:])
                else:
                    o = g0 - SPLIT
                    eng.dma_start(
                        out=t0hi[o:o + 512, :]
                        .rearrange("(g p) w -> p g w", p=128),
                        in_=row[:])

        # ---------------- shared edge phase -------------------------------
        def edge_phase(layer, tbl_lo, tbl_hi, tblad, fdim, trow, rw,
                       post_fn, fin):
            o_il, o_ih = 0, NL * 8
            o_al, o_ah = NL * 16, NL * 16 + NH * 8
            o_dr = NL * 16 + NH * 16
            with ExitStack() as pb:
                ip = pb.enter_context(
                    tc.tile_pool(name=f"ix{layer}", bufs=4))
                gp = pb.enter_context(
                    tc.tile_pool(name=f"gg{layer}", bufs=4))
                apl = pb.enter_context(
                    tc.tile_pool(name=f"ga{layer}", bufs=3))
                rp2 = pb.enter_context(
                    tc.tile_pool(name=f"rh{layer}", bufs=3))
                pp2 = pb.enter_context(
                    tc.tile_pool(name=f"ps{layer}", bufs=2, space="PSUM"))
                op = pb.enter_context(
                    tc.tile_pool(name=f"po{layer}", bufs=3))
                for t in range(lt):
                    ep = ip.tile([128, EPW], I16, tag="ep")
                    nc.sync.dma_start(out=ep[:], in_=epacki[t])
                    il = ep[:, o_il:o_il + NL * 8]
                    ih = ep[:, o_ih:o_ih + NH * 8]
                    al = ep[:, o_al:o_al + NL * 8]
                    ah = ep[:, o_ah:o_ah + NH * 8]
                    dr = ep[:, o_dr:o_dr + CH].bitcast(F16)

                    G = gp.tile([128, CH, trow], F16, tag="G")
                    nc.gpsimd.dma_gather(G[:, 0:NL, :], tbl_lo[:], il,
                                         NLI, reg_l, trow)
                    nc.gpsimd.dma_gather(G[:, NL:CH, :], tbl_hi[:],
                                         ih, NHI, reg_h, trow)
                    A = apl.tile([128, CH, 128], F16, tag="A")
                    nc.gpsimd.dma_gather(A[:, 0:NL, :], tblad[:], al,
                                         NLI, reg_l, 128)
                    nc.gpsimd.dma_gather(A[:, NL:CH, :], tblad[:], ah,
                                         NHI, reg_h, 128)

                    inc = rp2.tile([128, CH, 128], F16, tag="inc")
                    nc.vector.tensor_tensor(
                        out=inc[:],
                        in0=dr.unsqueeze(-1).broadcast_to([128, CH, 128]),
                        in1=colios[:].unsqueeze(1)
                        .broadcast_to([128, CH, 128]),
                        op=mybir.AluOpType.is_equal)
                    EX = rp2.tile([128, CH, 8], F16, tag="EX")
                    nc.vector.tensor_tensor(
                        out=EX[:], in0=G[:, :, fdim:fdim + 8],
                        in1=A[:, :, 0:8], op=mybir.AluOpType.add)
                    nc.scalar.activation(
                        EX[:], EX[:], mybir.ActivationFunctionType.Prelu,
                        alpha=SLOPE)
                    nc.scalar.activation(
                        EX[:], EX[:], mybir.ActivationFunctionType.Exp)

                    R = rp2.tile([128, CH, fdim], F16, tag="R")
                    H = HEADS
                    D = fdim // H
                    nc.vector.tensor_tensor(
                        out=R[:, :, 0:fdim]
                        .rearrange("p c (d h) -> p c d h", h=H),
                        in0=G[:, :, 0:fdim]
                        .rearrange("p c (d h) -> p c d h", h=H),
                        in1=EX[:].unsqueeze(2).broadcast_to([128, CH, D, H]),
                        op=mybir.AluOpType.mult)

                    P1 = pp2.tile([128, fdim], F32, tag="P1")
                    P2 = pp2.tile([128, 8], F32, tag="P2")
                    for ch in range(CH):
                        nc.tensor.matmul(P1[:], inc[:, ch, :],
                                         R[:, ch, 0:fdim],
                                         start=(ch == 0),
                                         stop=(ch == CH - 1))
                    for ch in range(CH):
                        nc.tensor.matmul(P2[:], inc[:, ch, :],
                                         EX[:, ch, :],
                                         start=(ch == 0),
                                         stop=(ch == CH - 1))
                    post_fn(t, P1, P2, op, pp2, fin)

        # ---- L0 post: softmax-div, +b0, ELU, transpose, store ------------
        def post0(t, P1, P2, op, pp2, fin):
            rows = 128 if t < lt - 1 else LAST_ROWS
            r8 = op.tile([128, 8], F32, tag="r8")
            nc.vector.tensor_scalar_add(r8[:], P2[:], 1e-16)
            nc.vector.reciprocal(r8[:], r8[:])
            z = op.tile([128, NHID], F32, tag="z")
            nc.vector.tensor_tensor(
                out=z[:].rearrange("p (d h) -> p d h", h=HEADS),
                in0=P1[:].rearrange("p (d h) -> p d h", h=HEADS),
                in1=r8[:].unsqueeze(1).broadcast_to([128, 16, HEADS]),
                op=mybir.AluOpType.mult)
            nc.vector.tensor_tensor(out=z[:], in0=z[:], in1=b0s[:],
                                    op=mybir.AluOpType.add)
            zr = op.tile([128, NHID], F32, tag="zr")
            nc.scalar.activation(zr[:], z[:],
                                 mybir.ActivationFunctionType.Relu)
            zm = op.tile([128, NHID], F32, tag="zm")
            nc.vector.tensor_tensor(out=zm[:], in0=z[:], in1=zr[:],
                                    op=mybir.AluOpType.subtract)
            nc.scalar.activation(zm[:], zm[:],
                                 mybir.ActivationFunctionType.Exp)
            h1 = op.tile([128, NHID], F16, tag="h1")
            nc.vector.scalar_tensor_tensor(
                h1[:], zm[:], -1.0, zr[:],
                op0=mybir.AluOpType.add, op1=mybir.AluOpType.add)
            pst = pp2.tile([128, 128], F16, tag="pst")
            nc.tensor.transpose(pst[:], h1[:], idents[:])
            hT = op.tile([128, 128], F8, tag="hT")
            nc.scalar.copy(hT[:], pst[:])
            if t < 25:
                nc.sync.dma_start(
                    out=aginA[:, t * 128:t * 128 + rows], in_=hT[:, 0:rows])
            else:
                o = (t - 25) * 128
                nc.sync.dma_start(
                    out=aginB[:, o:o + rows], in_=hT[:, 0:rows])

        if "B" in phases:
            edge_phase(0, t0lo, t0hi, t0ad, NHID, T0W, 8 + NHID, post0, None)

        # ---------------- phase C: AllGather + rotation -------------------
        sregs = None
        if "C" in phases:
            nc.gpsimd.collective_compute(
                "AllGather", mybir.AluOpType.bypass,
                replica_groups=[list(range(NCORES))],
                ins=[aginA[:]], outs=[agoutA[:]])
            nc.gpsimd.collective_compute(
                "AllGather", mybir.AluOpType.bypass,
                replica_groups=[list(range(NCORES))],
                ins=[aginB[:]], outs=[agoutB[:]])
            pid = nc.partition_id(engines=[mybir.EngineType.SP])
            sregs = [nc.sync.snap(((j + pid) % NCORES) * 128)
                     for j in range(NCORES)]

        # ---------------- phase D: layer-1 tables -------------------------
        with ExitStack() as pd:
            if "D" not in phases:
                pd.enter_context(ExitStack())  # keep structure
            ngt = min(gt, (N + 127) // 128)

            def in_a(g):
                r0, r1 = g * 128, min(g * 128 + 128, N)
                return all((r % NLOC) < 3200 for r in range(r0, r1))

            def spans_of(lst):
                runs = []
                for g in lst:
                    if runs and g == runs[-1][1] and g != 196:
                        runs[-1][1] = g + 1
                    else:
                        runs.append([g, g + 1])
                return [(a, min(a + 8, b)) for ra, b in runs
                        for a in range(ra, b, 8)]

            da = [g for g in range(ngt) if in_a(g)]
            db = [g for g in range(ngt) if not in_a(g)]
            spans = spans_of(da) + spans_of(db)
            if "D" not in phases or sregs is None:
                spans = []
            xp1 = pd.enter_context(tc.tile_pool(name="pd_x", bufs=3))
            pp1 = pd.enter_context(
                tc.tile_pool(name="pd_ps", bufs=2, space="PSUM"))
            ppA = pd.enter_context(
                tc.tile_pool(name="pd_pa", bufs=2, space="PSUM"))
            rp1 = pd.enter_context(tc.tile_pool(name="pd_row", bufs=4))
            for si, (sa, sb) in enumerate(spans):
                hx8 = xp1.tile([128, 8, 128], F8, tag="hx8")
                hx = xp1.tile([128, 8, 128], F16, tag="hx")
                r0, r1 = sa * 128, min(sb * 128, N)
                hxf = hx8[:].rearrange("p g j -> p (g j)")
                w0 = 0
                r = r0
                while r < r1:
                    j = r // NLOC
                    c = r - j * NLOC
                    lim = j * NLOC + (HCOL if c < HCOL else NLOC)
                    seg = min(r1, lim) - r
                    if c < HCOL:
                        srcap = agoutA[bass.ds(sregs[j % NCORES], 128),
                                       c:c + seg]
                    else:
                        srcap = agoutB[bass.ds(sregs[j % NCORES], 128),
                                       c - HCOL:c - HCOL + seg]
                    nc.sync.dma_start(out=hxf[:, w0:w0 + seg], in_=srcap)
                    w0 += seg
                    r += seg
                nc.scalar.copy(hx[:, 0:4, :], hx8[:, 0:4, :])
                nc.vector.tensor_copy(hx[:, 4:8, :], hx8[:, 4:8, :])
                for ga in range(sa, sb, 2):
                    gb = min(ga + 2, sb)
                    nsub = gb - ga
                    row = rp1.tile([128, 2, T1W], F16, tag="row")
                    psH = pp1.tile([128, 2, 512], F32, tag="psH")
                    psA = ppA.tile([128, 2, 512], F32, tag="psA")
                    for g2 in range(nsub):
                        nc.tensor.matmul(psH[:, g2, :],
                                         hx[:, ga - sa + g2, :], W1s[:],
                                         start=True, stop=True)
                        nc.tensor.matmul(psA[:, g2, 0:16],
                                         hx[:, ga - sa + g2, :], W1as[:],
                                         start=True, stop=True)
                    nc.scalar.copy(row[:, 0:nsub, 0:176],
                                   psH[:, 0:nsub, 0:176])
                    nc.vector.tensor_copy(row[:, 0:nsub, 176:512],
                                          psH[:, 0:nsub, 176:512])
                    nc.vector.tensor_copy(row[:, 0:nsub, 512:528],
                                          psA[:, 0:nsub, 0:16])
                    eng = nc.scalar if ga % 4 else nc.sync
                    g0 = ga * 128
                    if nsub == 2 and g0 + 256 <= SPLIT:
                        eng.dma_start(
                            out=t1lo[g0:g0 + 256, 0:528]
                            .rearrange("(g p) w -> p g w", p=128),
                            in_=row[:, :, 0:528])
                    elif nsub == 2:
                        o = g0 - SPLIT
                        eng.dma_start(
                            out=t1hi[o:o + 256, 0:528]
                            .rearrange("(g p) w -> p g w", p=128),
                            in_=row[:, :, 0:528])
                    elif g0 + 128 <= SPLIT:
                        eng.dma_start(out=t1lo[g0:g0 + 128, 0:528],
                                      in_=row[:, 0, 0:528])
                    else:
                        o = g0 - SPLIT
                        eng.dma_start(out=t1hi[o:o + 128, 0:528],
                                      in_=row[:, 0, 0:528])
                    for g2 in range(nsub):
                        g = ga + g2
                        if g < lt:
                            adr = rp1.tile([128, 8], F16, tag="adr")
                            nc.vector.tensor_copy(adr[:],
                                                  psA[:, g2, 8:16])
                            nc.sync.dma_start(
                                out=t1ad[g * 128:(g + 1) * 128, 0:8],
                                in_=adr[:])

        # ---------------- phase E: layer-1 edges + epilogue ---------------
        def post1(t, P1, P2, op, pp2, fin):
            zbig, nmxb, seb = fin
            r8 = op.tile([128, 8], F32, tag="r8")
            nc.vector.tensor_scalar_add(r8[:], P2[:], 1e-16)
            nc.vector.reciprocal(r8[:], r8[:])
            nc.vector.tensor_scalar_mul(r8[:], r8[:], 1.0 / HEADS)
            zw = op.tile([128, 512], F32, tag="zw")
            nc.vector.tensor_tensor(
                out=zw[:].rearrange("p (d h) -> p d h", h=HEADS),
                in0=P1[:].rearrange("p (d h) -> p d h", h=HEADS),
                in1=r8[:].unsqueeze(1).broadcast_to([128, 64, HEADS]),
                op=mybir.AluOpType.mult)
            z = zbig[:, t * NCLASS:(t + 1) * NCLASS]
            nc.vector.reduce_sum(
                z, zw[:].rearrange("p (d h) -> p d h", h=HEADS),
                axis=mybir.AxisListType.X)
            nc.vector.tensor_tensor(out=z, in0=z, in1=b1s[:],
                                    op=mybir.AluOpType.add)
            nmx = nmxb[:, t:t + 1]
            nc.vector.reduce_max(nmx, z, axis=mybir.AxisListType.X,
                                 negate=True)
            ez = op.tile([128, NCLASS], F32, tag="ez")
            nc.scalar.activation(ez[:], z,
                                 mybir.ActivationFunctionType.Exp,
                                 bias=nmx, accum_out=seb[:, t:t + 1])

        if "E" in phases:
            fpool = stk.enter_context(tc.tile_pool(name="fin", bufs=1))
            zbig = fpool.tile([128, lt * NCLASS], F32)
            nmxb = fpool.tile([128, lt], F32)
            seb = fpool.tile([128, lt], F32)
            edge_phase(1, t1lo, t1hi, t1ad, 512, T1W, 520, post1,
                       (zbig, nmxb, seb))
            # batched log-softmax tail: one Ln + two broadcast ops + 2 DMAs
            nc.scalar.activation(seb[:], seb[:],
                                 mybir.ActivationFunctionType.Ln)
            nc.vector.tensor_tensor(
                out=zbig[:].rearrange("p (t c) -> p t c", c=NCLASS),
                in0=zbig[:].rearrange("p (t c) -> p t c", c=NCLASS),
                in1=nmxb[:].unsqueeze(-1).broadcast_to([128, lt, NCLASS]),
                op=mybir.AluOpType.add)
            nc.vector.tensor_tensor(
                out=zbig[:].rearrange("p (t c) -> p t c", c=NCLASS),
                in0=zbig[:].rearrange("p (t c) -> p t c", c=NCLASS),
                in1=seb[:].unsqueeze(-1).broadcast_to([128, lt, NCLASS]),
                op=mybir.AluOpType.subtract)
            nfull = (lt - 1) * 128
            rlast = LAST_ROWS if lt == LT else 128
            nc.sync.dma_start(
                out=out[0:nfull, :].rearrange("(t p) c -> p t c", p=128),
                in_=zbig[:].rearrange("p (t c) -> p t c", c=NCLASS)
                [:, 0:lt - 1, :])
            nc.sync.dma_start(
                out=out[nfull:nfull + rlast, :],
                in_=zbig[0:rlast, (lt - 1) * NCLASS:lt * NCLASS])

    nc.compile()
    return nc


# --------------------------------------------------------------------------
# entry point
# --------------------------------------------------------------------------

def kernel(**inputs) -> np.ndarray:
    NLk, NHk, in_maps = _prep_inputs(**inputs)
    key = (NLk, NHk)
    if key not in _cache:
        _cache[key] = build(NLk, NHk)
    nc = _cache[key]
    res = run_bass_kernel_spmd(nc, in_maps, list(range(NCORES)))
    return np.concatenate([res.results[c]["out"] for c in range(NCORES)], 0)

